# revision 1
# baseline (speedup 1.0000x reference)
"""Cross-attention fusion block on 8 trn2 NeuronCores.

Sharding: data-parallel over the query sequence (S=4096 -> 512 rows/core).
K/V projections are computed redundantly on every core (cheap vs attention).
Everything runs in channel-major ("transposed") layout [C, S] so that no
on-chip transposes are needed anywhere:
  inputs  lidar/image [1,C,H,W] -> [C, S]   (natural memory layout)
  output  [C, S] -> [1, C, H, W]            (natural memory layout)

Per-core pipeline (q = 512 query rows of this core), fp32r matmuls:
  qT = Wq^T @ xT (+bq)            [256, 512]
  kT = Wk^T @ y                   [256, 4096]   (bk dropped: softmax-invariant)
  v2 = y^T @ Wv, stored per-head as [V_h | 1]   (bv folded in after softmax)
  per 4-head group, per 128-row key chunk (ST tiles of 2 heads, row-packed
  4-way on the PE):
    ST[k,q]   = K_h chunk @ Q_h^T
    AT        = exp(ST / sqrt(32))              (ACT, no max subtraction)
    AVCS_h   += [V_h | 1]^T-chunk @ AT          (rows 0-31 attn@v, row 32
                                                 softmax denominator; one
                                                 accumulation chain per bank)
  attn_h = AV_h * (1/CS_h) + bv    (stage copy + DMA partition shifts; CS
                                    rows bounce through DRAM for broadcast)
  o = Wo^T @ attn (+bo); r = qT + o; z = LN(r)  (stats via ones-matmuls)
  h1 = relu(W1^T @ z + bf1); h2 = W2^T @ h1 + bf2; out = LN(z + h2)
"""

import sys

for _p in ("/opt/trn_rl_repo", "/opt/pypackages"):
    if _p not in sys.path:
        sys.path.append(_p)

import numpy as np

import concourse.bass as bass
import concourse.bacc as bacc
import concourse.tile as tile
from concourse import mybir
from concourse.bass_utils import run_bass_kernel_spmd

F32 = mybir.dt.float32
F32R = mybir.dt.float32r
AFT = mybir.ActivationFunctionType
ALU = mybir.AluOpType

P = 128           # SBUF partitions
C = 256           # channels
S = 4096          # sequence (64*64)
NCORES = 8
SH = S // NCORES  # 512 query rows per core
NH = 8            # heads
HD = 32           # head dim
HD1 = HD + 1      # V block plus the ones column for the colsum
F = 4 * C         # FFN hidden = 1024
NKC = C // P      # 2 channel chunks
NFC = F // P      # 8 ffn chunks
NSC = S // P      # 32 key chunks
EPS = 1e-5
INV_SQRT_HD = 1.0 / float(np.sqrt(HD))
INV_C = 1.0 / C


def build_bass():
    nc = bacc.Bacc()

    xT = nc.declare_dram_parameter("xT", [C, SH], F32R, isOutput=False)
    y = nc.declare_dram_parameter("y", [C, S], F32R, isOutput=False)
    w4 = nc.declare_dram_parameter("w4", [4, C, C], F32R, isOutput=False)
    w1 = nc.declare_dram_parameter("w1", [C, F], F32R, isOutput=False)
    w2 = nc.declare_dram_parameter("w2", [F, C], F32R, isOutput=False)
    ones32 = nc.declare_dram_parameter("ones32", [P, 1], F32R, isOutput=False)
    emat = nc.declare_dram_parameter("emat", [4, P], F32, isOutput=False)
    bpack = nc.declare_dram_parameter("bpack", [12, C], F32, isOutput=False)
    out = nc.declare_dram_parameter("out", [C, SH], F32, isOutput=True)

    with tile.TileContext(nc) as tc:
        _emit(tc, xT, y, w4, w1, w2, ones32, emat, bpack, out)
    if not nc.is_finalized():
        nc.finalize()
    return nc


def _emit(tc, xT, y, w4, w1, w2, ones32, emat, bpack, out):
    nc = tc.nc

    import contextlib
    stack = contextlib.ExitStack()
    with stack:
        consts = stack.enter_context(tc.tile_pool(name="consts", bufs=1))
        big = stack.enter_context(tc.tile_pool(name="big", bufs=1))

        # ---------------- constants / weights into SBUF ----------------
        y_sb = big.tile([P, NKC, S], F32R)         # y[ch, s]; ch = kc*128 + p
        y_r = y.rearrange("(kc p) s -> p kc s", p=P)
        HS = S // 2
        for kc in range(NKC):
            for sh2 in range(2):
                eng = [nc.sync, nc.gpsimd, nc.sync, nc.gpsimd][kc * 2 + sh2]
                eng.dma_start(
                    out=y_sb[:, kc, sh2 * HS:(sh2 + 1) * HS],
                    in_=y_r[:, kc, sh2 * HS:(sh2 + 1) * HS])
        xT_sb = big.tile([P, NKC, SH], F32R)
        nc.sync.dma_start(out=xT_sb, in_=xT.rearrange("(kc p) s -> p kc s", p=P))

        w4_sb = consts.tile([P, 4, NKC, C], F32R)
        nc.gpsimd.dma_start(
            out=w4_sb, in_=w4.rearrange("w (kc p) m -> p w kc m", p=P))
        wq_sb, wk_sb, wv_sb, wo_sb = (w4_sb[:, i] for i in range(4))
        w1_sb = consts.tile([P, NKC, F], F32R)
        nc.gpsimd.dma_start(out=w1_sb, in_=w1.rearrange("(kc p) m -> p kc m", p=P))
        w2_sb = consts.tile([P, NFC, C], F32R)
        nc.gpsimd.dma_start(out=w2_sb, in_=w2.rearrange("(kc p) m -> p kc m", p=P))

        bp_sb = consts.tile([P, 12, NKC], F32)
        nc.gpsimd.dma_start(
            out=bp_sb, in_=bpack.rearrange("n (kc p) -> p n kc", p=P))
        bq_sb, bv_sb, bo_sb, bf2_sb = (bp_sb[:, i] for i in range(4))
        g1_sb, b1_sb, g2_sb, b2_sb = (bp_sb[:, i] for i in range(4, 8))

        ones1r = consts.tile([P, 1], F32R)      # LN-stats lhsT (f32r ones)
        nc.gpsimd.dma_start(out=ones1r, in_=ones32[:])
        emat_sb = consts.tile([4, P], F32)      # head-broadcast matrix
        nc.gpsimd.dma_start(out=emat_sb, in_=emat[:])
        ones_rep = consts.tile([1, P], F32)     # K=1 row-replication lhsT
        nc.vector.memset(ones_rep, 1.0)
        eps_sb = consts.tile([P, 1], F32)
        nc.vector.memset(eps_sb, EPS)

        # persistent activations
        qT_sb = big.tile([P, NKC, SH], F32R)    # q^T  (with bq)
        kT_sb = big.tile([P, NKC, S], F32R)     # k^T  (no bk; softmax-invariant)
        v2_sb = big.tile([P, NSC, NH, HD1], F32R)  # per head [V_h | 1]
        attn_sb = big.tile([P, NKC, SH], F32R)  # (attn @ v)^T + bv
        z_sb = big.tile([P, NKC, SH], F32R)     # LN1 output
        h1_sb = big.tile([P, NFC, SH], F32R)    # relu(ffn1)
        out_sb = big.tile([P, NKC, SH], F32)    # final
        r_sb = big.tile([P, NKC, SH], F32R)     # residual sums (LN inputs)

        # ones column of v2 (memset cannot write f32r -> DMA broadcast)
        ones_ap = ones32[:]
        ones_col = bass.AP(
            tensor=ones_ap.tensor, offset=ones_ap.offset,
            ap=[ones_ap.ap[0], [0, NSC * NH]])
        nc.sync.dma_start(out=v2_sb[:, :, :, HD:HD1].squeeze(),
                          in_=ones_col)

        # ---------------- preamble: qT, kT, v projections ----------------
        with tc.tile_pool(name="pre_k", bufs=3, space="PSUM") as pre_k, \
             tc.tile_pool(name="pre_v", bufs=3, space="PSUM") as pre_v:
            # q^T[c',q] = sum_ch Wq[ch,c'] xT[ch,q]
            for mc in range(NKC):
                ps = pre_k.tile([P, SH], F32, tag="ps_k")
                for kc in range(NKC):
                    nc.tensor.matmul(
                        ps, wq_sb[:, kc, mc * P:(mc + 1) * P],
                        xT_sb[:, kc, :],
                        start=(kc == 0), stop=(kc == NKC - 1))
                nc.scalar.activation(out=qT_sb[:, mc, :], in_=ps,
                                     func=AFT.Identity,
                                     bias=bq_sb[:, mc:mc + 1])
            # k^T[c',s] = sum_ch Wk[ch,c'] y[ch,s]   in 512-col blocks
            for sb in range(S // 512):
                for mc in range(NKC):
                    ps = pre_k.tile([P, 512], F32, tag="ps_k")
                    for kc in range(NKC):
                        nc.tensor.matmul(
                            ps, wk_sb[:, kc, mc * P:(mc + 1) * P],
                            y_sb[:, kc, sb * 512:(sb + 1) * 512],
                            start=(kc == 0), stop=(kc == NKC - 1))
                    nc.vector.tensor_copy(
                        kT_sb[:, mc, sb * 512:(sb + 1) * 512], ps)
            # v[s,c'] = sum_ch y[ch,s] Wv[ch,c']    per 128-row s chunk
            for ck in range(NSC):
                ps = pre_v.tile([P, C], F32, tag="ps_v")
                for kc in range(NKC):
                    nc.tensor.matmul(
                        ps, y_sb[:, kc, ck * P:(ck + 1) * P],
                        wv_sb[:, kc, :],
                        start=(kc == 0), stop=(kc == NKC - 1))
                # scatter the 8 per-head blocks into the [V_h | 1] layout
                dst = v2_sb[:, ck, :, 0:HD]
                nc.vector.tensor_copy(dst, ps.rearrange("p (h d) -> p h d",
                                                        d=HD))

        # ---------------- attention ----------------
        with tc.tile_pool(name="st", bufs=2, space="PSUM") as st_pool, \
             tc.tile_pool(name="avcs", bufs=4, space="PSUM") as avcs_pool, \
             tc.tile_pool(name="at", bufs=4) as at_pool, \
             tc.tile_pool(name="nrm", bufs=1) as nrm_pool:
            for grp in range(2):
                avcs = [avcs_pool.tile([HD1, SH], F32, tag="avcs",
                                       name=f"avcs_g{grp}_{j}")
                        for j in range(4)]
                for ck in range(NSC):
                    for pair in range(2):
                        st = st_pool.tile([P, 2, SH], F32, tag="st")
                        for j in range(2):
                            h = 4 * grp + 2 * pair + j
                            po = HD * (h % 4)
                            nc.tensor.matmul(
                                st[:, j, :],
                                kT_sb[po:po + HD, grp, ck * P:(ck + 1) * P],
                                qT_sb[po:po + HD, grp, :],
                                start=True, stop=True,
                                tile_position=(po, 0))
                        at = at_pool.tile([P, 2, SH], F32R, tag="at")
                        nc.scalar.activation(out=at, in_=st, func=AFT.Exp,
                                             scale=INV_SQRT_HD)
                        for j in range(2):
                            h = 4 * grp + 2 * pair + j
                            nc.tensor.matmul(
                                avcs[2 * pair + j],
                                v2_sb[:, ck, h, :],
                                at[:, j, :],
                                start=(ck == 0), stop=(ck == NSC - 1))
                # normalize: attn_h = av_h / cs_h (+ bv later, whole group)
                av_all = nrm_pool.tile([P, SH], F32, tag="av_all")
                stage = nrm_pool.tile([HD1, 4, SH], F32, tag="stage")
                for j in range(4):
                    nc.vector.tensor_copy(stage[:, j, :], avcs[j])
                    eng = [nc.sync, nc.gpsimd, nc.sync, nc.gpsimd][j]
                    eng.dma_start(out=av_all[HD * j:HD * (j + 1), :],
                                  in_=stage[0:HD, j, :])
                cs4 = nrm_pool.tile([4, SH], F32, tag="cs4")
                nc.sync.dma_start(out=cs4, in_=stage[HD:HD1, :, :])
                rec4 = nrm_pool.tile([4, SH], F32, tag="rec4")
                scr4 = nrm_pool.tile([4, SH], F32, tag="scr4")
                nc.vector.reciprocal_approx_accurate(out=rec4, in_=cs4,
                                                     scratch=scr4)
                rec_all = avcs_pool.tile([P, SH], F32, tag="avcs",
                                         name=f"rec_all_{grp}")
                nc.tensor.matmul(rec_all, emat_sb, rec4, start=True, stop=True)
                tmp = nrm_pool.tile([P, SH], F32, tag="tmp")
                nc.vector.tensor_mul(tmp, av_all, rec_all)
                nc.vector.tensor_scalar_add(out=attn_sb[:, grp, :], in0=tmp,
                                            scalar1=bv_sb[:, grp:grp + 1])

        # ---------------- tail: out-proj, LN1, FFN, LN2 ----------------
        with tc.tile_pool(name="mm", bufs=3, space="PSUM") as mm_pool, \
             tc.tile_pool(name="stat", bufs=1, space="PSUM") as stat_pool, \
             tc.tile_pool(name="rep", bufs=1, space="PSUM") as rep_pool, \
             tc.tile_pool(name="tl", bufs=2) as tl_pool, \
             tc.tile_pool(name="tr", bufs=1) as tr_pool:

            def layer_norm(x3, gamma, beta, out3):
                """out3 = LN(x3) over the channel axis (2 chunks of 128)."""
                mu_ps = stat_pool.tile([1, SH], F32, tag="mu")
                e2_ps = stat_pool.tile([1, SH], F32, tag="e2")
                for kc in range(NKC):
                    nc.tensor.matmul(mu_ps, ones1r, x3[:, kc, :],
                                     start=(kc == 0), stop=(kc == NKC - 1))
                for kc in range(NKC):
                    sq = tl_pool.tile([P, SH], F32R, tag="sq")
                    if kc == 0:
                        nc.scalar.activation(out=sq, in_=x3[:, kc, :],
                                             func=AFT.Square)
                    else:
                        nc.vector.tensor_mul(sq, x3[:, kc, :], x3[:, kc, :])
                    nc.tensor.matmul(e2_ps, ones1r, sq,
                                     start=(kc == 0), stop=(kc == NKC - 1))
                mu_row = tr_pool.tile([1, SH], F32, tag="mu_row")
                nc.vector.tensor_scalar_mul(out=mu_row, in0=mu_ps,
                                            scalar1=INV_C)
                mu2_row = tr_pool.tile([1, SH], F32, tag="mu2_row")
                nc.vector.tensor_mul(mu2_row, mu_row, mu_row)
                var_row = tr_pool.tile([1, SH], F32, tag="var_row")
                # var = E[x^2] - mu^2 = e2/C - mu^2
                nc.vector.scalar_tensor_tensor(
                    out=var_row, in0=e2_ps, scalar=INV_C, in1=mu2_row,
                    op0=ALU.mult, op1=ALU.subtract)
                std_row = tr_pool.tile([1, SH], F32, tag="std_row")
                nc.scalar.activation(out=std_row, in_=var_row, func=AFT.Sqrt,
                                     bias=eps_sb[:1, :])
                rstd_row = tr_pool.tile([1, SH], F32, tag="rstd_row")
                scr_row = tr_pool.tile([1, SH], F32, tag="mu2_row")
                nc.vector.reciprocal_approx_accurate(out=rstd_row, in_=std_row,
                                                     scratch=scr_row)
                mu_rep = rep_pool.tile([P, SH], F32, tag="mu_rep")
                nc.tensor.matmul(mu_rep, ones_rep, mu_row,
                                 start=True, stop=True)
                rstd_rep = rep_pool.tile([P, SH], F32, tag="rstd_rep")
                nc.tensor.matmul(rstd_rep, ones_rep, rstd_row,
                                 start=True, stop=True)
                for kc in range(NKC):
                    t = tl_pool.tile([P, SH], F32, tag="t")
                    nc.vector.tensor_sub(t, x3[:, kc, :], mu_rep)
                    t2 = tl_pool.tile([P, SH], F32, tag="t2")
                    nc.vector.tensor_mul(t2, t, rstd_rep)
                    nc.vector.tensor_scalar(
                        out=out3[:, kc, :], in0=t2,
                        scalar1=gamma[:, kc:kc + 1], scalar2=beta[:, kc:kc + 1],
                        op0=ALU.mult, op1=ALU.add)

            # out-projection + residual (r = qT + Wo^T attn + bo)
            for mc in range(NKC):
                ps = mm_pool.tile([P, SH], F32, tag="mm")
                for kc in range(NKC):
                    nc.tensor.matmul(
                        ps, wo_sb[:, kc, mc * P:(mc + 1) * P],
                        attn_sb[:, kc, :],
                        start=(kc == 0), stop=(kc == NKC - 1))
                o_t = tl_pool.tile([P, SH], F32, tag="o_t")
                nc.vector.tensor_scalar_add(out=o_t, in0=ps,
                                            scalar1=bo_sb[:, mc:mc + 1])
                nc.vector.tensor_add(r_sb[:, mc, :], qT_sb[:, mc, :], o_t)

            layer_norm(r_sb, g1_sb, b1_sb, z_sb)

            # FFN1 + relu
            for mf in range(NFC):
                ps = mm_pool.tile([P, SH], F32, tag="mm")
                for kc in range(NKC):
                    nc.tensor.matmul(
                        ps, w1_sb[:, kc, mf * P:(mf + 1) * P],
                        z_sb[:, kc, :],
                        start=(kc == 0), stop=(kc == NKC - 1))
                if mf % 2 == 0:
                    nc.scalar.activation(
                        out=h1_sb[:, mf, :], in_=ps, func=AFT.Relu,
                        bias=bp_sb[:, 8 + mf // 2, mf % 2:mf % 2 + 1])
                else:
                    nc.vector.tensor_scalar(
                        out=h1_sb[:, mf, :], in0=ps,
                        scalar1=bp_sb[:, 8 + mf // 2, mf % 2:mf % 2 + 1],
                        scalar2=0.0,
                        op0=ALU.add, op1=ALU.max)
            # FFN2 + bias + residual
            for mc in range(NKC):
                ps = mm_pool.tile([P, SH], F32, tag="mm")
                for kf in range(NFC):
                    nc.tensor.matmul(
                        ps, w2_sb[:, kf, mc * P:(mc + 1) * P],
                        h1_sb[:, kf, :],
                        start=(kf == 0), stop=(kf == NFC - 1))
                f2 = tl_pool.tile([P, SH], F32, tag="f2")
                nc.vector.tensor_scalar_add(out=f2, in0=ps,
                                            scalar1=bf2_sb[:, mc:mc + 1])
                nc.vector.tensor_add(r_sb[:, mc, :], z_sb[:, mc, :], f2)

            layer_norm(r_sb, g2_sb, b2_sb, out_sb)

            out_r = out.rearrange("(kc p) s -> p kc s", p=P)
            nc.sync.dma_start(out=out_r[:, 0, :], in_=out_sb[:, 0, :])
            nc.gpsimd.dma_start(out=out_r[:, 1, :], in_=out_sb[:, 1, :])


_NC_CACHE = None


def _get_nc():
    global _NC_CACHE
    if _NC_CACHE is None:
        _NC_CACHE = build_bass()
    return _NC_CACHE


def make_in_maps(lidar_features, image_features, Wq, bq, Wk, bk, Wv, bv,
                 Wo, bo, g1, b1, W1, bf1, W2, bf2, g2, b2):
    xT_full = np.ascontiguousarray(
        np.asarray(lidar_features, np.float32).reshape(C, S))
    y_full = np.ascontiguousarray(
        np.asarray(image_features, np.float32).reshape(C, S))
    w4 = np.ascontiguousarray(np.stack([
        np.asarray(Wq, np.float32), np.asarray(Wk, np.float32),
        np.asarray(Wv, np.float32), np.asarray(Wo, np.float32)]))
    bpack = np.ascontiguousarray(np.concatenate([
        np.asarray(bq, np.float32)[None], np.asarray(bv, np.float32)[None],
        np.asarray(bo, np.float32)[None], np.asarray(bf2, np.float32)[None],
        np.asarray(g1, np.float32)[None], np.asarray(b1, np.float32)[None],
        np.asarray(g2, np.float32)[None], np.asarray(b2, np.float32)[None],
        np.asarray(bf1, np.float32).reshape(4, C)]))
    em = np.zeros((4, P), np.float32)
    for j in range(4):
        em[j, HD * j:HD * (j + 1)] = 1.0
    common = {
        "y": y_full,
        "emat": em,
        "w4": w4,
        "w1": np.ascontiguousarray(np.asarray(W1, np.float32)),
        "w2": np.ascontiguousarray(np.asarray(W2, np.float32)),
        "ones32": np.ones((P, 1), np.float32),
        "bpack": bpack,
    }
    in_maps = []
    for c in range(NCORES):
        m = dict(common)
        m["xT"] = np.ascontiguousarray(xT_full[:, c * SH:(c + 1) * SH])
        in_maps.append(m)
    return in_maps


def kernel(lidar_features, image_features, Wq, bq, Wk, bk, Wv, bv, Wo, bo,
           g1, b1, W1, bf1, W2, bf2, g2, b2, num_heads, **run_kwargs):
    assert int(num_heads) == NH
    nc = _get_nc()
    in_maps = make_in_maps(lidar_features, image_features, Wq, bq, Wk, bk,
                           Wv, bv, Wo, bo, g1, b1, W1, bf1, W2, bf2, g2, b2)
    res = run_bass_kernel_spmd(nc, in_maps, core_ids=list(range(NCORES)),
                               **run_kwargs)
    full = np.concatenate([res.results[c]["out"] for c in range(NCORES)],
                          axis=1)
    kernel.last_results = res
    return full.reshape(1, C, 64, 64).astype(np.float32)


kernel.last_results = None



# revision 7
# speedup vs baseline: 1.2887x; 1.2887x over previous
"""Cross-attention fusion block on 8 trn2 NeuronCores.

Sharding: data-parallel over the query sequence (S=4096 -> 512 rows/core).
K/V projections are computed redundantly on every core. Channel-major
layout [C, S] throughout; no on-chip transposes.

v2 design (vs baseline): fp8 DoubleRow matmuls for the attention phase and
the K/V/Q8 projections, and the softmax exp split across ACT (true exp ->
fp8e5) / DVE / Pool (Schraudolph bit-hack exp via uint8 write + fp8e5
bitcast).  Key layout trick: Wk/Wq columns are permuted+zero-padded on the
host so the projection matmul lands K/Q directly in the [16, 2(half), ...]
partition layout DoubleRow needs (head h in grp g at partition band
32*(h%4), head-dim split 16+16 across the DoubleRow free axis).

Per-core pipeline (q = 512 query rows of this core):
  qT   = Wq^T xT + bq                  [256, 512] fp32r   (residual path)
  q8   = perm(Wq8)^T x8                [bands, 2, 512] fp8e4
  k8   = perm(Wk8)^T y8                [bands, 2, 4096] fp8e4
  v8_h = [y8^T Wv8 + bv | 1]           per head [128, 2, 33] fp8e4
  per (grp g, head j, 256-key chunk d):
    ST[k, q] = k8_h-chunk DR@ q8_h                  (2 DoubleRow matmuls)
    AT       = approx-exp(ST/sqrt(32)) -> fp8e5     (ACT exp | DVE/Pool hack)
    AVCS_h  += v8_h-chunk DR@ AT                    [33, 512] psum
  attn_h = AV_h * (1/CS_h)      (CS rows DMA-gathered, reciprocal, emat
                                 broadcast matmul, per-head psum*rec mul)
  o = Wo^T attn + bo; r = qT + o; z = LN1(r)
  h1 = relu(W1^T z + bf1); h2 = W2^T h1 + bf2; out = LN2(z + h2)
"""

import sys

for _p in ("/opt/trn_rl_repo", "/opt/pypackages"):
    if _p not in sys.path:
        sys.path.append(_p)

import numpy as np
import ml_dtypes

import concourse.bass as bass
import concourse.bacc as bacc
import concourse.tile as tile
from concourse import mybir
from concourse.bass_utils import run_bass_kernel_spmd

F32 = mybir.dt.float32
F32R = mybir.dt.float32r
FP8E4 = mybir.dt.float8e4
FP8E5 = mybir.dt.float8e5
U8 = mybir.dt.uint8
AFT = mybir.ActivationFunctionType
ALU = mybir.AluOpType
DR = mybir.MatmulPerfMode.DoubleRow

P = 128           # SBUF partitions
C = 256           # channels
S = 4096          # sequence (64*64)
NCORES = 8
SH = S // NCORES  # 512 query rows per core
NH = 8            # heads
HD = 32           # head dim
F = 4 * C         # FFN hidden = 1024
NKC = C // P      # 2 channel chunks
NFC = F // P      # 8 ffn chunks
ND = S // 256     # 16 double-row key chunks
NBLK = S // 512   # 8 key blocks for kT production
EPS = 1e-5
INV_SQRT_HD = 1.0 / float(np.sqrt(HD))
INV_C = 1.0 / C
# Schraudolph-style exp for fp8e5(=e5m2) bitcast: i = floor(A*st + B)
HACK_A = float(4.0 * np.log2(np.e)) * INV_SQRT_HD
HACK_B = 60.02


def build_bass():
    nc = bacc.Bacc()

    xT = nc.declare_dram_parameter("xT", [C, SH], F32R, isOutput=False)
    x8 = nc.declare_dram_parameter("x8", [C, SH], FP8E4, isOutput=False)
    y8 = nc.declare_dram_parameter("y8", [C, S], FP8E4, isOutput=False)
    wqo = nc.declare_dram_parameter("wqo", [2, C, C], F32R, isOutput=False)
    wq8p = nc.declare_dram_parameter("wq8p", [2, 2, P, 2, P], FP8E4,
                                     isOutput=False)
    wk8p = nc.declare_dram_parameter("wk8p", [2, 2, P, 2, P], FP8E4,
                                     isOutput=False)
    wv8 = nc.declare_dram_parameter("wv8", [2, P, C], FP8E4, isOutput=False)
    w1 = nc.declare_dram_parameter("w1", [C, F], F32R, isOutput=False)
    w2 = nc.declare_dram_parameter("w2", [F, C], F32R, isOutput=False)
    ones32 = nc.declare_dram_parameter("ones32", [P, 1], F32R, isOutput=False)
    bpack = nc.declare_dram_parameter("bpack", [12, C], F32, isOutput=False)
    bvb = nc.declare_dram_parameter("bvb", [P, C], F32, isOutput=False)
    out = nc.declare_dram_parameter("out", [C, SH], F32, isOutput=True)

    with tile.TileContext(nc) as tc:
        _emit(tc, xT, x8, y8, wqo, wq8p, wk8p, wv8, w1, w2, ones32,
              bpack, bvb, out)
    if not nc.is_finalized():
        nc.finalize()
    return nc


def _emit(tc, xT, x8, y8, wqo, wq8p, wk8p, wv8, w1, w2, ones32,
          bpack, bvb, out):
    nc = tc.nc

    import contextlib
    stack = contextlib.ExitStack()
    with stack:
        consts = stack.enter_context(tc.tile_pool(name="consts", bufs=1))
        big = stack.enter_context(tc.tile_pool(name="big", bufs=1))

        # ---------------- constants / inputs into SBUF ----------------
        y8_sb = big.tile([P, NKC, S], FP8E4)      # y8[ch, s]; ch = kc*128+p
        y8_r = y8.rearrange("(kc p) s -> p kc s", p=P)
        HS = S // 2
        for sh2 in range(2):
            eng = [nc.sync, nc.gpsimd][sh2]
            eng.dma_start(out=y8_sb[:, :, sh2 * HS:(sh2 + 1) * HS],
                          in_=y8_r[:, :, sh2 * HS:(sh2 + 1) * HS])
        xT_sb = big.tile([P, NKC, SH], F32R)
        nc.sync.dma_start(out=xT_sb, in_=xT.rearrange("(kc p) s -> p kc s",
                                                      p=P))
        x8_sb = big.tile([P, NKC, SH], FP8E4)
        nc.gpsimd.dma_start(out=x8_sb, in_=x8.rearrange("(kc p) s -> p kc s",
                                                        p=P))

        wqo_sb = consts.tile([P, 2, NKC, C], F32R)
        nc.sync.dma_start(
            out=wqo_sb, in_=wqo.rearrange("w (kc p) m -> p w kc m", p=P))
        wq_sb, wo_sb = (wqo_sb[:, i] for i in range(2))
        wq8p_sb = consts.tile([P, 2, 2, 2, P], FP8E4)
        nc.gpsimd.dma_start(
            out=wq8p_sb, in_=wq8p.rearrange("g hf p w m -> p g hf w m"))
        wk8p_sb = consts.tile([P, 2, 2, 2, P], FP8E4)
        nc.gpsimd.dma_start(
            out=wk8p_sb, in_=wk8p.rearrange("g hf p w m -> p g hf w m"))
        wv8_sb = consts.tile([P, 2, C], FP8E4)
        nc.gpsimd.dma_start(out=wv8_sb, in_=wv8.rearrange("w p m -> p w m"))
        w1_sb = consts.tile([P, NKC, F], F32R)
        nc.gpsimd.dma_start(out=w1_sb,
                            in_=w1.rearrange("(kc p) m -> p kc m", p=P))
        w2_sb = consts.tile([P, NFC, C], F32R)
        nc.sync.dma_start(out=w2_sb,
                          in_=w2.rearrange("(kc p) m -> p kc m", p=P))

        bp_sb = consts.tile([P, 12, NKC], F32)
        nc.sync.dma_start(out=bp_sb,
                          in_=bpack.rearrange("n (kc p) -> p n kc", p=P))
        bq_sb = bp_sb[:, 0]
        bo_sb, bf2_sb = bp_sb[:, 2], bp_sb[:, 3]
        g1_sb, b1_sb, g2_sb, b2_sb = (bp_sb[:, i] for i in range(4, 8))
        bvb_sb = consts.tile([P, C], F32)
        nc.sync.dma_start(out=bvb_sb, in_=bvb[:])

        ones1r = consts.tile([P, 1], F32R)      # LN-stats lhsT (f32r ones)
        nc.sync.dma_start(out=ones1r, in_=ones32[:])
        ones_rep = consts.tile([1, P], F32)     # K=1 row-replication lhsT
        nc.vector.memset(ones_rep, 1.0)
        ones132 = consts.tile([1, HD], F32R)    # rec band-broadcast lhsT
        nc.vector.memset(ones132, 1.0)
        eps_sb = consts.tile([P, 1], F32)
        nc.vector.memset(eps_sb, EPS)

        # persistent activations
        qT_sb = big.tile([P, NKC, SH], F32R)       # q^T (with bq), residual
        q8_sb = big.tile([P, 2, 2, SH], FP8E4)     # (band, g, half, q)
        k8_sb = big.tile([P, 2, 2, S], FP8E4)      # (band, g, half, s)
        v8_sb = big.tile([P, ND, 2, NH, HD + 1], FP8E4)  # (k, d, i, h, c|1)
        attn_sb = big.tile([P, NKC, SH], F32R)     # (attn@v)/cs + bv
        z_sb = big.tile([P, NKC, SH], F32R)        # LN1 output
        h1_sb = big.tile([P, NFC, SH], F32R)       # relu(ffn1)
        out_sb = big.tile([P, NKC, SH], F32)       # final
        r_sb = big.tile([P, NKC, SH], F32R)        # residual sums (LN inputs)

        # ones column of v8 (CS accumulator rows)
        nc.vector.memset(v8_sb[:, :, :, :, HD:HD + 1], 1.0)

        # ---------------- preamble: projections ----------------
        with tc.tile_pool(name="pre_k", bufs=2, space="PSUM") as pre_k, \
             tc.tile_pool(name="pre_v", bufs=3, space="PSUM") as pre_v:
            # q^T fp32 (residual): q[c',q] = sum_ch Wq[ch,c'] xT[ch,q]
            psq = pre_k.tile([P, 2, SH], F32, tag="ps")
            for mc in range(NKC):
                for kc in range(NKC):
                    nc.tensor.matmul(
                        psq[:, mc, :], wq_sb[:, kc, mc * P:(mc + 1) * P],
                        xT_sb[:, kc, :],
                        start=(kc == 0), stop=(kc == NKC - 1))
            for mc in range(NKC):
                nc.scalar.activation(out=qT_sb[:, mc, :], in_=psq[:, mc, :],
                                     func=AFT.Identity,
                                     bias=bq_sb[:, mc:mc + 1])
            # q8 in split-half band layout, via DoubleRow over channels
            for g in range(2):
                ps8 = pre_k.tile([P, 2, SH], F32, tag="ps")
                for hf in range(2):
                    nc.tensor.matmul(ps8[:, hf, :], wq8p_sb[:, g, hf],
                                     x8_sb, start=True, stop=True,
                                     perf_mode=DR)
                eng = [nc.vector, nc.gpsimd][g]
                eng.tensor_copy(q8_sb[:, g], ps8)
            # k8: per (g, 512-key blk): two DoubleRow matmuls + one convert
            conv_engs = [nc.scalar, nc.vector, nc.gpsimd]
            for g in range(2):
                for blk in range(NBLK):
                    psk = pre_k.tile([P, 2, SH], F32, tag="ps")
                    for hf in range(2):
                        nc.tensor.matmul(
                            psk[:, hf, :], wk8p_sb[:, g, hf],
                            y8_sb[:, :, blk * 512:(blk + 1) * 512],
                            start=True, stop=True, perf_mode=DR)
                    e = conv_engs[(g * NBLK + blk) % 3]
                    if e is nc.scalar:
                        nc.scalar.activation(
                            out=k8_sb[:, g, :, blk * 512:(blk + 1) * 512],
                            in_=psk, func=AFT.Copy)
                    else:
                        e.tensor_copy(
                            k8_sb[:, g, :, blk * 512:(blk + 1) * 512], psk)
            # v8: per 128-key chunk, DoubleRow over channels; +bv fused
            for ck in range(S // P):
                d, i = ck // 2, ck % 2
                psv = pre_v.tile([P, C], F32, tag="psv")
                nc.tensor.matmul(psv, y8_sb[:, :, ck * P:(ck + 1) * P],
                                 wv8_sb, start=True, stop=True, perf_mode=DR)
                e = [nc.vector, nc.gpsimd][ck % 2]
                e.tensor_add(
                    v8_sb[:, d, i, :, 0:HD],
                    psv.rearrange("p (h c) -> p h c", c=HD),
                    bvb_sb.rearrange("p (h c) -> p h c", c=HD))

        # ---------------- attention ----------------
        # weighted round-robin of the exp across ACT / DVE / Pool
        exp_w = [(nc.scalar, 1.20), (nc.vector, 0.96), (nc.gpsimd, 0.72)]
        credits = [0.0, 0.0, 0.0]
        exp_engs = []
        for _ in range(2 * ND * 4):
            for ii in range(3):
                credits[ii] += exp_w[ii][1]
            pick = max(range(3), key=lambda ii: credits[ii])
            credits[pick] -= sum(w for _, w in exp_w)
            exp_engs.append(exp_w[pick][0])

        with tc.tile_pool(name="st", bufs=2, space="PSUM") as st_pool, \
             tc.tile_pool(name="avcs", bufs=1, space="PSUM") as avcs_pool, \
             tc.tile_pool(name="at", bufs=5) as at_pool, \
             tc.tile_pool(name="nrm", bufs=1) as nrm_pool:
            uu = 0
            for g in range(2):
                avcs = avcs_pool.tile([HD + 1, 4, SH], F32, tag="avcs",
                                      name=f"avcs_{g}")
                pend = None
                for d in range(ND):
                    for j in range(4):
                        h = 4 * g + j
                        st = st_pool.tile([P, 2, SH], F32, tag="st")
                        for i in range(2):
                            nc.tensor.matmul(
                                st[:, i, :],
                                k8_sb[32 * j:32 * j + 16, g, :,
                                      256 * d + 128 * i:256 * d + 128 * i + 128],
                                q8_sb[32 * j:32 * j + 16, g],
                                start=True, stop=True, perf_mode=DR,
                                tile_position=(32 * j, 0))
                        at = at_pool.tile([P, 2, SH], FP8E5, tag="at")
                        e = exp_engs[uu]
                        uu += 1
                        if e is nc.scalar:
                            nc.scalar.activation(out=at, in_=st, func=AFT.Exp,
                                                 scale=INV_SQRT_HD)
                        else:
                            e.tensor_scalar(out=at.bitcast(U8), in0=st,
                                            scalar1=HACK_A, scalar2=HACK_B,
                                            op0=ALU.mult, op1=ALU.add)
                        if pend is not None:
                            pd, pj, pat = pend
                            nc.tensor.matmul(
                                avcs[:, pj, :], v8_sb[:, pd, :, 4 * g + pj, :],
                                pat, start=(pd == 0), stop=(pd == ND - 1),
                                perf_mode=DR)
                        pend = (d, j, at)
                pd, pj, pat = pend
                nc.tensor.matmul(avcs[:, pj, :],
                                 v8_sb[:, pd, :, 4 * g + pj, :], pat,
                                 start=(pd == 0), stop=(pd == ND - 1),
                                 perf_mode=DR)
                # normalize: attn_h = av_h / cs_h  (bv already folded into v8)
                rec_row = nrm_pool.tile([1, 4, SH], F32, tag="rec_row",
                                        name=f"rec_row_{g}")
                nc.vector.reciprocal_approx_fast(out=rec_row,
                                                 in_=avcs[HD:HD + 1, :, :])
                rec_all = st_pool.tile([P, 2, SH], F32, tag="st",
                                       name=f"rec_all_{g}")
                rr_r = rec_row.bitcast(F32R)
                for j in range(4):
                    nc.tensor.matmul(rec_all[32 * j:32 * (j + 1), 0, :],
                                     ones132, rr_r[:, j, :],
                                     start=True, stop=True,
                                     tile_position=(0, 32 * j))
                for j in range(4):
                    e = [nc.vector, nc.gpsimd][j % 2]
                    e.tensor_mul(attn_sb[32 * j:32 * (j + 1), g, :],
                                 avcs[0:HD, j, :],
                                 rec_all[32 * j:32 * (j + 1), 0, :])

        # ---------------- tail: out-proj, LN1, FFN, LN2 ----------------
        with tc.tile_pool(name="mm", bufs=3, space="PSUM") as mm_pool, \
             tc.tile_pool(name="stat", bufs=1, space="PSUM") as stat_pool, \
             tc.tile_pool(name="rep", bufs=1, space="PSUM") as rep_pool, \
             tc.tile_pool(name="tl", bufs=2) as tl_pool, \
             tc.tile_pool(name="tr", bufs=1) as tr_pool:

            def layer_norm(x3, gamma, beta, out3):
                """out3 = LN(x3) over the channel axis (2 chunks of 128)."""
                mu_ps = stat_pool.tile([1, SH], F32, tag="mu")
                e2_ps = stat_pool.tile([1, SH], F32, tag="e2")
                for kc in range(NKC):
                    nc.tensor.matmul(mu_ps, ones1r, x3[:, kc, :],
                                     start=(kc == 0), stop=(kc == NKC - 1))
                for kc in range(NKC):
                    sq = tl_pool.tile([P, SH], F32R, tag="sq")
                    if kc == 0:
                        nc.scalar.activation(out=sq, in_=x3[:, kc, :],
                                             func=AFT.Square)
                    else:
                        nc.vector.tensor_mul(sq, x3[:, kc, :], x3[:, kc, :])
                    nc.tensor.matmul(e2_ps, ones1r, sq,
                                     start=(kc == 0), stop=(kc == NKC - 1))
                mu_row = tr_pool.tile([1, SH], F32, tag="mu_row")
                nc.vector.tensor_scalar_mul(out=mu_row, in0=mu_ps,
                                            scalar1=INV_C)
                mu2_row = tr_pool.tile([1, SH], F32, tag="mu2_row")
                nc.vector.tensor_mul(mu2_row, mu_row, mu_row)
                var_row = tr_pool.tile([1, SH], F32, tag="var_row")
                # var = E[x^2] - mu^2 = e2/C - mu^2
                nc.vector.scalar_tensor_tensor(
                    out=var_row, in0=e2_ps, scalar=INV_C, in1=mu2_row,
                    op0=ALU.mult, op1=ALU.subtract)
                std_row = tr_pool.tile([1, SH], F32, tag="std_row")
                nc.scalar.activation(out=std_row, in_=var_row, func=AFT.Sqrt,
                                     bias=eps_sb[:1, :])
                rstd_row = tr_pool.tile([1, SH], F32, tag="rstd_row")
                scr_row = tr_pool.tile([1, SH], F32, tag="mu2_row")
                nc.vector.reciprocal_approx_accurate(out=rstd_row,
                                                     in_=std_row,
                                                     scratch=scr_row)
                mu_rep = rep_pool.tile([P, SH], F32, tag="mu_rep")
                nc.tensor.matmul(mu_rep, ones_rep, mu_row,
                                 start=True, stop=True)
                rstd_rep = rep_pool.tile([P, SH], F32, tag="rstd_rep")
                nc.tensor.matmul(rstd_rep, ones_rep, rstd_row,
                                 start=True, stop=True)
                for kc in range(NKC):
                    t = tl_pool.tile([P, SH], F32, tag="t")
                    e1 = [nc.vector, nc.gpsimd][kc]
                    e1.tensor_sub(t, x3[:, kc, :], mu_rep)
                    t2 = tl_pool.tile([P, SH], F32, tag="t2")
                    e1.tensor_mul(t2, t, rstd_rep)
                    nc.vector.tensor_scalar(
                        out=out3[:, kc, :], in0=t2,
                        scalar1=gamma[:, kc:kc + 1],
                        scalar2=beta[:, kc:kc + 1],
                        op0=ALU.mult, op1=ALU.add)

            # out-projection + residual (r = qT + Wo^T attn + bo)
            for mc in range(NKC):
                ps = mm_pool.tile([P, SH], F32, tag="mm")
                for kc in range(NKC):
                    nc.tensor.matmul(
                        ps, wo_sb[:, kc, mc * P:(mc + 1) * P],
                        attn_sb[:, kc, :],
                        start=(kc == 0), stop=(kc == NKC - 1))
                o_t = tl_pool.tile([P, SH], F32, tag="o_t")
                nc.gpsimd.tensor_scalar_add(out=o_t, in0=ps,
                                            scalar1=bo_sb[:, mc:mc + 1])
                nc.vector.tensor_add(r_sb[:, mc, :], qT_sb[:, mc, :], o_t)

            layer_norm(r_sb, g1_sb, b1_sb, z_sb)

            # FFN1 + relu
            for mf in range(NFC):
                ps = mm_pool.tile([P, SH], F32, tag="mm")
                for kc in range(NKC):
                    nc.tensor.matmul(
                        ps, w1_sb[:, kc, mf * P:(mf + 1) * P],
                        z_sb[:, kc, :],
                        start=(kc == 0), stop=(kc == NKC - 1))
                if mf % 2 == 0:
                    nc.scalar.activation(
                        out=h1_sb[:, mf, :], in_=ps, func=AFT.Relu,
                        bias=bp_sb[:, 8 + mf // 2, mf % 2:mf % 2 + 1])
                else:
                    nc.vector.tensor_scalar(
                        out=h1_sb[:, mf, :], in0=ps,
                        scalar1=bp_sb[:, 8 + mf // 2, mf % 2:mf % 2 + 1],
                        scalar2=0.0,
                        op0=ALU.add, op1=ALU.max)
            # FFN2 + bias + residual
            for mc in range(NKC):
                ps = mm_pool.tile([P, SH], F32, tag="mm")
                for kf in range(NFC):
                    nc.tensor.matmul(
                        ps, w2_sb[:, kf, mc * P:(mc + 1) * P],
                        h1_sb[:, kf, :],
                        start=(kf == 0), stop=(kf == NFC - 1))
                f2 = tl_pool.tile([P, SH], F32, tag="f2")
                nc.gpsimd.tensor_scalar_add(out=f2, in0=ps,
                                            scalar1=bf2_sb[:, mc:mc + 1])
                nc.vector.tensor_add(r_sb[:, mc, :], z_sb[:, mc, :], f2)

            layer_norm(r_sb, g2_sb, b2_sb, out_sb)

            out_r = out.rearrange("(kc p) s -> p kc s", p=P)
            nc.sync.dma_start(out=out_r[:, 0, :], in_=out_sb[:, 0, :])
            nc.gpsimd.dma_start(out=out_r[:, 1, :], in_=out_sb[:, 1, :])


_NC_CACHE = None


def _get_nc():
    global _NC_CACHE
    if _NC_CACHE is None:
        _NC_CACHE = build_bass()
    return _NC_CACHE


FP8_NP = ml_dtypes.float8_e4m3


def _pack_qk8(W):
    """Permute+pad Wq/Wk columns into the [g, hf, chl, chh, m] fp8 layout.

    Column m = 32*j + p' (p' < 16) of pass (g, hf) holds original column
    c' = (4g + j)*32 + hf*16 + p'; columns with p' >= 16 are zero."""
    W8 = np.asarray(W, np.float32).astype(FP8_NP)
    outp = np.zeros((2, 2, P, 2, P), FP8_NP)
    for g in range(2):
        for hf in range(2):
            for j in range(4):
                cols = (4 * g + j) * 32 + hf * 16 + np.arange(16)
                blk = W8[:, cols]                       # [C, 16]
                blk = blk.reshape(2, P, 16)             # (chh, chl, p')
                outp[g, hf, :, :, 32 * j:32 * j + 16] = \
                    blk.transpose(1, 0, 2)
    return np.ascontiguousarray(outp)


def make_in_maps(lidar_features, image_features, Wq, bq, Wk, bk, Wv, bv,
                 Wo, bo, g1, b1, W1, bf1, W2, bf2, g2, b2):
    xT_full = np.ascontiguousarray(
        np.asarray(lidar_features, np.float32).reshape(C, S))
    y_full = np.ascontiguousarray(
        np.asarray(image_features, np.float32).reshape(C, S))
    wqo = np.ascontiguousarray(np.stack([
        np.asarray(Wq, np.float32), np.asarray(Wo, np.float32)]))
    bpack = np.ascontiguousarray(np.concatenate([
        np.asarray(bq, np.float32)[None], np.asarray(bv, np.float32)[None],
        np.asarray(bo, np.float32)[None], np.asarray(bf2, np.float32)[None],
        np.asarray(g1, np.float32)[None], np.asarray(b1, np.float32)[None],
        np.asarray(g2, np.float32)[None], np.asarray(b2, np.float32)[None],
        np.asarray(bf1, np.float32).reshape(4, C)]))
    wv8 = np.asarray(Wv, np.float32).astype(FP8_NP).reshape(2, P, C)
    bvb = np.broadcast_to(np.asarray(bv, np.float32)[None, :],
                          (P, C)).copy()
    common = {
        "y8": y_full.astype(FP8_NP),
        "wqo": wqo,
        "wq8p": _pack_qk8(Wq),
        "wk8p": _pack_qk8(Wk),
        "wv8": np.ascontiguousarray(wv8),
        "w1": np.ascontiguousarray(np.asarray(W1, np.float32)),
        "w2": np.ascontiguousarray(np.asarray(W2, np.float32)),
        "ones32": np.ones((P, 1), np.float32),
        "bpack": bpack,
        "bvb": bvb,
    }
    in_maps = []
    for c in range(NCORES):
        m = dict(common)
        shard = np.ascontiguousarray(xT_full[:, c * SH:(c + 1) * SH])
        m["xT"] = shard
        m["x8"] = shard.astype(FP8_NP)
        in_maps.append(m)
    return in_maps


def kernel(lidar_features, image_features, Wq, bq, Wk, bk, Wv, bv, Wo, bo,
           g1, b1, W1, bf1, W2, bf2, g2, b2, num_heads, **run_kwargs):
    assert int(num_heads) == NH
    nc = _get_nc()
    in_maps = make_in_maps(lidar_features, image_features, Wq, bq, Wk, bk,
                           Wv, bv, Wo, bo, g1, b1, W1, bf1, W2, bf2, g2, b2)
    res = run_bass_kernel_spmd(nc, in_maps, core_ids=list(range(NCORES)),
                               **run_kwargs)
    full = np.concatenate([res.results[c]["out"] for c in range(NCORES)],
                          axis=1)
    kernel.last_results = res
    return full.reshape(1, C, 64, 64).astype(np.float32)


kernel.last_results = None


# revision 9
# speedup vs baseline: 1.5611x; 1.2114x over previous
"""Cross-attention fusion block on 8 trn2 NeuronCores.

Sharding: data-parallel over the query sequence (S=4096 -> 512 rows/core).
K/V projections are computed redundantly on every core. Channel-major
layout [C, S] throughout; no on-chip transposes.

v2 design (vs baseline): fp8 DoubleRow matmuls for the attention phase and
the K/V/Q8 projections, and the softmax exp split across ACT (true exp ->
fp8e5) / DVE / Pool (Schraudolph bit-hack exp via uint8 write + fp8e5
bitcast).  Key layout trick: Wk/Wq columns are permuted+zero-padded on the
host so the projection matmul lands K/Q directly in the [16, 2(half), ...]
partition layout DoubleRow needs (head h in grp g at partition band
32*(h%4), head-dim split 16+16 across the DoubleRow free axis).

Per-core pipeline (q = 512 query rows of this core):
  qT   = Wq^T xT + bq                  [256, 512] fp32r   (residual path)
  q8   = perm(Wq8)^T x8                [bands, 2, 512] fp8e4
  k8   = perm(Wk8)^T y8                [bands, 2, 4096] fp8e4
  v8_h = [y8^T Wv8 + bv | 1]           per head [128, 2, 33] fp8e4
  per (grp g, head j, 256-key chunk d):
    ST[k, q] = k8_h-chunk DR@ q8_h                  (2 DoubleRow matmuls)
    AT       = approx-exp(ST/sqrt(32)) -> fp8e5     (ACT exp | DVE/Pool hack)
    AVCS_h  += v8_h-chunk DR@ AT                    [33, 512] psum
  attn_h = AV_h * (1/CS_h)      (CS rows DMA-gathered, reciprocal, emat
                                 broadcast matmul, per-head psum*rec mul)
  o = Wo^T attn + bo; r = qT + o; z = LN1(r)
  h1 = relu(W1^T z + bf1); h2 = W2^T h1 + bf2; out = LN2(z + h2)
"""

import sys

for _p in ("/opt/trn_rl_repo", "/opt/pypackages"):
    if _p not in sys.path:
        sys.path.append(_p)

import numpy as np
import ml_dtypes

import concourse.bass as bass
import concourse.bacc as bacc
import concourse.tile as tile
from concourse import mybir
from concourse.bass_utils import run_bass_kernel_spmd

F32 = mybir.dt.float32
F32R = mybir.dt.float32r
FP8E4 = mybir.dt.float8e4
FP8E5 = mybir.dt.float8e5
U8 = mybir.dt.uint8
AFT = mybir.ActivationFunctionType
ALU = mybir.AluOpType
DR = mybir.MatmulPerfMode.DoubleRow

P = 128           # SBUF partitions
C = 256           # channels
S = 4096          # sequence (64*64)
NCORES = 8
SH = S // NCORES  # 512 query rows per core
NH = 8            # heads
HD = 32           # head dim
F = 4 * C         # FFN hidden = 1024
NKC = C // P      # 2 channel chunks
NFC = F // P      # 8 ffn chunks
ND = S // 256     # 16 double-row key chunks
NBLK = S // 512   # 8 key blocks for kT production
EPS = 1e-5
INV_SQRT_HD = 1.0 / float(np.sqrt(HD))
INV_C = 1.0 / C
# Schraudolph-style exp for fp8e5(=e5m2) bitcast: i = floor(A*st + B)
HACK_A = float(4.0 * np.log2(np.e)) * INV_SQRT_HD
HACK_B = 60.02


def build_bass():
    nc = bacc.Bacc()

    xT = nc.declare_dram_parameter("xT", [C, SH], F32R, isOutput=False)
    x8 = nc.declare_dram_parameter("x8", [C, SH], FP8E4, isOutput=False)
    y8 = nc.declare_dram_parameter("y8", [C, S], FP8E4, isOutput=False)
    wqo = nc.declare_dram_parameter("wqo", [2, C, C], F32R, isOutput=False)
    wq8p = nc.declare_dram_parameter("wq8p", [2, 2, P, 2, P], FP8E4,
                                     isOutput=False)
    wk8p = nc.declare_dram_parameter("wk8p", [2, 2, P, 2, P], FP8E4,
                                     isOutput=False)
    wv8 = nc.declare_dram_parameter("wv8", [2, P, C], FP8E4, isOutput=False)
    w1 = nc.declare_dram_parameter("w1", [C, F], F32R, isOutput=False)
    w2 = nc.declare_dram_parameter("w2", [F, C], F32R, isOutput=False)
    ones32 = nc.declare_dram_parameter("ones32", [P, 1], F32R, isOutput=False)
    bpack = nc.declare_dram_parameter("bpack", [12, C], F32, isOutput=False)
    bvb = nc.declare_dram_parameter("bvb", [P, C], F32, isOutput=False)
    out = nc.declare_dram_parameter("out", [C, SH], F32, isOutput=True)

    with tile.TileContext(nc) as tc:
        _emit(tc, xT, x8, y8, wqo, wq8p, wk8p, wv8, w1, w2, ones32,
              bpack, bvb, out)
    if not nc.is_finalized():
        nc.finalize()
    return nc


def _emit(tc, xT, x8, y8, wqo, wq8p, wk8p, wv8, w1, w2, ones32,
          bpack, bvb, out):
    nc = tc.nc

    import contextlib
    stack = contextlib.ExitStack()
    with stack:
        consts = stack.enter_context(tc.tile_pool(name="consts", bufs=1))
        big = stack.enter_context(tc.tile_pool(name="big", bufs=1))

        # ---------------- constants / inputs into SBUF ----------------
        y8_sb = big.tile([P, NKC, S], FP8E4)      # y8[ch, s]; ch = kc*128+p
        y8_r = y8.rearrange("(kc p) s -> p kc s", p=P)
        HS = S // 2
        for sh2 in range(2):
            eng = [nc.sync, nc.gpsimd][sh2]
            eng.dma_start(out=y8_sb[:, :, sh2 * HS:(sh2 + 1) * HS],
                          in_=y8_r[:, :, sh2 * HS:(sh2 + 1) * HS])
        xT_sb = big.tile([P, NKC, SH], F32R)
        nc.sync.dma_start(out=xT_sb, in_=xT.rearrange("(kc p) s -> p kc s",
                                                      p=P))
        x8_sb = big.tile([P, NKC, SH], FP8E4)
        nc.gpsimd.dma_start(out=x8_sb, in_=x8.rearrange("(kc p) s -> p kc s",
                                                        p=P))

        wqo_sb = consts.tile([P, 2, NKC, C], F32R)
        nc.sync.dma_start(
            out=wqo_sb, in_=wqo.rearrange("w (kc p) m -> p w kc m", p=P))
        wq_sb, wo_sb = (wqo_sb[:, i] for i in range(2))
        wq8p_sb = consts.tile([P, 2, 2, 2, P], FP8E4)
        nc.gpsimd.dma_start(
            out=wq8p_sb, in_=wq8p.rearrange("g hf p w m -> p g hf w m"))
        wk8p_sb = consts.tile([P, 2, 2, 2, P], FP8E4)
        nc.gpsimd.dma_start(
            out=wk8p_sb, in_=wk8p.rearrange("g hf p w m -> p g hf w m"))
        wv8_sb = consts.tile([P, 2, C], FP8E4)
        nc.gpsimd.dma_start(out=wv8_sb, in_=wv8.rearrange("w p m -> p w m"))

        bp_sb = consts.tile([P, 12, NKC], F32)
        nc.sync.dma_start(out=bp_sb,
                          in_=bpack.rearrange("n (kc p) -> p n kc", p=P))
        bq_sb = bp_sb[:, 0]
        bo_sb, bf2_sb = bp_sb[:, 2], bp_sb[:, 3]
        g1_sb, b1_sb, g2_sb, b2_sb = (bp_sb[:, i] for i in range(4, 8))
        bvb_sb = consts.tile([P, C], F32)
        nc.sync.dma_start(out=bvb_sb, in_=bvb[:])

        # late-needed weights issued after the attention-critical loads
        w1_sb = consts.tile([P, NKC, F], F32R)
        nc.gpsimd.dma_start(out=w1_sb,
                            in_=w1.rearrange("(kc p) m -> p kc m", p=P))
        w2_sb = consts.tile([P, NFC, C], F32R)
        nc.sync.dma_start(out=w2_sb,
                          in_=w2.rearrange("(kc p) m -> p kc m", p=P))
        ones1r = consts.tile([P, 1], F32R)      # LN-stats lhsT (f32r ones)
        nc.sync.dma_start(out=ones1r, in_=ones32[:])
        ones_rep = consts.tile([1, P], F32)     # K=1 row-replication lhsT
        nc.vector.memset(ones_rep, 1.0)
        ones132 = consts.tile([1, HD], F32R)    # rec band-broadcast lhsT
        nc.vector.memset(ones132, 1.0)
        eps_sb = consts.tile([P, 1], F32)
        nc.vector.memset(eps_sb, EPS)

        # persistent activations
        qT_sb = big.tile([P, NKC, SH], F32R)       # q^T (with bq), residual
        q8_sb = big.tile([P, 2, 2, SH], FP8E4)     # (band, g, half, q)
        k8_sb = big.tile([P, 2, 2, S], FP8E4)      # (band, g, half, s)
        v8_sb = big.tile([P, ND, 2, NH, HD + 1], FP8E4)  # (k, d, i, h, c|1)
        attn_sb = big.tile([P, NKC, SH], F32R)     # (attn@v)/cs + bv
        z_sb = big.tile([P, NKC, SH], F32R)        # LN1 output
        h1_sb = big.tile([P, NFC, SH], F32R)       # relu(ffn1)
        out_sb = big.tile([P, NKC, SH], F32)       # final
        r_sb = big.tile([P, NKC, SH], F32R)        # residual sums (LN inputs)

        # ones column of v8 (CS accumulator rows)
        nc.vector.memset(v8_sb[:, :, :, :, HD:HD + 1], 1.0)

        # ---------------- preamble: projections ----------------
        with tc.tile_pool(name="pre_k", bufs=2, space="PSUM") as pre_k, \
             tc.tile_pool(name="pre_v", bufs=3, space="PSUM") as pre_v:
            # q^T fp32 (residual): q[c',q] = sum_ch Wq[ch,c'] xT[ch,q]
            psq = pre_k.tile([P, 2, SH], F32, tag="ps")
            for mc in range(NKC):
                for kc in range(NKC):
                    nc.tensor.matmul(
                        psq[:, mc, :], wq_sb[:, kc, mc * P:(mc + 1) * P],
                        xT_sb[:, kc, :],
                        start=(kc == 0), stop=(kc == NKC - 1))
            for mc in range(NKC):
                nc.scalar.activation(out=qT_sb[:, mc, :], in_=psq[:, mc, :],
                                     func=AFT.Identity,
                                     bias=bq_sb[:, mc:mc + 1])
            # q8 in split-half band layout, via DoubleRow over channels
            for g in range(2):
                ps8 = pre_k.tile([P, 2, SH], F32, tag="ps")
                for hf in range(2):
                    nc.tensor.matmul(ps8[:, hf, :], wq8p_sb[:, g, hf],
                                     x8_sb, start=True, stop=True,
                                     perf_mode=DR)
                eng = [nc.vector, nc.gpsimd][g]
                eng.tensor_copy(q8_sb[:, g], ps8)
            # k8: per (g, 512-key blk): two DoubleRow matmuls + one convert
            conv_engs = [nc.scalar, nc.vector, nc.gpsimd]
            for g in range(2):
                for blk in range(NBLK):
                    psk = pre_k.tile([P, 2, SH], F32, tag="ps")
                    for hf in range(2):
                        nc.tensor.matmul(
                            psk[:, hf, :], wk8p_sb[:, g, hf],
                            y8_sb[:, :, blk * 512:(blk + 1) * 512],
                            start=True, stop=True, perf_mode=DR)
                    e = conv_engs[(g * NBLK + blk) % 3]
                    if e is nc.scalar:
                        nc.scalar.activation(
                            out=k8_sb[:, g, :, blk * 512:(blk + 1) * 512],
                            in_=psk, func=AFT.Copy)
                    else:
                        e.tensor_copy(
                            k8_sb[:, g, :, blk * 512:(blk + 1) * 512], psk)
            # v8: per 128-key chunk, DoubleRow over channels; +bv fused
            for ck in range(S // P):
                d, i = ck // 2, ck % 2
                psv = pre_v.tile([P, C], F32, tag="psv")
                nc.tensor.matmul(psv, y8_sb[:, :, ck * P:(ck + 1) * P],
                                 wv8_sb, start=True, stop=True, perf_mode=DR)
                e = [nc.vector, nc.gpsimd][ck % 2]
                e.tensor_add(
                    v8_sb[:, d, i, :, 0:HD],
                    psv.rearrange("p (h c) -> p h c", c=HD),
                    bvb_sb.rearrange("p (h c) -> p h c", c=HD))

        # ---------------- attention ----------------
        # weighted round-robin of the exp across ACT / DVE / Pool
        exp_w = [(nc.scalar, 1.20), (nc.vector, 0.96), (nc.gpsimd, 0.72)]
        credits = [0.0, 0.0, 0.0]
        exp_engs = []
        for _ in range(2 * ND * 4):
            for ii in range(3):
                credits[ii] += exp_w[ii][1]
            pick = max(range(3), key=lambda ii: credits[ii])
            credits[pick] -= sum(w for _, w in exp_w)
            exp_engs.append(exp_w[pick][0])

        with tc.tile_pool(name="st", bufs=3, space="PSUM") as st_pool, \
             tc.tile_pool(name="avcs", bufs=1, space="PSUM") as avcs_pool, \
             tc.tile_pool(name="at", bufs=6) as at_pool, \
             tc.tile_pool(name="nrm", bufs=1) as nrm_pool:
            uu = 0
            for hg in range(4):      # half-groups: 2 heads x 16 d-chunks
                g, jp = hg // 2, hg % 2
                avcs = avcs_pool.tile([HD + 1, 2, SH], F32, tag="avcs",
                                      name=f"avcs_{hg}")
                pend = []
                for d in range(ND):
                    for jj in range(2):
                        j = 2 * jp + jj
                        st = st_pool.tile([P, 2, SH], F32, tag="st")
                        for i in range(2):
                            nc.tensor.matmul(
                                st[:, i, :],
                                k8_sb[32 * j:32 * j + 16, g, :,
                                      256 * d + 128 * i:256 * d + 128 * i + 128],
                                q8_sb[32 * j:32 * j + 16, g],
                                start=True, stop=True, perf_mode=DR,
                                tile_position=(32 * j, 0))
                        at = at_pool.tile([P, 2, SH], FP8E5, tag="at")
                        e = exp_engs[uu]
                        uu += 1
                        if e is nc.scalar:
                            nc.scalar.activation(out=at, in_=st, func=AFT.Exp,
                                                 scale=INV_SQRT_HD)
                        else:
                            e.tensor_scalar(out=at.bitcast(U8), in0=st,
                                            scalar1=HACK_A, scalar2=HACK_B,
                                            op0=ALU.mult, op1=ALU.add)
                        pend.append((d, jj, at))
                        if len(pend) > 2:
                            pd, pjj, pat = pend.pop(0)
                            nc.tensor.matmul(
                                avcs[:, pjj, :],
                                v8_sb[:, pd, :, 4 * g + 2 * jp + pjj, :],
                                pat, start=(pd == 0), stop=(pd == ND - 1),
                                perf_mode=DR)
                for pd, pjj, pat in pend:
                    nc.tensor.matmul(
                        avcs[:, pjj, :],
                        v8_sb[:, pd, :, 4 * g + 2 * jp + pjj, :],
                        pat, start=(pd == 0), stop=(pd == ND - 1),
                        perf_mode=DR)
                # normalize: attn_h = av_h / cs_h  (bv already folded into v8)
                rec_row = nrm_pool.tile([1, 2, SH], F32, tag="rec_row",
                                        name=f"rec_row_{hg}")
                nc.vector.reciprocal_approx_fast(out=rec_row,
                                                 in_=avcs[HD:HD + 1, :, :])
                rec_all = st_pool.tile([P, 2, SH], F32, tag="st",
                                       name=f"rec_all_{hg}")
                rr_r = rec_row.bitcast(F32R)
                for jj in range(2):
                    j = 2 * jp + jj
                    nc.tensor.matmul(rec_all[32 * j:32 * (j + 1), 0, :],
                                     ones132, rr_r[:, jj, :],
                                     start=True, stop=True,
                                     tile_position=(0, 32 * j))
                for jj in range(2):
                    j = 2 * jp + jj
                    e = [nc.vector, nc.gpsimd][jj]
                    e.tensor_mul(attn_sb[32 * j:32 * (j + 1), g, :],
                                 avcs[0:HD, jj, :],
                                 rec_all[32 * j:32 * (j + 1), 0, :])

        # ---------------- tail: out-proj, LN1, FFN, LN2 ----------------
        with tc.tile_pool(name="mm", bufs=3, space="PSUM") as mm_pool, \
             tc.tile_pool(name="stat", bufs=1, space="PSUM") as stat_pool, \
             tc.tile_pool(name="rep", bufs=1, space="PSUM") as rep_pool, \
             tc.tile_pool(name="tl", bufs=2) as tl_pool, \
             tc.tile_pool(name="tr", bufs=1) as tr_pool:

            def layer_norm(x3, gamma, beta, out3):
                """out3 = LN(x3) over the channel axis (2 chunks of 128)."""
                mu_ps = stat_pool.tile([1, SH], F32, tag="mu")
                e2_ps = stat_pool.tile([1, SH], F32, tag="e2")
                for kc in range(NKC):
                    nc.tensor.matmul(mu_ps, ones1r, x3[:, kc, :],
                                     start=(kc == 0), stop=(kc == NKC - 1))
                for kc in range(NKC):
                    sq = tl_pool.tile([P, SH], F32R, tag="sq")
                    if kc == 0:
                        nc.scalar.activation(out=sq, in_=x3[:, kc, :],
                                             func=AFT.Square)
                    else:
                        nc.vector.tensor_mul(sq, x3[:, kc, :], x3[:, kc, :])
                    nc.tensor.matmul(e2_ps, ones1r, sq,
                                     start=(kc == 0), stop=(kc == NKC - 1))
                mu_row = tr_pool.tile([1, SH], F32, tag="mu_row")
                nc.vector.tensor_scalar_mul(out=mu_row, in0=mu_ps,
                                            scalar1=INV_C)
                mu2_row = tr_pool.tile([1, SH], F32, tag="mu2_row")
                nc.vector.tensor_mul(mu2_row, mu_row, mu_row)
                var_row = tr_pool.tile([1, SH], F32, tag="var_row")
                # var = E[x^2] - mu^2 = e2/C - mu^2
                nc.vector.scalar_tensor_tensor(
                    out=var_row, in0=e2_ps, scalar=INV_C, in1=mu2_row,
                    op0=ALU.mult, op1=ALU.subtract)
                std_row = tr_pool.tile([1, SH], F32, tag="std_row")
                nc.scalar.activation(out=std_row, in_=var_row, func=AFT.Sqrt,
                                     bias=eps_sb[:1, :])
                rstd_row = tr_pool.tile([1, SH], F32, tag="rstd_row")
                scr_row = tr_pool.tile([1, SH], F32, tag="mu2_row")
                nc.vector.reciprocal_approx_accurate(out=rstd_row,
                                                     in_=std_row,
                                                     scratch=scr_row)
                mu_rep = rep_pool.tile([P, SH], F32, tag="mu_rep")
                nc.tensor.matmul(mu_rep, ones_rep, mu_row,
                                 start=True, stop=True)
                rstd_rep = rep_pool.tile([P, SH], F32, tag="rstd_rep")
                nc.tensor.matmul(rstd_rep, ones_rep, rstd_row,
                                 start=True, stop=True)
                for kc in range(NKC):
                    t = tl_pool.tile([P, SH], F32, tag="t")
                    e1 = [nc.vector, nc.gpsimd][kc]
                    e1.tensor_sub(t, x3[:, kc, :], mu_rep)
                    t2 = tl_pool.tile([P, SH], F32, tag="t2")
                    e1.tensor_mul(t2, t, rstd_rep)
                    nc.vector.tensor_scalar(
                        out=out3[:, kc, :], in0=t2,
                        scalar1=gamma[:, kc:kc + 1],
                        scalar2=beta[:, kc:kc + 1],
                        op0=ALU.mult, op1=ALU.add)

            # out-projection + residual (r = qT + Wo^T attn + bo)
            for mc in range(NKC):
                ps = mm_pool.tile([P, SH], F32, tag="mm")
                for kc in range(NKC):
                    nc.tensor.matmul(
                        ps, wo_sb[:, kc, mc * P:(mc + 1) * P],
                        attn_sb[:, kc, :],
                        start=(kc == 0), stop=(kc == NKC - 1))
                o_t = tl_pool.tile([P, SH], F32, tag="o_t")
                nc.gpsimd.tensor_scalar_add(out=o_t, in0=ps,
                                            scalar1=bo_sb[:, mc:mc + 1])
                nc.vector.tensor_add(r_sb[:, mc, :], qT_sb[:, mc, :], o_t)

            layer_norm(r_sb, g1_sb, b1_sb, z_sb)

            # FFN1 + relu
            for mf in range(NFC):
                ps = mm_pool.tile([P, SH], F32, tag="mm")
                for kc in range(NKC):
                    nc.tensor.matmul(
                        ps, w1_sb[:, kc, mf * P:(mf + 1) * P],
                        z_sb[:, kc, :],
                        start=(kc == 0), stop=(kc == NKC - 1))
                if mf % 2 == 0:
                    nc.scalar.activation(
                        out=h1_sb[:, mf, :], in_=ps, func=AFT.Relu,
                        bias=bp_sb[:, 8 + mf // 2, mf % 2:mf % 2 + 1])
                else:
                    nc.vector.tensor_scalar(
                        out=h1_sb[:, mf, :], in0=ps,
                        scalar1=bp_sb[:, 8 + mf // 2, mf % 2:mf % 2 + 1],
                        scalar2=0.0,
                        op0=ALU.add, op1=ALU.max)
            # FFN2 + bias + residual
            for mc in range(NKC):
                ps = mm_pool.tile([P, SH], F32, tag="mm")
                for kf in range(NFC):
                    nc.tensor.matmul(
                        ps, w2_sb[:, kf, mc * P:(mc + 1) * P],
                        h1_sb[:, kf, :],
                        start=(kf == 0), stop=(kf == NFC - 1))
                f2 = tl_pool.tile([P, SH], F32, tag="f2")
                nc.gpsimd.tensor_scalar_add(out=f2, in0=ps,
                                            scalar1=bf2_sb[:, mc:mc + 1])
                nc.vector.tensor_add(r_sb[:, mc, :], z_sb[:, mc, :], f2)

            layer_norm(r_sb, g2_sb, b2_sb, out_sb)

            out_r = out.rearrange("(kc p) s -> p kc s", p=P)
            nc.sync.dma_start(out=out_r[:, 0, :], in_=out_sb[:, 0, :])
            nc.gpsimd.dma_start(out=out_r[:, 1, :], in_=out_sb[:, 1, :])


_NC_CACHE = None


def _get_nc():
    global _NC_CACHE
    if _NC_CACHE is None:
        _NC_CACHE = build_bass()
    return _NC_CACHE


FP8_NP = ml_dtypes.float8_e4m3


def _pack_qk8(W):
    """Permute+pad Wq/Wk columns into the [g, hf, chl, chh, m] fp8 layout.

    Column m = 32*j + p' (p' < 16) of pass (g, hf) holds original column
    c' = (4g + j)*32 + hf*16 + p'; columns with p' >= 16 are zero."""
    W8 = np.asarray(W, np.float32).astype(FP8_NP)
    outp = np.zeros((2, 2, P, 2, P), FP8_NP)
    for g in range(2):
        for hf in range(2):
            for j in range(4):
                cols = (4 * g + j) * 32 + hf * 16 + np.arange(16)
                blk = W8[:, cols]                       # [C, 16]
                blk = blk.reshape(2, P, 16)             # (chh, chl, p')
                outp[g, hf, :, :, 32 * j:32 * j + 16] = \
                    blk.transpose(1, 0, 2)
    return np.ascontiguousarray(outp)


def make_in_maps(lidar_features, image_features, Wq, bq, Wk, bk, Wv, bv,
                 Wo, bo, g1, b1, W1, bf1, W2, bf2, g2, b2):
    xT_full = np.ascontiguousarray(
        np.asarray(lidar_features, np.float32).reshape(C, S))
    y_full = np.ascontiguousarray(
        np.asarray(image_features, np.float32).reshape(C, S))
    wqo = np.ascontiguousarray(np.stack([
        np.asarray(Wq, np.float32), np.asarray(Wo, np.float32)]))
    bpack = np.ascontiguousarray(np.concatenate([
        np.asarray(bq, np.float32)[None], np.asarray(bv, np.float32)[None],
        np.asarray(bo, np.float32)[None], np.asarray(bf2, np.float32)[None],
        np.asarray(g1, np.float32)[None], np.asarray(b1, np.float32)[None],
        np.asarray(g2, np.float32)[None], np.asarray(b2, np.float32)[None],
        np.asarray(bf1, np.float32).reshape(4, C)]))
    wv8 = np.asarray(Wv, np.float32).astype(FP8_NP).reshape(2, P, C)
    bvb = np.broadcast_to(np.asarray(bv, np.float32)[None, :],
                          (P, C)).copy()
    common = {
        "y8": y_full.astype(FP8_NP),
        "wqo": wqo,
        "wq8p": _pack_qk8(Wq),
        "wk8p": _pack_qk8(Wk),
        "wv8": np.ascontiguousarray(wv8),
        "w1": np.ascontiguousarray(np.asarray(W1, np.float32)),
        "w2": np.ascontiguousarray(np.asarray(W2, np.float32)),
        "ones32": np.ones((P, 1), np.float32),
        "bpack": bpack,
        "bvb": bvb,
    }
    in_maps = []
    for c in range(NCORES):
        m = dict(common)
        shard = np.ascontiguousarray(xT_full[:, c * SH:(c + 1) * SH])
        m["xT"] = shard
        m["x8"] = shard.astype(FP8_NP)
        in_maps.append(m)
    return in_maps


def kernel(lidar_features, image_features, Wq, bq, Wk, bk, Wv, bv, Wo, bo,
           g1, b1, W1, bf1, W2, bf2, g2, b2, num_heads, **run_kwargs):
    assert int(num_heads) == NH
    nc = _get_nc()
    in_maps = make_in_maps(lidar_features, image_features, Wq, bq, Wk, bk,
                           Wv, bv, Wo, bo, g1, b1, W1, bf1, W2, bf2, g2, b2)
    res = run_bass_kernel_spmd(nc, in_maps, core_ids=list(range(NCORES)),
                               **run_kwargs)
    full = np.concatenate([res.results[c]["out"] for c in range(NCORES)],
                          axis=1)
    kernel.last_results = res
    return full.reshape(1, C, 64, 64).astype(np.float32)


kernel.last_results = None


# revision 12
# speedup vs baseline: 1.5952x; 1.0218x over previous
"""Cross-attention fusion block on 8 trn2 NeuronCores.

Sharding: data-parallel over the query sequence (S=4096 -> 512 rows/core).
K/V projections are computed redundantly on every core. Channel-major
layout [C, S] throughout; no on-chip transposes.

v2 design (vs baseline): fp8 DoubleRow matmuls for the attention phase and
the K/V/Q8 projections, and the softmax exp split across ACT (true exp ->
fp8e5) / DVE / Pool (Schraudolph bit-hack exp via uint8 write + fp8e5
bitcast).  Key layout trick: Wk/Wq columns are permuted+zero-padded on the
host so the projection matmul lands K/Q directly in the [16, 2(half), ...]
partition layout DoubleRow needs (head h in grp g at partition band
32*(h%4), head-dim split 16+16 across the DoubleRow free axis).

Per-core pipeline (q = 512 query rows of this core):
  qT   = Wq^T xT + bq                  [256, 512] fp32r   (residual path)
  q8   = perm(Wq8)^T x8                [bands, 2, 512] fp8e4
  k8   = perm(Wk8)^T y8                [bands, 2, 4096] fp8e4
  v8_h = [y8^T Wv8 + bv | 1]           per head [128, 2, 33] fp8e4
  per (grp g, head j, 256-key chunk d):
    ST[k, q] = k8_h-chunk DR@ q8_h                  (2 DoubleRow matmuls)
    AT       = approx-exp(ST/sqrt(32)) -> fp8e5     (ACT exp | DVE/Pool hack)
    AVCS_h  += v8_h-chunk DR@ AT                    [33, 512] psum
  attn_h = AV_h * (1/CS_h)      (CS rows DMA-gathered, reciprocal, emat
                                 broadcast matmul, per-head psum*rec mul)
  o = Wo^T attn + bo; r = qT + o; z = LN1(r)
  h1 = relu(W1^T z + bf1); h2 = W2^T h1 + bf2; out = LN2(z + h2)
"""

import sys

for _p in ("/opt/trn_rl_repo", "/opt/pypackages"):
    if _p not in sys.path:
        sys.path.append(_p)

import numpy as np
import ml_dtypes

import concourse.bass as bass
import concourse.bacc as bacc
import concourse.tile as tile
from concourse import mybir
from concourse.bass_utils import run_bass_kernel_spmd

F32 = mybir.dt.float32
F32R = mybir.dt.float32r
FP8E4 = mybir.dt.float8e4
FP8E5 = mybir.dt.float8e5
U8 = mybir.dt.uint8
AFT = mybir.ActivationFunctionType
ALU = mybir.AluOpType
DR = mybir.MatmulPerfMode.DoubleRow

P = 128           # SBUF partitions
C = 256           # channels
S = 4096          # sequence (64*64)
NCORES = 8
SH = S // NCORES  # 512 query rows per core
NH = 8            # heads
HD = 32           # head dim
F = 4 * C         # FFN hidden = 1024
NKC = C // P      # 2 channel chunks
NFC = F // P      # 8 ffn chunks
ND = S // 256     # 16 double-row key chunks
NBLK = S // 512   # 8 key blocks for kT production
EPS = 1e-5
INV_SQRT_HD = 1.0 / float(np.sqrt(HD))
INV_C = 1.0 / C
# Schraudolph-style exp for fp8e5(=e5m2) bitcast: i = floor(A*st + B)
HACK_A = float(4.0 * np.log2(np.e)) * INV_SQRT_HD
HACK_B = 60.02


def build_bass():
    nc = bacc.Bacc()

    xT = nc.declare_dram_parameter("xT", [C, SH], F32R, isOutput=False)
    x8 = nc.declare_dram_parameter("x8", [C, SH], FP8E4, isOutput=False)
    y8 = nc.declare_dram_parameter("y8", [C, S], FP8E4, isOutput=False)
    wqo = nc.declare_dram_parameter("wqo", [2, C, C], F32R, isOutput=False)
    wq8p = nc.declare_dram_parameter("wq8p", [2, 2, P, 2, P], FP8E4,
                                     isOutput=False)
    wk8p = nc.declare_dram_parameter("wk8p", [2, 2, P, 2, P], FP8E4,
                                     isOutput=False)
    wv8 = nc.declare_dram_parameter("wv8", [2, P, C], FP8E4, isOutput=False)
    w1 = nc.declare_dram_parameter("w1", [C, F], F32R, isOutput=False)
    w2 = nc.declare_dram_parameter("w2", [F, C], F32R, isOutput=False)
    ones32 = nc.declare_dram_parameter("ones32", [P, 1], F32R, isOutput=False)
    bpack = nc.declare_dram_parameter("bpack", [12, C], F32, isOutput=False)
    bvb = nc.declare_dram_parameter("bvb", [P, C], F32, isOutput=False)
    out = nc.declare_dram_parameter("out", [C, SH], F32, isOutput=True)

    with tile.TileContext(nc) as tc:
        _emit(tc, xT, x8, y8, wqo, wq8p, wk8p, wv8, w1, w2, ones32,
              bpack, bvb, out)
    if not nc.is_finalized():
        nc.finalize()
    return nc


def _emit(tc, xT, x8, y8, wqo, wq8p, wk8p, wv8, w1, w2, ones32,
          bpack, bvb, out):
    nc = tc.nc

    import contextlib
    stack = contextlib.ExitStack()
    with stack:
        consts = stack.enter_context(tc.tile_pool(name="consts", bufs=1))
        big = stack.enter_context(tc.tile_pool(name="big", bufs=1))

        # ---------------- constants / inputs into SBUF ----------------
        y8_sb = big.tile([P, NKC, S], FP8E4)      # y8[ch, s]; ch = kc*128+p
        y8_r = y8.rearrange("(kc p) s -> p kc s", p=P)
        HS = S // 2
        for sh2 in range(2):
            eng = [nc.sync, nc.gpsimd][sh2]
            eng.dma_start(out=y8_sb[:, :, sh2 * HS:(sh2 + 1) * HS],
                          in_=y8_r[:, :, sh2 * HS:(sh2 + 1) * HS])
        xT_sb = big.tile([P, NKC, SH], F32R)
        nc.sync.dma_start(out=xT_sb, in_=xT.rearrange("(kc p) s -> p kc s",
                                                      p=P))
        x8_sb = big.tile([P, NKC, SH], FP8E4)
        nc.gpsimd.dma_start(out=x8_sb, in_=x8.rearrange("(kc p) s -> p kc s",
                                                        p=P))

        wqo_sb = consts.tile([P, 2, NKC, C], F32R)
        nc.sync.dma_start(
            out=wqo_sb, in_=wqo.rearrange("w (kc p) m -> p w kc m", p=P))
        wq_sb, wo_sb = (wqo_sb[:, i] for i in range(2))
        wq8p_sb = consts.tile([P, 2, 2, 2, P], FP8E4)
        nc.gpsimd.dma_start(
            out=wq8p_sb, in_=wq8p.rearrange("g hf p w m -> p g hf w m"))
        wk8p_sb = consts.tile([P, 2, 2, 2, P], FP8E4)
        nc.gpsimd.dma_start(
            out=wk8p_sb, in_=wk8p.rearrange("g hf p w m -> p g hf w m"))
        wv8_sb = consts.tile([P, 2, C], FP8E4)
        nc.gpsimd.dma_start(out=wv8_sb, in_=wv8.rearrange("w p m -> p w m"))

        bp_sb = consts.tile([P, 12, NKC], F32)
        nc.sync.dma_start(out=bp_sb,
                          in_=bpack.rearrange("n (kc p) -> p n kc", p=P))
        bq_sb = bp_sb[:, 0]
        bo_sb, bf2_sb = bp_sb[:, 2], bp_sb[:, 3]
        g1_sb, b1_sb, g2_sb, b2_sb = (bp_sb[:, i] for i in range(4, 8))
        bvb_sb = consts.tile([P, C], F32)
        nc.sync.dma_start(out=bvb_sb, in_=bvb[:])

        # late-needed weights issued after the attention-critical loads
        w1_sb = consts.tile([P, NKC, F], F32R)
        nc.gpsimd.dma_start(out=w1_sb,
                            in_=w1.rearrange("(kc p) m -> p kc m", p=P))
        w2_sb = consts.tile([P, NFC, C], F32R)
        nc.sync.dma_start(out=w2_sb,
                          in_=w2.rearrange("(kc p) m -> p kc m", p=P))
        ones1r = consts.tile([P, 1], F32R)      # LN-stats lhsT (f32r ones)
        nc.sync.dma_start(out=ones1r, in_=ones32[:])
        ones_rep = consts.tile([1, P], F32)     # K=1 row-replication lhsT
        nc.vector.memset(ones_rep, 1.0)
        ones132 = consts.tile([1, HD], F32R)    # rec band-broadcast lhsT
        nc.vector.memset(ones132, 1.0)
        eps_sb = consts.tile([P, 1], F32)
        nc.vector.memset(eps_sb, EPS)

        # persistent activations
        qT_sb = big.tile([P, NKC, SH], F32R)       # q^T (with bq), residual
        q8_sb = big.tile([P, 2, 2, SH], FP8E4)     # (band, g, half, q)
        k8_sb = big.tile([P, 2, 2, S], FP8E4)      # (band, g, half, s)
        v8_sb = big.tile([P, ND, 2, NH, HD + 1], FP8E4)  # (k, d, i, h, c|1)
        attn_sb = big.tile([P, NKC, SH], F32R)     # (attn@v)/cs + bv
        z_sb = big.tile([P, NKC, SH], F32R)        # LN1 output
        h1_sb = big.tile([P, NFC, SH], F32R)       # relu(ffn1)
        out_sb = big.tile([P, NKC, SH], F32)       # final
        r_sb = big.tile([P, NKC, SH], F32R)        # residual sums (LN inputs)

        # ones column of v8 (CS accumulator rows)
        nc.vector.memset(v8_sb[:, :, :, :, HD:HD + 1], 1.0)

        # ---------------- preamble: projections ----------------
        with tc.tile_pool(name="pre_k", bufs=2, space="PSUM") as pre_k, \
             tc.tile_pool(name="pre_v", bufs=3, space="PSUM") as pre_v:
            # q^T fp32 (residual): q[c',q] = sum_ch Wq[ch,c'] xT[ch,q]
            psq = pre_k.tile([P, 2, SH], F32, tag="ps")
            for mc in range(NKC):
                for kc in range(NKC):
                    nc.tensor.matmul(
                        psq[:, mc, :], wq_sb[:, kc, mc * P:(mc + 1) * P],
                        xT_sb[:, kc, :],
                        start=(kc == 0), stop=(kc == NKC - 1))
            for mc in range(NKC):
                nc.scalar.activation(out=qT_sb[:, mc, :], in_=psq[:, mc, :],
                                     func=AFT.Identity,
                                     bias=bq_sb[:, mc:mc + 1])
            # q8 in split-half band layout, via DoubleRow over channels
            for g in range(2):
                ps8 = pre_k.tile([P, 2, SH], F32, tag="ps")
                for hf in range(2):
                    nc.tensor.matmul(ps8[:, hf, :], wq8p_sb[:, g, hf],
                                     x8_sb, start=True, stop=True,
                                     perf_mode=DR)
                nc.gpsimd.tensor_copy(q8_sb[:, g], ps8)
            # k8: per (g, 512-key blk): two DoubleRow matmuls + one convert
            for g in range(2):
                for blk in range(NBLK):
                    psk = pre_k.tile([P, 2, SH], F32, tag="ps")
                    for hf in range(2):
                        nc.tensor.matmul(
                            psk[:, hf, :], wk8p_sb[:, g, hf],
                            y8_sb[:, :, blk * 512:(blk + 1) * 512],
                            start=True, stop=True, perf_mode=DR)
                    if (g * NBLK + blk) % 2 == 0:
                        nc.scalar.activation(
                            out=k8_sb[:, g, :, blk * 512:(blk + 1) * 512],
                            in_=psk, func=AFT.Copy)
                    else:
                        nc.gpsimd.tensor_copy(
                            k8_sb[:, g, :, blk * 512:(blk + 1) * 512], psk)
            # v8: per 128-key chunk, DoubleRow over channels; +bv fused
            for ck in range(S // P):
                d, i = ck // 2, ck % 2
                psv = pre_v.tile([P, C], F32, tag="psv")
                nc.tensor.matmul(psv, y8_sb[:, :, ck * P:(ck + 1) * P],
                                 wv8_sb, start=True, stop=True, perf_mode=DR)
                e = nc.vector if ck % 4 == 0 else nc.gpsimd
                e.tensor_add(
                    v8_sb[:, d, i, :, 0:HD],
                    psv.rearrange("p (h c) -> p h c", c=HD),
                    bvb_sb.rearrange("p (h c) -> p h c", c=HD))

        # ---------------- attention ----------------
        # strict round-robin of the exp across ACT / Pool / DVE, aligned
        # with the 3-deep st psum ring (33 allocations per half-group)
        rr = [nc.scalar, nc.gpsimd, nc.vector]
        exp_engs = [rr[u % 3] for u in range(2 * ND * 4)]

        with tc.tile_pool(name="st", bufs=3, space="PSUM") as st_pool, \
             tc.tile_pool(name="avcs", bufs=1, space="PSUM") as avcs_pool, \
             tc.tile_pool(name="at", bufs=6) as at_pool, \
             tc.tile_pool(name="nrm", bufs=1) as nrm_pool:
            uu = 0
            for hg in range(4):      # half-groups: 2 heads x 16 d-chunks
                g, jp = hg // 2, hg % 2
                avcs = avcs_pool.tile([HD + 1, 2, SH], F32, tag="avcs",
                                      name=f"avcs_{hg}")
                pend = []
                for d in range(ND):
                    for jj in range(2):
                        j = 2 * jp + jj
                        st = st_pool.tile([P, 2, SH], F32, tag="st")
                        for i in range(2):
                            nc.tensor.matmul(
                                st[:, i, :],
                                k8_sb[32 * j:32 * j + 16, g, :,
                                      256 * d + 128 * i:256 * d + 128 * i + 128],
                                q8_sb[32 * j:32 * j + 16, g],
                                start=True, stop=True, perf_mode=DR,
                                tile_position=(32 * j, 0))
                        at = at_pool.tile([P, 2, SH], FP8E5, tag="at")
                        e = exp_engs[uu]
                        uu += 1
                        if e is nc.scalar:
                            nc.scalar.activation(out=at, in_=st, func=AFT.Exp,
                                                 scale=INV_SQRT_HD)
                        else:
                            e.tensor_scalar(out=at.bitcast(U8), in0=st,
                                            scalar1=HACK_A, scalar2=HACK_B,
                                            op0=ALU.mult, op1=ALU.add)
                        pend.append((d, jj, at))
                        if len(pend) > 2:
                            pd, pjj, pat = pend.pop(0)
                            nc.tensor.matmul(
                                avcs[:, pjj, :],
                                v8_sb[:, pd, :, 4 * g + 2 * jp + pjj, :],
                                pat, start=(pd == 0), stop=(pd == ND - 1),
                                perf_mode=DR)
                for pd, pjj, pat in pend:
                    nc.tensor.matmul(
                        avcs[:, pjj, :],
                        v8_sb[:, pd, :, 4 * g + 2 * jp + pjj, :],
                        pat, start=(pd == 0), stop=(pd == ND - 1),
                        perf_mode=DR)
                # normalize: attn_h = av_h / cs_h  (bv already folded into v8)
                rec_row = nrm_pool.tile([1, 2, SH], F32, tag="rec_row",
                                        name=f"rec_row_{hg}")
                nc.vector.reciprocal_approx_fast(out=rec_row,
                                                 in_=avcs[HD:HD + 1, :, :])
                rec_all = st_pool.tile([P, 2, SH], F32, tag="st",
                                       name=f"rec_all_{hg}")
                rr_r = rec_row.bitcast(F32R)
                for jj in range(2):
                    j = 2 * jp + jj
                    nc.tensor.matmul(rec_all[32 * j:32 * (j + 1), 0, :],
                                     ones132, rr_r[:, jj, :],
                                     start=True, stop=True,
                                     tile_position=(0, 32 * j))
                for jj in range(2):
                    j = 2 * jp + jj
                    e = [nc.vector, nc.gpsimd][jj]
                    e.tensor_mul(attn_sb[32 * j:32 * (j + 1), g, :],
                                 avcs[0:HD, jj, :],
                                 rec_all[32 * j:32 * (j + 1), 0, :])

        # ---------------- tail: out-proj, LN1, FFN, LN2 ----------------
        with tc.tile_pool(name="mm", bufs=3, space="PSUM") as mm_pool, \
             tc.tile_pool(name="stat", bufs=1, space="PSUM") as stat_pool, \
             tc.tile_pool(name="rep", bufs=1, space="PSUM") as rep_pool, \
             tc.tile_pool(name="tl", bufs=2) as tl_pool, \
             tc.tile_pool(name="tr", bufs=1) as tr_pool:

            def layer_norm(x3, gamma, beta, out3):
                """out3 = LN(x3) over the channel axis (2 chunks of 128)."""
                mu_ps = stat_pool.tile([1, SH], F32, tag="mu")
                e2_ps = stat_pool.tile([1, SH], F32, tag="e2")
                for kc in range(NKC):
                    nc.tensor.matmul(mu_ps, ones1r, x3[:, kc, :],
                                     start=(kc == 0), stop=(kc == NKC - 1))
                for kc in range(NKC):
                    sq = tl_pool.tile([P, SH], F32R, tag="sq")
                    if kc == 0:
                        nc.scalar.activation(out=sq, in_=x3[:, kc, :],
                                             func=AFT.Square)
                    else:
                        nc.vector.tensor_mul(sq, x3[:, kc, :], x3[:, kc, :])
                    nc.tensor.matmul(e2_ps, ones1r, sq,
                                     start=(kc == 0), stop=(kc == NKC - 1))
                mu_row = tr_pool.tile([1, SH], F32, tag="mu_row")
                nc.vector.tensor_scalar_mul(out=mu_row, in0=mu_ps,
                                            scalar1=INV_C)
                mu2_row = tr_pool.tile([1, SH], F32, tag="mu2_row")
                nc.vector.tensor_mul(mu2_row, mu_row, mu_row)
                var_row = tr_pool.tile([1, SH], F32, tag="var_row")
                # var = E[x^2] - mu^2 = e2/C - mu^2
                nc.vector.scalar_tensor_tensor(
                    out=var_row, in0=e2_ps, scalar=INV_C, in1=mu2_row,
                    op0=ALU.mult, op1=ALU.subtract)
                std_row = tr_pool.tile([1, SH], F32, tag="std_row")
                nc.scalar.activation(out=std_row, in_=var_row, func=AFT.Sqrt,
                                     bias=eps_sb[:1, :])
                rstd_row = tr_pool.tile([1, SH], F32, tag="rstd_row")
                scr_row = tr_pool.tile([1, SH], F32, tag="mu2_row")
                nc.vector.reciprocal_approx_accurate(out=rstd_row,
                                                     in_=std_row,
                                                     scratch=scr_row)
                mu_rep = rep_pool.tile([P, SH], F32, tag="mu_rep")
                nc.tensor.matmul(mu_rep, ones_rep, mu_row,
                                 start=True, stop=True)
                rstd_rep = rep_pool.tile([P, SH], F32, tag="rstd_rep")
                nc.tensor.matmul(rstd_rep, ones_rep, rstd_row,
                                 start=True, stop=True)
                for kc in range(NKC):
                    t = tl_pool.tile([P, SH], F32, tag="t")
                    e1 = [nc.vector, nc.gpsimd][kc]
                    e1.tensor_sub(t, x3[:, kc, :], mu_rep)
                    t2 = tl_pool.tile([P, SH], F32, tag="t2")
                    e1.tensor_mul(t2, t, rstd_rep)
                    nc.vector.tensor_scalar(
                        out=out3[:, kc, :], in0=t2,
                        scalar1=gamma[:, kc:kc + 1],
                        scalar2=beta[:, kc:kc + 1],
                        op0=ALU.mult, op1=ALU.add)

            # out-projection + residual (r = qT + Wo^T attn + bo)
            for mc in range(NKC):
                ps = mm_pool.tile([P, SH], F32, tag="mm")
                for kc in range(NKC):
                    nc.tensor.matmul(
                        ps, wo_sb[:, kc, mc * P:(mc + 1) * P],
                        attn_sb[:, kc, :],
                        start=(kc == 0), stop=(kc == NKC - 1))
                o_t = tl_pool.tile([P, SH], F32, tag="o_t")
                nc.gpsimd.tensor_scalar_add(out=o_t, in0=ps,
                                            scalar1=bo_sb[:, mc:mc + 1])
                nc.vector.tensor_add(r_sb[:, mc, :], qT_sb[:, mc, :], o_t)

            layer_norm(r_sb, g1_sb, b1_sb, z_sb)

            # FFN1 + relu
            for mf in range(NFC):
                ps = mm_pool.tile([P, SH], F32, tag="mm")
                for kc in range(NKC):
                    nc.tensor.matmul(
                        ps, w1_sb[:, kc, mf * P:(mf + 1) * P],
                        z_sb[:, kc, :],
                        start=(kc == 0), stop=(kc == NKC - 1))
                if mf % 2 == 0:
                    nc.scalar.activation(
                        out=h1_sb[:, mf, :], in_=ps, func=AFT.Relu,
                        bias=bp_sb[:, 8 + mf // 2, mf % 2:mf % 2 + 1])
                else:
                    nc.vector.tensor_scalar(
                        out=h1_sb[:, mf, :], in0=ps,
                        scalar1=bp_sb[:, 8 + mf // 2, mf % 2:mf % 2 + 1],
                        scalar2=0.0,
                        op0=ALU.add, op1=ALU.max)
            # FFN2 + bias + residual
            for mc in range(NKC):
                ps = mm_pool.tile([P, SH], F32, tag="mm")
                for kf in range(NFC):
                    nc.tensor.matmul(
                        ps, w2_sb[:, kf, mc * P:(mc + 1) * P],
                        h1_sb[:, kf, :],
                        start=(kf == 0), stop=(kf == NFC - 1))
                f2 = tl_pool.tile([P, SH], F32, tag="f2")
                nc.gpsimd.tensor_scalar_add(out=f2, in0=ps,
                                            scalar1=bf2_sb[:, mc:mc + 1])
                nc.vector.tensor_add(r_sb[:, mc, :], z_sb[:, mc, :], f2)

            layer_norm(r_sb, g2_sb, b2_sb, out_sb)

            out_r = out.rearrange("(kc p) s -> p kc s", p=P)
            nc.sync.dma_start(out=out_r[:, 0, :], in_=out_sb[:, 0, :])
            nc.gpsimd.dma_start(out=out_r[:, 1, :], in_=out_sb[:, 1, :])


_NC_CACHE = None


def _get_nc():
    global _NC_CACHE
    if _NC_CACHE is None:
        _NC_CACHE = build_bass()
    return _NC_CACHE


FP8_NP = ml_dtypes.float8_e4m3


def _pack_qk8(W):
    """Permute+pad Wq/Wk columns into the [g, hf, chl, chh, m] fp8 layout.

    Column m = 32*j + p' (p' < 16) of pass (g, hf) holds original column
    c' = (4g + j)*32 + hf*16 + p'; columns with p' >= 16 are zero."""
    W8 = np.asarray(W, np.float32).astype(FP8_NP)
    outp = np.zeros((2, 2, P, 2, P), FP8_NP)
    for g in range(2):
        for hf in range(2):
            for j in range(4):
                cols = (4 * g + j) * 32 + hf * 16 + np.arange(16)
                blk = W8[:, cols]                       # [C, 16]
                blk = blk.reshape(2, P, 16)             # (chh, chl, p')
                outp[g, hf, :, :, 32 * j:32 * j + 16] = \
                    blk.transpose(1, 0, 2)
    return np.ascontiguousarray(outp)


def make_in_maps(lidar_features, image_features, Wq, bq, Wk, bk, Wv, bv,
                 Wo, bo, g1, b1, W1, bf1, W2, bf2, g2, b2):
    xT_full = np.ascontiguousarray(
        np.asarray(lidar_features, np.float32).reshape(C, S))
    y_full = np.ascontiguousarray(
        np.asarray(image_features, np.float32).reshape(C, S))
    wqo = np.ascontiguousarray(np.stack([
        np.asarray(Wq, np.float32), np.asarray(Wo, np.float32)]))
    bpack = np.ascontiguousarray(np.concatenate([
        np.asarray(bq, np.float32)[None], np.asarray(bv, np.float32)[None],
        np.asarray(bo, np.float32)[None], np.asarray(bf2, np.float32)[None],
        np.asarray(g1, np.float32)[None], np.asarray(b1, np.float32)[None],
        np.asarray(g2, np.float32)[None], np.asarray(b2, np.float32)[None],
        np.asarray(bf1, np.float32).reshape(4, C)]))
    wv8 = np.asarray(Wv, np.float32).astype(FP8_NP).reshape(2, P, C)
    bvb = np.broadcast_to(np.asarray(bv, np.float32)[None, :],
                          (P, C)).copy()
    common = {
        "y8": y_full.astype(FP8_NP),
        "wqo": wqo,
        "wq8p": _pack_qk8(Wq),
        "wk8p": _pack_qk8(Wk),
        "wv8": np.ascontiguousarray(wv8),
        "w1": np.ascontiguousarray(np.asarray(W1, np.float32)),
        "w2": np.ascontiguousarray(np.asarray(W2, np.float32)),
        "ones32": np.ones((P, 1), np.float32),
        "bpack": bpack,
        "bvb": bvb,
    }
    in_maps = []
    for c in range(NCORES):
        m = dict(common)
        shard = np.ascontiguousarray(xT_full[:, c * SH:(c + 1) * SH])
        m["xT"] = shard
        m["x8"] = shard.astype(FP8_NP)
        in_maps.append(m)
    return in_maps


def kernel(lidar_features, image_features, Wq, bq, Wk, bk, Wv, bv, Wo, bo,
           g1, b1, W1, bf1, W2, bf2, g2, b2, num_heads, **run_kwargs):
    assert int(num_heads) == NH
    nc = _get_nc()
    in_maps = make_in_maps(lidar_features, image_features, Wq, bq, Wk, bk,
                           Wv, bv, Wo, bo, g1, b1, W1, bf1, W2, bf2, g2, b2)
    res = run_bass_kernel_spmd(nc, in_maps, core_ids=list(range(NCORES)),
                               **run_kwargs)
    full = np.concatenate([res.results[c]["out"] for c in range(NCORES)],
                          axis=1)
    kernel.last_results = res
    return full.reshape(1, C, 64, 64).astype(np.float32)


kernel.last_results = None


# revision 14
# speedup vs baseline: 1.8148x; 1.1377x over previous
"""Cross-attention fusion block on 8 trn2 NeuronCores.

Sharding: data-parallel over the query sequence (S=4096 -> 512 rows/core).
K/V projections are computed redundantly on every core. Channel-major
layout [C, S] throughout; no on-chip transposes.

v2 design (vs baseline): fp8 DoubleRow matmuls for the attention phase and
the K/V/Q8 projections, and the softmax exp split across ACT (true exp ->
fp8e5) / DVE / Pool (Schraudolph bit-hack exp via uint8 write + fp8e5
bitcast).  Key layout trick: Wk/Wq columns are permuted+zero-padded on the
host so the projection matmul lands K/Q directly in the [16, 2(half), ...]
partition layout DoubleRow needs (head h in grp g at partition band
32*(h%4), head-dim split 16+16 across the DoubleRow free axis).

Per-core pipeline (q = 512 query rows of this core):
  qT   = Wq^T xT + bq                  [256, 512] fp32r   (residual path)
  q8   = perm(Wq8)^T x8                [bands, 2, 512] fp8e4
  k8   = perm(Wk8)^T y8                [bands, 2, 4096] fp8e4
  v8_h = [y8^T Wv8 + bv | 1]           per head [128, 2, 33] fp8e4
  per (grp g, head j, 256-key chunk d):
    ST[k, q] = k8_h-chunk DR@ q8_h                  (2 DoubleRow matmuls)
    AT       = approx-exp(ST/sqrt(32)) -> fp8e5     (ACT exp | DVE/Pool hack)
    AVCS_h  += v8_h-chunk DR@ AT                    [33, 512] psum
  attn_h = AV_h * (1/CS_h)      (CS rows DMA-gathered, reciprocal, emat
                                 broadcast matmul, per-head psum*rec mul)
  o = Wo^T attn + bo; r = qT + o; z = LN1(r)
  h1 = relu(W1^T z + bf1); h2 = W2^T h1 + bf2; out = LN2(z + h2)
"""

import sys

for _p in ("/opt/trn_rl_repo", "/opt/pypackages"):
    if _p not in sys.path:
        sys.path.append(_p)

import numpy as np
import ml_dtypes

import concourse.bass as bass
import concourse.bacc as bacc
import concourse.tile as tile
from concourse import mybir
from concourse.bass_utils import run_bass_kernel_spmd

F32 = mybir.dt.float32
F32R = mybir.dt.float32r
FP8E4 = mybir.dt.float8e4
FP8E5 = mybir.dt.float8e5
U8 = mybir.dt.uint8
AFT = mybir.ActivationFunctionType
ALU = mybir.AluOpType
DR = mybir.MatmulPerfMode.DoubleRow

P = 128           # SBUF partitions
C = 256           # channels
S = 4096          # sequence (64*64)
NCORES = 8
SH = S // NCORES  # 512 query rows per core
NH = 8            # heads
HD = 32           # head dim
F = 4 * C         # FFN hidden = 1024
NKC = C // P      # 2 channel chunks
NFC = F // P      # 8 ffn chunks
ND = S // 256     # 16 double-row key chunks
NBLK = S // 512   # 8 key blocks for kT production
EPS = 1e-5
INV_SQRT_HD = 1.0 / float(np.sqrt(HD))
INV_C = 1.0 / C
# Schraudolph-style exp for fp8e5(=e5m2) bitcast: i = floor(A*st + B)
HACK_A = float(4.0 * np.log2(np.e)) * INV_SQRT_HD
HACK_B = 60.02


def build_bass():
    nc = bacc.Bacc()

    xT = nc.declare_dram_parameter("xT", [C, SH], F32R, isOutput=False)
    x8 = nc.declare_dram_parameter("x8", [C, SH], FP8E4, isOutput=False)
    y8 = nc.declare_dram_parameter("y8", [C, S], FP8E4, isOutput=False)
    wqo = nc.declare_dram_parameter("wqo", [2, C, C], F32R, isOutput=False)
    wq8p = nc.declare_dram_parameter("wq8p", [2, 2, P, 2, P], FP8E4,
                                     isOutput=False)
    wk8p = nc.declare_dram_parameter("wk8p", [2, 2, P, 2, P], FP8E4,
                                     isOutput=False)
    wv8 = nc.declare_dram_parameter("wv8", [2, P, C], FP8E4, isOutput=False)
    w1 = nc.declare_dram_parameter("w1", [C, F], F32R, isOutput=False)
    w2 = nc.declare_dram_parameter("w2", [F, C], F32R, isOutput=False)
    ones32 = nc.declare_dram_parameter("ones32", [P, 1], F32R, isOutput=False)
    bpack = nc.declare_dram_parameter("bpack", [12, C], F32, isOutput=False)
    bvb = nc.declare_dram_parameter("bvb", [P, C], F32, isOutput=False)
    out = nc.declare_dram_parameter("out", [C, SH], F32, isOutput=True)

    with tile.TileContext(nc) as tc:
        _emit(tc, xT, x8, y8, wqo, wq8p, wk8p, wv8, w1, w2, ones32,
              bpack, bvb, out)
    if not nc.is_finalized():
        nc.finalize()
    return nc


def _emit(tc, xT, x8, y8, wqo, wq8p, wk8p, wv8, w1, w2, ones32,
          bpack, bvb, out):
    nc = tc.nc

    import contextlib
    stack = contextlib.ExitStack()
    with stack:
        consts = stack.enter_context(tc.tile_pool(name="consts", bufs=1))
        big = stack.enter_context(tc.tile_pool(name="big", bufs=1))

        # ---------------- constants / inputs into SBUF ----------------
        y8_sb = big.tile([P, NKC, S], FP8E4)      # y8[ch, s]; ch = kc*128+p
        y8_r = y8.rearrange("(kc p) s -> p kc s", p=P)
        HS = S // 2
        for sh2 in range(2):
            eng = [nc.sync, nc.gpsimd][sh2]
            eng.dma_start(out=y8_sb[:, :, sh2 * HS:(sh2 + 1) * HS],
                          in_=y8_r[:, :, sh2 * HS:(sh2 + 1) * HS])
        xT_sb = big.tile([P, NKC, SH], F32R)
        nc.sync.dma_start(out=xT_sb, in_=xT.rearrange("(kc p) s -> p kc s",
                                                      p=P))
        x8_sb = big.tile([P, NKC, SH], FP8E4)
        nc.gpsimd.dma_start(out=x8_sb, in_=x8.rearrange("(kc p) s -> p kc s",
                                                        p=P))

        wqo_sb = consts.tile([P, 2, NKC, C], F32R)
        nc.sync.dma_start(
            out=wqo_sb, in_=wqo.rearrange("w (kc p) m -> p w kc m", p=P))
        wq_sb, wo_sb = (wqo_sb[:, i] for i in range(2))
        wq8p_sb = consts.tile([P, 2, 2, 2, P], FP8E4)
        nc.gpsimd.dma_start(
            out=wq8p_sb, in_=wq8p.rearrange("g hf p w m -> p g hf w m"))
        wk8p_sb = consts.tile([P, 2, 2, 2, P], FP8E4)
        nc.gpsimd.dma_start(
            out=wk8p_sb, in_=wk8p.rearrange("g hf p w m -> p g hf w m"))
        wv8_sb = consts.tile([P, 2, C], FP8E4)
        nc.gpsimd.dma_start(out=wv8_sb, in_=wv8.rearrange("w p m -> p w m"))

        bp_sb = consts.tile([P, 12, NKC], F32)
        nc.sync.dma_start(out=bp_sb,
                          in_=bpack.rearrange("n (kc p) -> p n kc", p=P))
        bq_sb = bp_sb[:, 0]
        bo_sb, bf2_sb = bp_sb[:, 2], bp_sb[:, 3]
        g1_sb, b1_sb, g2_sb, b2_sb = (bp_sb[:, i] for i in range(4, 8))
        bvb_sb = consts.tile([P, C], F32)
        nc.sync.dma_start(out=bvb_sb, in_=bvb[:])

        # late-needed weights issued after the attention-critical loads
        w1_sb = consts.tile([P, NKC, F], F32R)
        nc.gpsimd.dma_start(out=w1_sb,
                            in_=w1.rearrange("(kc p) m -> p kc m", p=P))
        w2_sb = consts.tile([P, NFC, C], F32R)
        nc.sync.dma_start(out=w2_sb,
                          in_=w2.rearrange("(kc p) m -> p kc m", p=P))
        ones1r = consts.tile([P, 1], F32R)      # LN-stats lhsT (f32r ones)
        nc.sync.dma_start(out=ones1r, in_=ones32[:])
        ones_rep = consts.tile([1, P], F32)     # K=1 row-replication lhsT
        nc.vector.memset(ones_rep, 1.0)
        ones132 = consts.tile([1, HD], F32R)    # rec band-broadcast lhsT
        nc.vector.memset(ones132, 1.0)
        eps_sb = consts.tile([P, 1], F32)
        nc.vector.memset(eps_sb, EPS)

        # persistent activations
        qT_sb = big.tile([P, NKC, SH], F32R)       # q^T (with bq), residual
        q8_sb = big.tile([P, 2, 2, SH], FP8E4)     # (band, g, half, q)
        k8_sb = big.tile([P, 2, 2, S], FP8E4)      # (band, g, half, s)
        v8_sb = big.tile([P, ND, 2, NH, HD + 1], FP8E4)  # (k, d, i, h, c|1)
        attn_sb = big.tile([P, NKC, SH], F32R)     # (attn@v)/cs + bv
        z_sb = big.tile([P, NKC, SH], F32R)        # LN1 output
        h1_sb = big.tile([P, NFC, SH], F32R)       # relu(ffn1)
        out_sb = big.tile([P, NKC, SH], F32)       # final
        r_sb = big.tile([P, NKC, SH], F32R)        # residual sums (LN inputs)

        # ones column of v8 (CS accumulator rows)
        nc.vector.memset(v8_sb[:, :, :, :, HD:HD + 1], 1.0)

        # ---------------- preamble: projections ----------------
        with tc.tile_pool(name="pre_k", bufs=2, space="PSUM") as pre_k, \
             tc.tile_pool(name="pre_v", bufs=3, space="PSUM") as pre_v:
            # q^T fp32 (residual): q[c',q] = sum_ch Wq[ch,c'] xT[ch,q]
            psq = pre_k.tile([P, 2, SH], F32, tag="ps")
            for mc in range(NKC):
                for kc in range(NKC):
                    nc.tensor.matmul(
                        psq[:, mc, :], wq_sb[:, kc, mc * P:(mc + 1) * P],
                        xT_sb[:, kc, :],
                        start=(kc == 0), stop=(kc == NKC - 1))
            for mc in range(NKC):
                nc.scalar.activation(out=qT_sb[:, mc, :], in_=psq[:, mc, :],
                                     func=AFT.Identity,
                                     bias=bq_sb[:, mc:mc + 1])
            # q8 in split-half band layout, via DoubleRow over channels
            for g in range(2):
                ps8 = pre_k.tile([P, 2, SH], F32, tag="ps")
                for hf in range(2):
                    nc.tensor.matmul(ps8[:, hf, :], wq8p_sb[:, g, hf],
                                     x8_sb, start=True, stop=True,
                                     perf_mode=DR)
                nc.gpsimd.tensor_copy(q8_sb[:, g], ps8)
            # k8: per (g, 512-key blk): two DoubleRow matmuls + one convert
            for g in range(2):
                for blk in range(NBLK):
                    psk = pre_k.tile([P, 2, SH], F32, tag="ps")
                    for hf in range(2):
                        nc.tensor.matmul(
                            psk[:, hf, :], wk8p_sb[:, g, hf],
                            y8_sb[:, :, blk * 512:(blk + 1) * 512],
                            start=True, stop=True, perf_mode=DR)
                    if (g * NBLK + blk) % 2 == 0:
                        nc.scalar.activation(
                            out=k8_sb[:, g, :, blk * 512:(blk + 1) * 512],
                            in_=psk, func=AFT.Copy)
                    else:
                        nc.gpsimd.tensor_copy(
                            k8_sb[:, g, :, blk * 512:(blk + 1) * 512], psk)
            # v8: per 128-key chunk, DoubleRow over channels; +bv fused
            for ck in range(S // P):
                d, i = ck // 2, ck % 2
                psv = pre_v.tile([P, C], F32, tag="psv")
                nc.tensor.matmul(psv, y8_sb[:, :, ck * P:(ck + 1) * P],
                                 wv8_sb, start=True, stop=True, perf_mode=DR)
                e = nc.vector if ck % 4 == 0 else nc.gpsimd
                e.tensor_add(
                    v8_sb[:, d, i, :, 0:HD],
                    psv.rearrange("p (h c) -> p h c", c=HD),
                    bvb_sb.rearrange("p (h c) -> p h c", c=HD))

        # ---------------- attention ----------------
        # strict round-robin of the exp across ACT / Pool / DVE
        rr = [nc.scalar, nc.gpsimd, nc.vector]

        with tc.tile_pool(name="st", bufs=6, space="PSUM") as st_pool, \
             tc.tile_pool(name="avcs", bufs=1, space="PSUM") as avcs_pool, \
             tc.tile_pool(name="at", bufs=8) as at_pool, \
             tc.tile_pool(name="nrm", bufs=1) as nrm_pool:

            def make_normalize(hg, g, jp, avcs):
                # attn_h = av_h / cs_h  (bv already folded into v8)
                def _norm():
                    rec_row = nrm_pool.tile([1, 2, SH], F32, tag="rec_row",
                                            name=f"rec_row_{hg}")
                    nc.vector.reciprocal_approx_fast(
                        out=rec_row, in_=avcs[HD:HD + 1, :, :])
                    rec_all = st_pool.tile([P, SH], F32, tag="st",
                                           name=f"rec_all_{hg}")
                    rr_r = rec_row.bitcast(F32R)
                    for jj in range(2):
                        j = 2 * jp + jj
                        nc.tensor.matmul(rec_all[32 * j:32 * (j + 1), :],
                                         ones132, rr_r[:, jj, :],
                                         start=True, stop=True,
                                         tile_position=(0, 32 * j))
                    for jj in range(2):
                        j = 2 * jp + jj
                        e = [nc.vector, nc.gpsimd][jj]
                        e.tensor_mul(attn_sb[32 * j:32 * (j + 1), g, :],
                                     avcs[0:HD, jj, :],
                                     rec_all[32 * j:32 * (j + 1), :])
                return _norm

            uu = 0
            pending_norm = None
            for hg in range(4):      # half-groups: 2 heads x 16 d-chunks
                g, jp = hg // 2, hg % 2
                avcs = avcs_pool.tile([HD + 1, 2, SH], F32, tag="avcs",
                                      name=f"avcs_{hg}")
                pend = []
                at = None
                for d in range(ND):
                    for jj in range(2):
                        j = 2 * jp + jj
                        for i in range(2):
                            st = st_pool.tile([P, SH], F32, tag="st")
                            nc.tensor.matmul(
                                st,
                                k8_sb[32 * j:32 * j + 16, g, :,
                                      256 * d + 128 * i:256 * d + 128 * i + 128],
                                q8_sb[32 * j:32 * j + 16, g],
                                start=True, stop=True, perf_mode=DR,
                                tile_position=(32 * j, 0))
                            if i == 0:
                                at = at_pool.tile([P, 2, SH], FP8E5,
                                                  tag="at")
                            e = rr[uu % 3]
                            uu += 1
                            if e is nc.scalar:
                                nc.scalar.activation(out=at[:, i, :], in_=st,
                                                     func=AFT.Exp,
                                                     scale=INV_SQRT_HD)
                            else:
                                e.tensor_scalar(out=at[:, i, :].bitcast(U8),
                                                in0=st,
                                                scalar1=HACK_A,
                                                scalar2=HACK_B,
                                                op0=ALU.mult, op1=ALU.add)
                        pend.append((d, jj, at))
                        if len(pend) > 3:
                            pd, pjj, pat = pend.pop(0)
                            nc.tensor.matmul(
                                avcs[:, pjj, :],
                                v8_sb[:, pd, :, 4 * g + 2 * jp + pjj, :],
                                pat, start=(pd == 0), stop=(pd == ND - 1),
                                perf_mode=DR)
                        if d == 1 and jj == 1 and pending_norm is not None:
                            pending_norm()
                            pending_norm = None
                for pd, pjj, pat in pend:
                    nc.tensor.matmul(
                        avcs[:, pjj, :],
                        v8_sb[:, pd, :, 4 * g + 2 * jp + pjj, :],
                        pat, start=(pd == 0), stop=(pd == ND - 1),
                        perf_mode=DR)
                pending_norm = make_normalize(hg, g, jp, avcs)
            pending_norm()

        # ---------------- tail: out-proj, LN1, FFN, LN2 ----------------
        with tc.tile_pool(name="mm", bufs=3, space="PSUM") as mm_pool, \
             tc.tile_pool(name="stat", bufs=1, space="PSUM") as stat_pool, \
             tc.tile_pool(name="rep", bufs=1, space="PSUM") as rep_pool, \
             tc.tile_pool(name="tl", bufs=2) as tl_pool, \
             tc.tile_pool(name="tr", bufs=1) as tr_pool:

            def layer_norm(x3, gamma, beta, out3):
                """out3 = LN(x3) over the channel axis (2 chunks of 128)."""
                mu_ps = stat_pool.tile([1, SH], F32, tag="mu")
                e2_ps = stat_pool.tile([1, SH], F32, tag="e2")
                for kc in range(NKC):
                    nc.tensor.matmul(mu_ps, ones1r, x3[:, kc, :],
                                     start=(kc == 0), stop=(kc == NKC - 1))
                for kc in range(NKC):
                    sq = tl_pool.tile([P, SH], F32R, tag="sq")
                    if kc == 0:
                        nc.scalar.activation(out=sq, in_=x3[:, kc, :],
                                             func=AFT.Square)
                    else:
                        nc.vector.tensor_mul(sq, x3[:, kc, :], x3[:, kc, :])
                    nc.tensor.matmul(e2_ps, ones1r, sq,
                                     start=(kc == 0), stop=(kc == NKC - 1))
                mu_row = tr_pool.tile([1, SH], F32, tag="mu_row")
                nc.vector.tensor_scalar_mul(out=mu_row, in0=mu_ps,
                                            scalar1=INV_C)
                mu2_row = tr_pool.tile([1, SH], F32, tag="mu2_row")
                nc.vector.tensor_mul(mu2_row, mu_row, mu_row)
                var_row = tr_pool.tile([1, SH], F32, tag="var_row")
                # var = E[x^2] - mu^2 = e2/C - mu^2
                nc.vector.scalar_tensor_tensor(
                    out=var_row, in0=e2_ps, scalar=INV_C, in1=mu2_row,
                    op0=ALU.mult, op1=ALU.subtract)
                std_row = tr_pool.tile([1, SH], F32, tag="std_row")
                nc.scalar.activation(out=std_row, in_=var_row, func=AFT.Sqrt,
                                     bias=eps_sb[:1, :])
                rstd_row = tr_pool.tile([1, SH], F32, tag="rstd_row")
                scr_row = tr_pool.tile([1, SH], F32, tag="mu2_row")
                nc.vector.reciprocal_approx_accurate(out=rstd_row,
                                                     in_=std_row,
                                                     scratch=scr_row)
                mu_rep = rep_pool.tile([P, SH], F32, tag="mu_rep")
                nc.tensor.matmul(mu_rep, ones_rep, mu_row,
                                 start=True, stop=True)
                rstd_rep = rep_pool.tile([P, SH], F32, tag="rstd_rep")
                nc.tensor.matmul(rstd_rep, ones_rep, rstd_row,
                                 start=True, stop=True)
                for kc in range(NKC):
                    t = tl_pool.tile([P, SH], F32, tag="t")
                    e1 = [nc.vector, nc.gpsimd][kc]
                    e1.tensor_sub(t, x3[:, kc, :], mu_rep)
                    t2 = tl_pool.tile([P, SH], F32, tag="t2")
                    e1.tensor_mul(t2, t, rstd_rep)
                    nc.vector.tensor_scalar(
                        out=out3[:, kc, :], in0=t2,
                        scalar1=gamma[:, kc:kc + 1],
                        scalar2=beta[:, kc:kc + 1],
                        op0=ALU.mult, op1=ALU.add)

            # out-projection + residual (r = qT + Wo^T attn + bo)
            for mc in range(NKC):
                ps = mm_pool.tile([P, SH], F32, tag="mm")
                for kc in range(NKC):
                    nc.tensor.matmul(
                        ps, wo_sb[:, kc, mc * P:(mc + 1) * P],
                        attn_sb[:, kc, :],
                        start=(kc == 0), stop=(kc == NKC - 1))
                o_t = tl_pool.tile([P, SH], F32, tag="o_t")
                nc.gpsimd.tensor_scalar_add(out=o_t, in0=ps,
                                            scalar1=bo_sb[:, mc:mc + 1])
                nc.vector.tensor_add(r_sb[:, mc, :], qT_sb[:, mc, :], o_t)

            layer_norm(r_sb, g1_sb, b1_sb, z_sb)

            # FFN1 + relu
            for mf in range(NFC):
                ps = mm_pool.tile([P, SH], F32, tag="mm")
                for kc in range(NKC):
                    nc.tensor.matmul(
                        ps, w1_sb[:, kc, mf * P:(mf + 1) * P],
                        z_sb[:, kc, :],
                        start=(kc == 0), stop=(kc == NKC - 1))
                if mf % 2 == 0:
                    nc.scalar.activation(
                        out=h1_sb[:, mf, :], in_=ps, func=AFT.Relu,
                        bias=bp_sb[:, 8 + mf // 2, mf % 2:mf % 2 + 1])
                else:
                    nc.vector.tensor_scalar(
                        out=h1_sb[:, mf, :], in0=ps,
                        scalar1=bp_sb[:, 8 + mf // 2, mf % 2:mf % 2 + 1],
                        scalar2=0.0,
                        op0=ALU.add, op1=ALU.max)
            # FFN2 + bias + residual
            for mc in range(NKC):
                ps = mm_pool.tile([P, SH], F32, tag="mm")
                for kf in range(NFC):
                    nc.tensor.matmul(
                        ps, w2_sb[:, kf, mc * P:(mc + 1) * P],
                        h1_sb[:, kf, :],
                        start=(kf == 0), stop=(kf == NFC - 1))
                f2 = tl_pool.tile([P, SH], F32, tag="f2")
                nc.gpsimd.tensor_scalar_add(out=f2, in0=ps,
                                            scalar1=bf2_sb[:, mc:mc + 1])
                nc.vector.tensor_add(r_sb[:, mc, :], z_sb[:, mc, :], f2)

            layer_norm(r_sb, g2_sb, b2_sb, out_sb)

            out_r = out.rearrange("(kc p) s -> p kc s", p=P)
            nc.sync.dma_start(out=out_r[:, 0, :], in_=out_sb[:, 0, :])
            nc.gpsimd.dma_start(out=out_r[:, 1, :], in_=out_sb[:, 1, :])


_NC_CACHE = None


def _get_nc():
    global _NC_CACHE
    if _NC_CACHE is None:
        _NC_CACHE = build_bass()
    return _NC_CACHE


FP8_NP = ml_dtypes.float8_e4m3


def _pack_qk8(W):
    """Permute+pad Wq/Wk columns into the [g, hf, chl, chh, m] fp8 layout.

    Column m = 32*j + p' (p' < 16) of pass (g, hf) holds original column
    c' = (4g + j)*32 + hf*16 + p'; columns with p' >= 16 are zero."""
    W8 = np.asarray(W, np.float32).astype(FP8_NP)
    outp = np.zeros((2, 2, P, 2, P), FP8_NP)
    for g in range(2):
        for hf in range(2):
            for j in range(4):
                cols = (4 * g + j) * 32 + hf * 16 + np.arange(16)
                blk = W8[:, cols]                       # [C, 16]
                blk = blk.reshape(2, P, 16)             # (chh, chl, p')
                outp[g, hf, :, :, 32 * j:32 * j + 16] = \
                    blk.transpose(1, 0, 2)
    return np.ascontiguousarray(outp)


def make_in_maps(lidar_features, image_features, Wq, bq, Wk, bk, Wv, bv,
                 Wo, bo, g1, b1, W1, bf1, W2, bf2, g2, b2):
    xT_full = np.ascontiguousarray(
        np.asarray(lidar_features, np.float32).reshape(C, S))
    y_full = np.ascontiguousarray(
        np.asarray(image_features, np.float32).reshape(C, S))
    wqo = np.ascontiguousarray(np.stack([
        np.asarray(Wq, np.float32), np.asarray(Wo, np.float32)]))
    bpack = np.ascontiguousarray(np.concatenate([
        np.asarray(bq, np.float32)[None], np.asarray(bv, np.float32)[None],
        np.asarray(bo, np.float32)[None], np.asarray(bf2, np.float32)[None],
        np.asarray(g1, np.float32)[None], np.asarray(b1, np.float32)[None],
        np.asarray(g2, np.float32)[None], np.asarray(b2, np.float32)[None],
        np.asarray(bf1, np.float32).reshape(4, C)]))
    wv8 = np.asarray(Wv, np.float32).astype(FP8_NP).reshape(2, P, C)
    bvb = np.broadcast_to(np.asarray(bv, np.float32)[None, :],
                          (P, C)).copy()
    common = {
        "y8": y_full.astype(FP8_NP),
        "wqo": wqo,
        "wq8p": _pack_qk8(Wq),
        "wk8p": _pack_qk8(Wk),
        "wv8": np.ascontiguousarray(wv8),
        "w1": np.ascontiguousarray(np.asarray(W1, np.float32)),
        "w2": np.ascontiguousarray(np.asarray(W2, np.float32)),
        "ones32": np.ones((P, 1), np.float32),
        "bpack": bpack,
        "bvb": bvb,
    }
    in_maps = []
    for c in range(NCORES):
        m = dict(common)
        shard = np.ascontiguousarray(xT_full[:, c * SH:(c + 1) * SH])
        m["xT"] = shard
        m["x8"] = shard.astype(FP8_NP)
        in_maps.append(m)
    return in_maps


def kernel(lidar_features, image_features, Wq, bq, Wk, bk, Wv, bv, Wo, bo,
           g1, b1, W1, bf1, W2, bf2, g2, b2, num_heads, **run_kwargs):
    assert int(num_heads) == NH
    nc = _get_nc()
    in_maps = make_in_maps(lidar_features, image_features, Wq, bq, Wk, bk,
                           Wv, bv, Wo, bo, g1, b1, W1, bf1, W2, bf2, g2, b2)
    res = run_bass_kernel_spmd(nc, in_maps, core_ids=list(range(NCORES)),
                               **run_kwargs)
    full = np.concatenate([res.results[c]["out"] for c in range(NCORES)],
                          axis=1)
    kernel.last_results = res
    return full.reshape(1, C, 64, 64).astype(np.float32)


kernel.last_results = None


# revision 22
# speedup vs baseline: 1.9144x; 1.0549x over previous
"""Cross-attention fusion block on 8 trn2 NeuronCores.

Sharding: data-parallel over the query sequence (S=4096 -> 512 rows/core).
K/V projections are computed redundantly on every core. Channel-major
layout [C, S] throughout; no on-chip transposes.

v2 design (vs baseline): fp8 DoubleRow matmuls for the attention phase and
the K/V/Q8 projections, and the softmax exp split across ACT (true exp ->
fp8e5) / DVE / Pool (Schraudolph bit-hack exp via uint8 write + fp8e5
bitcast).  Key layout trick: Wk/Wq columns are permuted+zero-padded on the
host so the projection matmul lands K/Q directly in the [16, 2(half), ...]
partition layout DoubleRow needs (head h in grp g at partition band
32*(h%4), head-dim split 16+16 across the DoubleRow free axis).

Per-core pipeline (q = 512 query rows of this core):
  qT   = Wq^T xT + bq                  [256, 512] fp32r   (residual path)
  q8   = perm(Wq8)^T x8                [bands, 2, 512] fp8e4
  k8   = perm(Wk8)^T y8                [bands, 2, 4096] fp8e4
  v8_h = [y8^T Wv8 + bv | 1]           per head [128, 2, 33] fp8e4
  per (grp g, head j, 256-key chunk d):
    ST[k, q] = k8_h-chunk DR@ q8_h                  (2 DoubleRow matmuls)
    AT       = approx-exp(ST/sqrt(32)) -> fp8e5     (ACT exp | DVE/Pool hack)
    AVCS_h  += v8_h-chunk DR@ AT                    [33, 512] psum
  attn_h = AV_h * (1/CS_h)      (CS rows DMA-gathered, reciprocal, emat
                                 broadcast matmul, per-head psum*rec mul)
  o = Wo^T attn + bo; r = qT + o; z = LN1(r)
  h1 = relu(W1^T z + bf1); h2 = W2^T h1 + bf2; out = LN2(z + h2)
"""

import sys

for _p in ("/opt/trn_rl_repo", "/opt/pypackages"):
    if _p not in sys.path:
        sys.path.append(_p)

import numpy as np
import ml_dtypes

import concourse.bass as bass
import concourse.bacc as bacc
import concourse.tile as tile
from concourse import mybir
from concourse.bass_utils import run_bass_kernel_spmd

F32 = mybir.dt.float32
F32R = mybir.dt.float32r
FP8E4 = mybir.dt.float8e4
FP8E5 = mybir.dt.float8e5
U8 = mybir.dt.uint8
AFT = mybir.ActivationFunctionType
ALU = mybir.AluOpType
DR = mybir.MatmulPerfMode.DoubleRow

P = 128           # SBUF partitions
C = 256           # channels
S = 4096          # sequence (64*64)
NCORES = 8
SH = S // NCORES  # 512 query rows per core
NH = 8            # heads
HD = 32           # head dim
F = 4 * C         # FFN hidden = 1024
NKC = C // P      # 2 channel chunks
NFC = F // P      # 8 ffn chunks
ND = S // 256     # 16 double-row key chunks
NBLK = S // 512   # 8 key blocks for kT production
EPS = 1e-5
INV_SQRT_HD = 1.0 / float(np.sqrt(HD))
INV_C = 1.0 / C
# Schraudolph-style exp for fp8e5(=e5m2) bitcast: i = floor(A*st + B)
HACK_A = float(4.0 * np.log2(np.e)) * INV_SQRT_HD
HACK_B = 60.02


def build_bass():
    nc = bacc.Bacc()

    xT = nc.declare_dram_parameter("xT", [C, SH], F32R, isOutput=False)
    x8 = nc.declare_dram_parameter("x8", [C, SH], FP8E4, isOutput=False)
    y8 = nc.declare_dram_parameter("y8", [C, S], FP8E4, isOutput=False)
    wqo = nc.declare_dram_parameter("wqo", [2, C, C], F32R, isOutput=False)
    wq8p = nc.declare_dram_parameter("wq8p", [2, 2, P, 2, P], FP8E4,
                                     isOutput=False)
    wk8p = nc.declare_dram_parameter("wk8p", [2, 2, P, 2, P], FP8E4,
                                     isOutput=False)
    wv8 = nc.declare_dram_parameter("wv8", [2, P, C], FP8E4, isOutput=False)
    w1 = nc.declare_dram_parameter("w1", [C, F], F32R, isOutput=False)
    w2 = nc.declare_dram_parameter("w2", [F, C], F32R, isOutput=False)
    ones32 = nc.declare_dram_parameter("ones32", [P, 1], F32R, isOutput=False)
    bpack = nc.declare_dram_parameter("bpack", [12, C], F32, isOutput=False)
    bvb = nc.declare_dram_parameter("bvb", [P, C], F32, isOutput=False)
    out = nc.declare_dram_parameter("out", [C, SH], F32, isOutput=True)

    with tile.TileContext(nc) as tc:
        _emit(tc, xT, x8, y8, wqo, wq8p, wk8p, wv8, w1, w2, ones32,
              bpack, bvb, out)
    if not nc.is_finalized():
        nc.finalize()
    return nc


def _emit(tc, xT, x8, y8, wqo, wq8p, wk8p, wv8, w1, w2, ones32,
          bpack, bvb, out):
    nc = tc.nc

    import contextlib
    stack = contextlib.ExitStack()
    with stack:
        consts = stack.enter_context(tc.tile_pool(name="consts", bufs=1))
        big = stack.enter_context(tc.tile_pool(name="big", bufs=1))

        # ---------------- constants / inputs into SBUF ----------------
        y8_sb = big.tile([P, NKC, S], FP8E4)      # y8[ch, s]; ch = kc*128+p
        y8_r = y8.rearrange("(kc p) s -> p kc s", p=P)
        HS = S // 2
        for sh2 in range(2):
            eng = [nc.sync, nc.gpsimd][sh2]
            eng.dma_start(out=y8_sb[:, :, sh2 * HS:(sh2 + 1) * HS],
                          in_=y8_r[:, :, sh2 * HS:(sh2 + 1) * HS])
        xT_sb = big.tile([P, NKC, SH], F32R)
        nc.sync.dma_start(out=xT_sb, in_=xT.rearrange("(kc p) s -> p kc s",
                                                      p=P))
        x8_sb = big.tile([P, NKC, SH], FP8E4)
        nc.gpsimd.dma_start(out=x8_sb, in_=x8.rearrange("(kc p) s -> p kc s",
                                                        p=P))

        wqo_sb = consts.tile([P, 2, NKC, C], F32R)
        nc.sync.dma_start(
            out=wqo_sb, in_=wqo.rearrange("w (kc p) m -> p w kc m", p=P))
        wq_sb, wo_sb = (wqo_sb[:, i] for i in range(2))
        wq8p_sb = consts.tile([P, 2, 2, 2, P], FP8E4)
        nc.gpsimd.dma_start(
            out=wq8p_sb, in_=wq8p.rearrange("g hf p w m -> p g hf w m"))
        wk8p_sb = consts.tile([P, 2, 2, 2, P], FP8E4)
        nc.gpsimd.dma_start(
            out=wk8p_sb, in_=wk8p.rearrange("g hf p w m -> p g hf w m"))
        wv8_sb = consts.tile([P, 2, C], FP8E4)
        nc.gpsimd.dma_start(out=wv8_sb, in_=wv8.rearrange("w p m -> p w m"))

        bp_sb = consts.tile([P, 12, NKC], F32)
        nc.sync.dma_start(out=bp_sb,
                          in_=bpack.rearrange("n (kc p) -> p n kc", p=P))
        bq_sb = bp_sb[:, 0]
        bo_sb, bf2_sb = bp_sb[:, 2], bp_sb[:, 3]
        g1_sb, b1_sb, g2_sb, b2_sb = (bp_sb[:, i] for i in range(4, 8))
        bvb_sb = consts.tile([P, C], F32)
        nc.sync.dma_start(out=bvb_sb, in_=bvb[:])

        # late-needed weights issued after the attention-critical loads
        w1_sb = consts.tile([P, NKC, F], F32R)
        nc.gpsimd.dma_start(out=w1_sb,
                            in_=w1.rearrange("(kc p) m -> p kc m", p=P))
        w2_sb = consts.tile([P, NFC, C], F32R)
        nc.sync.dma_start(out=w2_sb,
                          in_=w2.rearrange("(kc p) m -> p kc m", p=P))
        ones1r = consts.tile([P, 1], F32R)      # LN-stats lhsT (f32r ones)
        nc.sync.dma_start(out=ones1r, in_=ones32[:])
        ones_rep = consts.tile([1, P], F32)     # K=1 row-replication lhsT
        nc.vector.memset(ones_rep, 1.0)
        ones132 = consts.tile([1, HD], F32R)    # rec band-broadcast lhsT
        nc.vector.memset(ones132, 1.0)
        eps_sb = consts.tile([P, 1], F32)
        nc.vector.memset(eps_sb, EPS)

        # persistent activations
        qT_sb = big.tile([P, NKC, SH], F32R)       # q^T (with bq), residual
        q8_sb = big.tile([P, 2, 2, SH], FP8E4)     # (band, g, half, q)
        k8_sb = big.tile([P, 2, 2, S], FP8E4)      # (band, g, half, s)
        v8_sb = big.tile([P, ND, 2, NH, HD + 1], FP8E4)  # (k, d, i, h, c|1)
        attn_sb = big.tile([P, NKC, SH], F32R)     # (attn@v)/cs + bv
        z_sb = big.tile([P, NKC, SH], F32R)        # LN1 output
        h1_sb = big.tile([P, NFC, SH], F32R)       # relu(ffn1)
        out_sb = big.tile([P, NKC, SH], F32)       # final
        r_sb = big.tile([P, NKC, SH], F32R)        # residual sums (LN inputs)

        # ones column of v8 (CS accumulator rows)
        nc.vector.memset(v8_sb[:, :, :, :, HD:HD + 1], 1.0)

        # ---------------- preamble: projections ----------------
        with tc.tile_pool(name="pre_k", bufs=2, space="PSUM") as pre_k, \
             tc.tile_pool(name="pre_v", bufs=3, space="PSUM") as pre_v:
            # q^T fp32 (residual): q[c',q] = sum_ch Wq[ch,c'] xT[ch,q]
            psq = pre_k.tile([P, 2, SH], F32, tag="ps")
            for mc in range(NKC):
                for kc in range(NKC):
                    nc.tensor.matmul(
                        psq[:, mc, :], wq_sb[:, kc, mc * P:(mc + 1) * P],
                        xT_sb[:, kc, :],
                        start=(kc == 0), stop=(kc == NKC - 1))
            for mc in range(NKC):
                nc.scalar.activation(out=qT_sb[:, mc, :], in_=psq[:, mc, :],
                                     func=AFT.Identity,
                                     bias=bq_sb[:, mc:mc + 1])
            # q8 in split-half band layout, via DoubleRow over channels
            for g in range(2):
                ps8 = pre_k.tile([P, 2, SH], F32, tag="ps")
                for hf in range(2):
                    nc.tensor.matmul(ps8[:, hf, :], wq8p_sb[:, g, hf],
                                     x8_sb, start=True, stop=True,
                                     perf_mode=DR)
                nc.gpsimd.tensor_copy(q8_sb[:, g], ps8)
            # k8: per (g, 512-key blk): two DoubleRow matmuls + one convert
            for g in range(2):
                for blk in range(NBLK):
                    psk = pre_k.tile([P, 2, SH], F32, tag="ps")
                    for hf in range(2):
                        nc.tensor.matmul(
                            psk[:, hf, :], wk8p_sb[:, g, hf],
                            y8_sb[:, :, blk * 512:(blk + 1) * 512],
                            start=True, stop=True, perf_mode=DR)
                    if (g * NBLK + blk) % 2 == 0:
                        nc.scalar.activation(
                            out=k8_sb[:, g, :, blk * 512:(blk + 1) * 512],
                            in_=psk, func=AFT.Copy)
                    else:
                        nc.gpsimd.tensor_copy(
                            k8_sb[:, g, :, blk * 512:(blk + 1) * 512], psk)
            # v8: per 256-key d-chunk (two 128-key DoubleRow matmuls into one
            # psum bank), one fused convert+bias per chunk
            bvb2 = bvb_sb.rearrange("p (i h c) -> p i h c", i=1, c=HD)
            bvb2 = bass.AP(tensor=bvb2.tensor, offset=bvb2.offset,
                           ap=[bvb2.ap[0], [0, 2]] + bvb2.ap[2:])
            for d in range(ND):
                psv = pre_v.tile([P, 2, C], F32, tag="psv")
                for i in range(2):
                    ck = 2 * d + i
                    nc.tensor.matmul(psv[:, i, :],
                                     y8_sb[:, :, ck * P:(ck + 1) * P],
                                     wv8_sb, start=True, stop=True,
                                     perf_mode=DR)
                e = nc.vector if d % 4 == 0 else nc.gpsimd
                e.tensor_add(
                    v8_sb[:, d, :, :, 0:HD],
                    psv.rearrange("p i (h c) -> p i h c", c=HD),
                    bvb2)

        # ---------------- attention ----------------
        # weighted round-robin of the exp half-ops across ACT / Pool / DVE
        exp_w = [(nc.scalar, 0.88), (nc.gpsimd, 1.06), (nc.vector, 0.61)]
        credits = [0.0, 0.0, 0.0]
        rr = []
        for _ in range(4 * ND * 2 * 2):
            for ii in range(3):
                credits[ii] += exp_w[ii][1]
            pick = max(range(3), key=lambda ii: credits[ii])
            credits[pick] -= sum(w for _, w in exp_w)
            rr.append(exp_w[pick][0])

        with tc.tile_pool(name="st", bufs=6, space="PSUM") as st_pool, \
             tc.tile_pool(name="avcs", bufs=1, space="PSUM") as avcs_pool, \
             tc.tile_pool(name="at", bufs=8) as at_pool, \
             tc.tile_pool(name="nrm", bufs=1) as nrm_pool:

            def make_normalize(hg, g, jp, avcs):
                # attn_h = av_h / cs_h  (bv already folded into v8)
                def _norm():
                    rec_row = nrm_pool.tile([1, 2, SH], F32, tag="rec_row",
                                            name=f"rec_row_{hg}")
                    nc.vector.reciprocal_approx_fast(
                        out=rec_row, in_=avcs[HD:HD + 1, :, :])
                    rec_all = st_pool.tile([P, SH], F32, tag="st",
                                           name=f"rec_all_{hg}")
                    rr_r = rec_row.bitcast(F32R)
                    for jj in range(2):
                        j = 2 * jp + jj
                        nc.tensor.matmul(rec_all[32 * j:32 * (j + 1), :],
                                         ones132, rr_r[:, jj, :],
                                         start=True, stop=True,
                                         tile_position=(0, 32 * j))
                    for jj in range(2):
                        j = 2 * jp + jj
                        e = [nc.vector, nc.gpsimd][jj]
                        e.tensor_mul(attn_sb[32 * j:32 * (j + 1), g, :],
                                     avcs[0:HD, jj, :],
                                     rec_all[32 * j:32 * (j + 1), :])
                return _norm

            uu = 0
            pending_norm = None
            for hg in range(4):      # half-groups: 2 heads x 16 d-chunks
                g, jp = hg // 2, hg % 2
                avcs = avcs_pool.tile([HD + 1, 2, SH], F32, tag="avcs",
                                      name=f"avcs_{hg}")
                pend = []
                at = None
                for d in range(ND):
                    for jj in range(2):
                        j = 2 * jp + jj
                        for i in range(2):
                            st = st_pool.tile([P, SH], F32, tag="st")
                            nc.tensor.matmul(
                                st,
                                k8_sb[32 * j:32 * j + 16, g, :,
                                      256 * d + 128 * i:256 * d + 128 * i + 128],
                                q8_sb[32 * j:32 * j + 16, g],
                                start=True, stop=True, perf_mode=DR,
                                tile_position=(32 * j, 0))
                            if i == 0:
                                at = at_pool.tile([P, 2, SH], FP8E5,
                                                  tag="at")
                            e = rr[uu]
                            uu += 1
                            if e is nc.scalar:
                                nc.scalar.activation(out=at[:, i, :], in_=st,
                                                     func=AFT.Exp,
                                                     scale=INV_SQRT_HD)
                            else:
                                e.tensor_scalar(out=at[:, i, :].bitcast(U8),
                                                in0=st,
                                                scalar1=HACK_A,
                                                scalar2=HACK_B,
                                                op0=ALU.mult, op1=ALU.add)
                        pend.append((d, jj, at))
                        if len(pend) > 3:
                            pd, pjj, pat = pend.pop(0)
                            nc.tensor.matmul(
                                avcs[:, pjj, :],
                                v8_sb[:, pd, :, 4 * g + 2 * jp + pjj, :],
                                pat, start=(pd == 0), stop=(pd == ND - 1),
                                perf_mode=DR)
                        if d == 1 and jj == 1 and pending_norm is not None:
                            pending_norm()
                            pending_norm = None
                for pd, pjj, pat in pend:
                    nc.tensor.matmul(
                        avcs[:, pjj, :],
                        v8_sb[:, pd, :, 4 * g + 2 * jp + pjj, :],
                        pat, start=(pd == 0), stop=(pd == ND - 1),
                        perf_mode=DR)
                pending_norm = make_normalize(hg, g, jp, avcs)
            pending_norm()

        # ---------------- tail: out-proj, LN1, FFN, LN2 ----------------
        with tc.tile_pool(name="mm", bufs=3, space="PSUM") as mm_pool, \
             tc.tile_pool(name="stat", bufs=1, space="PSUM") as stat_pool, \
             tc.tile_pool(name="rep", bufs=1, space="PSUM") as rep_pool, \
             tc.tile_pool(name="tl", bufs=2) as tl_pool, \
             tc.tile_pool(name="tr", bufs=1) as tr_pool:

            def layer_norm(x3, gamma, beta, out3):
                """out3 = LN(x3) over the channel axis (2 chunks of 128)."""
                mu_ps = stat_pool.tile([1, SH], F32, tag="mu")
                e2_ps = stat_pool.tile([1, SH], F32, tag="e2")
                for kc in range(NKC):
                    nc.tensor.matmul(mu_ps, ones1r, x3[:, kc, :],
                                     start=(kc == 0), stop=(kc == NKC - 1))
                for kc in range(NKC):
                    sq = tl_pool.tile([P, SH], F32R, tag="sq")
                    if kc == 0:
                        nc.scalar.activation(out=sq, in_=x3[:, kc, :],
                                             func=AFT.Square)
                    else:
                        nc.gpsimd.tensor_mul(sq, x3[:, kc, :], x3[:, kc, :])
                    nc.tensor.matmul(e2_ps, ones1r, sq,
                                     start=(kc == 0), stop=(kc == NKC - 1))
                mu_row = tr_pool.tile([1, SH], F32, tag="mu_row")
                nc.vector.tensor_scalar_mul(out=mu_row, in0=mu_ps,
                                            scalar1=INV_C)
                mu2_row = tr_pool.tile([1, SH], F32, tag="mu2_row")
                nc.vector.tensor_mul(mu2_row, mu_row, mu_row)
                var_row = tr_pool.tile([1, SH], F32, tag="var_row")
                # var = E[x^2] - mu^2 = e2/C - mu^2
                nc.vector.scalar_tensor_tensor(
                    out=var_row, in0=e2_ps, scalar=INV_C, in1=mu2_row,
                    op0=ALU.mult, op1=ALU.subtract)
                std_row = tr_pool.tile([1, SH], F32, tag="std_row")
                nc.scalar.activation(out=std_row, in_=var_row, func=AFT.Sqrt,
                                     bias=eps_sb[:1, :])
                rstd_row = tr_pool.tile([1, SH], F32, tag="rstd_row")
                scr_row = tr_pool.tile([1, SH], F32, tag="mu2_row")
                nc.vector.reciprocal_approx_accurate(out=rstd_row,
                                                     in_=std_row,
                                                     scratch=scr_row)
                mu_rep = rep_pool.tile([P, SH], F32, tag="mu_rep")
                nc.tensor.matmul(mu_rep, ones_rep, mu_row,
                                 start=True, stop=True)
                rstd_rep = rep_pool.tile([P, SH], F32, tag="rstd_rep")
                nc.tensor.matmul(rstd_rep, ones_rep, rstd_row,
                                 start=True, stop=True)
                for kc in range(NKC):
                    t = tl_pool.tile([P, SH], F32, tag="t")
                    e1 = [nc.vector, nc.gpsimd][kc]
                    e1.tensor_sub(t, x3[:, kc, :], mu_rep)
                    # t2 = (t * gamma) * rstd_rep
                    t2 = tl_pool.tile([P, SH], F32, tag="t2")
                    e1.scalar_tensor_tensor(
                        out=t2, in0=t, scalar=gamma[:, kc:kc + 1],
                        in1=rstd_rep, op0=ALU.mult, op1=ALU.mult)
                    e2 = [nc.gpsimd, nc.vector][kc]
                    e2.tensor_scalar_add(out=out3[:, kc, :], in0=t2,
                                         scalar1=beta[:, kc:kc + 1])

            # out-projection + residual (r = qT + Wo^T attn + bo)
            for mc in range(NKC):
                ps = mm_pool.tile([P, SH], F32, tag="mm")
                for kc in range(NKC):
                    nc.tensor.matmul(
                        ps, wo_sb[:, kc, mc * P:(mc + 1) * P],
                        attn_sb[:, kc, :],
                        start=(kc == 0), stop=(kc == NKC - 1))
                # r = (ps + bo) + qT in one fused op
                e = [nc.vector, nc.gpsimd][mc]
                e.scalar_tensor_tensor(
                    out=r_sb[:, mc, :], in0=ps, scalar=bo_sb[:, mc:mc + 1],
                    in1=qT_sb[:, mc, :], op0=ALU.add, op1=ALU.add)

            layer_norm(r_sb, g1_sb, b1_sb, z_sb)

            # FFN1 + relu
            for mf in range(NFC):
                ps = mm_pool.tile([P, SH], F32, tag="mm")
                for kc in range(NKC):
                    nc.tensor.matmul(
                        ps, w1_sb[:, kc, mf * P:(mf + 1) * P],
                        z_sb[:, kc, :],
                        start=(kc == 0), stop=(kc == NKC - 1))
                if mf % 2 == 0:
                    nc.scalar.activation(
                        out=h1_sb[:, mf, :], in_=ps, func=AFT.Relu,
                        bias=bp_sb[:, 8 + mf // 2, mf % 2:mf % 2 + 1])
                else:
                    nc.vector.tensor_scalar(
                        out=h1_sb[:, mf, :], in0=ps,
                        scalar1=bp_sb[:, 8 + mf // 2, mf % 2:mf % 2 + 1],
                        scalar2=0.0,
                        op0=ALU.add, op1=ALU.max)
            # FFN2 + bias + residual
            for mc in range(NKC):
                ps = mm_pool.tile([P, SH], F32, tag="mm")
                for kf in range(NFC):
                    nc.tensor.matmul(
                        ps, w2_sb[:, kf, mc * P:(mc + 1) * P],
                        h1_sb[:, kf, :],
                        start=(kf == 0), stop=(kf == NFC - 1))
                # r = (ps + bf2) + z in one fused op
                e = [nc.vector, nc.gpsimd][mc]
                e.scalar_tensor_tensor(
                    out=r_sb[:, mc, :], in0=ps, scalar=bf2_sb[:, mc:mc + 1],
                    in1=z_sb[:, mc, :], op0=ALU.add, op1=ALU.add)

            layer_norm(r_sb, g2_sb, b2_sb, out_sb)

            out_r = out.rearrange("(kc p) s -> p kc s", p=P)
            nc.sync.dma_start(out=out_r[:, 0, :], in_=out_sb[:, 0, :])
            nc.gpsimd.dma_start(out=out_r[:, 1, :], in_=out_sb[:, 1, :])


_NC_CACHE = None


def _get_nc():
    global _NC_CACHE
    if _NC_CACHE is None:
        _NC_CACHE = build_bass()
    return _NC_CACHE


FP8_NP = ml_dtypes.float8_e4m3


def _pack_qk8(W):
    """Permute+pad Wq/Wk columns into the [g, hf, chl, chh, m] fp8 layout.

    Column m = 32*j + p' (p' < 16) of pass (g, hf) holds original column
    c' = (4g + j)*32 + hf*16 + p'; columns with p' >= 16 are zero."""
    W8 = np.asarray(W, np.float32).astype(FP8_NP)
    outp = np.zeros((2, 2, P, 2, P), FP8_NP)
    for g in range(2):
        for hf in range(2):
            for j in range(4):
                cols = (4 * g + j) * 32 + hf * 16 + np.arange(16)
                blk = W8[:, cols]                       # [C, 16]
                blk = blk.reshape(2, P, 16)             # (chh, chl, p')
                outp[g, hf, :, :, 32 * j:32 * j + 16] = \
                    blk.transpose(1, 0, 2)
    return np.ascontiguousarray(outp)


def make_in_maps(lidar_features, image_features, Wq, bq, Wk, bk, Wv, bv,
                 Wo, bo, g1, b1, W1, bf1, W2, bf2, g2, b2):
    xT_full = np.ascontiguousarray(
        np.asarray(lidar_features, np.float32).reshape(C, S))
    y_full = np.ascontiguousarray(
        np.asarray(image_features, np.float32).reshape(C, S))
    wqo = np.ascontiguousarray(np.stack([
        np.asarray(Wq, np.float32), np.asarray(Wo, np.float32)]))
    bpack = np.ascontiguousarray(np.concatenate([
        np.asarray(bq, np.float32)[None], np.asarray(bv, np.float32)[None],
        np.asarray(bo, np.float32)[None], np.asarray(bf2, np.float32)[None],
        np.asarray(g1, np.float32)[None], np.asarray(b1, np.float32)[None],
        np.asarray(g2, np.float32)[None], np.asarray(b2, np.float32)[None],
        np.asarray(bf1, np.float32).reshape(4, C)]))
    wv8 = np.asarray(Wv, np.float32).astype(FP8_NP).reshape(2, P, C)
    bvb = np.broadcast_to(np.asarray(bv, np.float32)[None, :],
                          (P, C)).copy()
    common = {
        "y8": y_full.astype(FP8_NP),
        "wqo": wqo,
        "wq8p": _pack_qk8(Wq),
        "wk8p": _pack_qk8(Wk),
        "wv8": np.ascontiguousarray(wv8),
        "w1": np.ascontiguousarray(np.asarray(W1, np.float32)),
        "w2": np.ascontiguousarray(np.asarray(W2, np.float32)),
        "ones32": np.ones((P, 1), np.float32),
        "bpack": bpack,
        "bvb": bvb,
    }
    in_maps = []
    for c in range(NCORES):
        m = dict(common)
        shard = np.ascontiguousarray(xT_full[:, c * SH:(c + 1) * SH])
        m["xT"] = shard
        m["x8"] = shard.astype(FP8_NP)
        in_maps.append(m)
    return in_maps


def kernel(lidar_features, image_features, Wq, bq, Wk, bk, Wv, bv, Wo, bo,
           g1, b1, W1, bf1, W2, bf2, g2, b2, num_heads, **run_kwargs):
    assert int(num_heads) == NH
    nc = _get_nc()
    in_maps = make_in_maps(lidar_features, image_features, Wq, bq, Wk, bk,
                           Wv, bv, Wo, bo, g1, b1, W1, bf1, W2, bf2, g2, b2)
    res = run_bass_kernel_spmd(nc, in_maps, core_ids=list(range(NCORES)),
                               **run_kwargs)
    full = np.concatenate([res.results[c]["out"] for c in range(NCORES)],
                          axis=1)
    kernel.last_results = res
    return full.reshape(1, C, 64, 64).astype(np.float32)


kernel.last_results = None


# revision 30
# speedup vs baseline: 1.9395x; 1.0131x over previous
"""Cross-attention fusion block on 8 trn2 NeuronCores.

Sharding: data-parallel over the query sequence (S=4096 -> 512 rows/core).
K/V projections are computed redundantly on every core. Channel-major
layout [C, S] throughout; no on-chip transposes.

v2 design (vs baseline): fp8 DoubleRow matmuls for the attention phase and
the K/V/Q8 projections, and the softmax exp split across ACT (true exp ->
fp8e5) / DVE / Pool (Schraudolph bit-hack exp via uint8 write + fp8e5
bitcast).  Key layout trick: Wk/Wq columns are permuted+zero-padded on the
host so the projection matmul lands K/Q directly in the [16, 2(half), ...]
partition layout DoubleRow needs (head h in grp g at partition band
32*(h%4), head-dim split 16+16 across the DoubleRow free axis).

Per-core pipeline (q = 512 query rows of this core):
  qT   = Wq^T xT + bq                  [256, 512] fp32r   (residual path)
  q8   = perm(Wq8)^T x8                [bands, 2, 512] fp8e4
  k8   = perm(Wk8)^T y8                [bands, 2, 4096] fp8e4
  v8_h = [y8^T Wv8 + bv | 1]           per head [128, 2, 33] fp8e4
  per (grp g, head j, 256-key chunk d):
    ST[k, q] = k8_h-chunk DR@ q8_h                  (2 DoubleRow matmuls)
    AT       = approx-exp(ST/sqrt(32)) -> fp8e5     (ACT exp | DVE/Pool hack)
    AVCS_h  += v8_h-chunk DR@ AT                    [33, 512] psum
  attn_h = AV_h * (1/CS_h)      (CS rows DMA-gathered, reciprocal, emat
                                 broadcast matmul, per-head psum*rec mul)
  o = Wo^T attn + bo; r = qT + o; z = LN1(r)
  h1 = relu(W1^T z + bf1); h2 = W2^T h1 + bf2; out = LN2(z + h2)
"""

import sys

for _p in ("/opt/trn_rl_repo", "/opt/pypackages"):
    if _p not in sys.path:
        sys.path.append(_p)

import numpy as np
import ml_dtypes

import concourse.bass as bass
import concourse.bacc as bacc
import concourse.tile as tile
from concourse import mybir
from concourse.bass_utils import run_bass_kernel_spmd

F32 = mybir.dt.float32
F32R = mybir.dt.float32r
FP8E4 = mybir.dt.float8e4
FP8E5 = mybir.dt.float8e5
U8 = mybir.dt.uint8
AFT = mybir.ActivationFunctionType
ALU = mybir.AluOpType
DR = mybir.MatmulPerfMode.DoubleRow

P = 128           # SBUF partitions
C = 256           # channels
S = 4096          # sequence (64*64)
NCORES = 8
SH = S // NCORES  # 512 query rows per core
NH = 8            # heads
HD = 32           # head dim
F = 4 * C         # FFN hidden = 1024
NKC = C // P      # 2 channel chunks
NFC = F // P      # 8 ffn chunks
ND = S // 256     # 16 double-row key chunks
NBLK = S // 512   # 8 key blocks for kT production
EPS = 1e-5
INV_SQRT_HD = 1.0 / float(np.sqrt(HD))
INV_C = 1.0 / C
# Schraudolph-style exp for fp8e5(=e5m2) bitcast: i = floor(A*st + B)
HACK_A = float(4.0 * np.log2(np.e)) * INV_SQRT_HD
HACK_B = 60.02


def build_bass():
    nc = bacc.Bacc()

    xT = nc.declare_dram_parameter("xT", [C, SH], F32R, isOutput=False)
    x8 = nc.declare_dram_parameter("x8", [C, SH], FP8E4, isOutput=False)
    y8 = nc.declare_dram_parameter("y8", [C, S], FP8E4, isOutput=False)
    wqo = nc.declare_dram_parameter("wqo", [2, C, C], F32R, isOutput=False)
    wq8p = nc.declare_dram_parameter("wq8p", [2, 2, P, 2, P], FP8E4,
                                     isOutput=False)
    wk8p = nc.declare_dram_parameter("wk8p", [2, 2, P, 2, P], FP8E4,
                                     isOutput=False)
    wv8 = nc.declare_dram_parameter("wv8", [2, P, C], FP8E4, isOutput=False)
    w1 = nc.declare_dram_parameter("w1", [C, F], F32R, isOutput=False)
    w2 = nc.declare_dram_parameter("w2", [F, C], F32R, isOutput=False)
    ones32 = nc.declare_dram_parameter("ones32", [P, 1], F32R, isOutput=False)
    bpack = nc.declare_dram_parameter("bpack", [12, C], F32, isOutput=False)
    bvb = nc.declare_dram_parameter("bvb", [P, C], F32, isOutput=False)
    out = nc.declare_dram_parameter("out", [C, SH], F32, isOutput=True)

    with tile.TileContext(nc) as tc:
        _emit(tc, xT, x8, y8, wqo, wq8p, wk8p, wv8, w1, w2, ones32,
              bpack, bvb, out)
    if not nc.is_finalized():
        nc.finalize()
    return nc


def _emit(tc, xT, x8, y8, wqo, wq8p, wk8p, wv8, w1, w2, ones32,
          bpack, bvb, out):
    nc = tc.nc

    import contextlib
    stack = contextlib.ExitStack()
    with stack:
        consts = stack.enter_context(tc.tile_pool(name="consts", bufs=1))
        big = stack.enter_context(tc.tile_pool(name="big", bufs=1))

        # ---------------- constants / inputs into SBUF ----------------
        y8_sb = big.tile([P, NKC, S], FP8E4)      # y8[ch, s]; ch = kc*128+p
        y8_r = y8.rearrange("(kc p) s -> p kc s", p=P)
        HS = S // 2
        for sh2 in range(2):
            eng = [nc.sync, nc.gpsimd][sh2]
            eng.dma_start(out=y8_sb[:, :, sh2 * HS:(sh2 + 1) * HS],
                          in_=y8_r[:, :, sh2 * HS:(sh2 + 1) * HS])
        xT_sb = big.tile([P, NKC, SH], F32R)
        nc.sync.dma_start(out=xT_sb, in_=xT.rearrange("(kc p) s -> p kc s",
                                                      p=P))
        x8_sb = big.tile([P, NKC, SH], FP8E4)
        nc.gpsimd.dma_start(out=x8_sb, in_=x8.rearrange("(kc p) s -> p kc s",
                                                        p=P))

        wqo_sb = consts.tile([P, 2, NKC, C], F32R)
        nc.sync.dma_start(
            out=wqo_sb, in_=wqo.rearrange("w (kc p) m -> p w kc m", p=P))
        wq_sb, wo_sb = (wqo_sb[:, i] for i in range(2))
        wq8p_sb = consts.tile([P, 2, 2, 2, P], FP8E4)
        nc.gpsimd.dma_start(
            out=wq8p_sb, in_=wq8p.rearrange("g hf p w m -> p g hf w m"))
        wk8p_sb = consts.tile([P, 2, 2, 2, P], FP8E4)
        nc.gpsimd.dma_start(
            out=wk8p_sb, in_=wk8p.rearrange("g hf p w m -> p g hf w m"))
        wv8_sb = consts.tile([P, 2, C], FP8E4)
        nc.gpsimd.dma_start(out=wv8_sb, in_=wv8.rearrange("w p m -> p w m"))

        bp_sb = consts.tile([P, 12, NKC], F32)
        nc.sync.dma_start(out=bp_sb,
                          in_=bpack.rearrange("n (kc p) -> p n kc", p=P))
        bq_sb = bp_sb[:, 0]
        bo_sb, bf2_sb = bp_sb[:, 2], bp_sb[:, 3]
        g1_sb, b1_sb, g2_sb, b2_sb = (bp_sb[:, i] for i in range(4, 8))
        bvb_sb = consts.tile([P, C], F32)
        nc.sync.dma_start(out=bvb_sb, in_=bvb[:])

        # late-needed weights issued after the attention-critical loads
        w1_sb = consts.tile([P, NKC, F], F32R)
        nc.gpsimd.dma_start(out=w1_sb,
                            in_=w1.rearrange("(kc p) m -> p kc m", p=P))
        w2_sb = consts.tile([P, NFC, C], F32R)
        nc.sync.dma_start(out=w2_sb,
                          in_=w2.rearrange("(kc p) m -> p kc m", p=P))
        ones1r = consts.tile([P, 1], F32R)      # LN-stats lhsT (f32r ones)
        nc.sync.dma_start(out=ones1r, in_=ones32[:])
        ones_rep = consts.tile([1, P], F32R)    # K=1 row-replication lhsT
        nc.vector.memset(ones_rep, 1.0)
        ones132 = consts.tile([1, HD], F32R)    # rec band-broadcast lhsT
        nc.vector.memset(ones132, 1.0)
        eps_sb = consts.tile([P, 1], F32)
        nc.vector.memset(eps_sb, EPS)

        # persistent activations
        qT_sb = big.tile([P, NKC, SH], F32R)       # q^T (with bq), residual
        q8_sb = big.tile([P, 2, 2, SH], FP8E4)     # (band, g, half, q)
        k8_sb = big.tile([P, 2, 2, S], FP8E4)      # (band, g, half, s)
        v8_sb = big.tile([P, ND, 2, NH, HD + 1], FP8E4)  # (k, d, i, h, c|1)
        attn_sb = big.tile([P, NKC, SH], F32R)     # (attn@v)/cs + bv
        z_sb = big.tile([P, NKC, SH], F32R)        # LN1 output
        h1_sb = big.tile([P, NFC, SH], F32R)       # relu(ffn1)
        out_sb = big.tile([P, NKC, SH], F32)       # final
        r_sb = big.tile([P, NKC, SH], F32R)        # residual sums (LN inputs)

        # ones column of v8 (CS accumulator rows)
        nc.vector.memset(v8_sb[:, :, :, :, HD:HD + 1], 1.0)

        # ---------------- preamble: projections ----------------
        with tc.tile_pool(name="pre_k", bufs=2, space="PSUM") as pre_k, \
             tc.tile_pool(name="pre_v", bufs=3, space="PSUM") as pre_v:
            # q^T fp32 (residual): q[c',q] = sum_ch Wq[ch,c'] xT[ch,q]
            psq = pre_k.tile([P, 2, SH], F32, tag="ps")
            for mc in range(NKC):
                for kc in range(NKC):
                    nc.tensor.matmul(
                        psq[:, mc, :], wq_sb[:, kc, mc * P:(mc + 1) * P],
                        xT_sb[:, kc, :],
                        start=(kc == 0), stop=(kc == NKC - 1))
            for mc in range(NKC):
                nc.scalar.activation(out=qT_sb[:, mc, :], in_=psq[:, mc, :],
                                     func=AFT.Identity,
                                     bias=bq_sb[:, mc:mc + 1])
            # q8 in split-half band layout, via DoubleRow over channels
            for g in range(2):
                ps8 = pre_k.tile([P, 2, SH], F32, tag="ps")
                for hf in range(2):
                    nc.tensor.matmul(ps8[:, hf, :], wq8p_sb[:, g, hf],
                                     x8_sb, start=True, stop=True,
                                     perf_mode=DR)
                [nc.vector, nc.gpsimd][g].tensor_copy(q8_sb[:, g], ps8)
            # k8: per (g, 512-key blk): two DoubleRow matmuls + one convert
            for g in range(2):
                for blk in range(NBLK):
                    psk = pre_k.tile([P, 2, SH], F32, tag="ps")
                    for hf in range(2):
                        nc.tensor.matmul(
                            psk[:, hf, :], wk8p_sb[:, g, hf],
                            y8_sb[:, :, blk * 512:(blk + 1) * 512],
                            start=True, stop=True, perf_mode=DR)
                    ke = [nc.scalar, nc.vector, nc.gpsimd][(g * NBLK + blk) % 3]
                    if ke is nc.scalar:
                        nc.scalar.activation(
                            out=k8_sb[:, g, :, blk * 512:(blk + 1) * 512],
                            in_=psk, func=AFT.Copy)
                    else:
                        ke.tensor_copy(
                            k8_sb[:, g, :, blk * 512:(blk + 1) * 512], psk)
            # v8: per 256-key d-chunk (two 128-key DoubleRow matmuls into one
            # psum bank), one fused convert+bias per chunk
            bvb2 = bvb_sb.rearrange("p (i h c) -> p i h c", i=1, c=HD)
            bvb2 = bass.AP(tensor=bvb2.tensor, offset=bvb2.offset,
                           ap=[bvb2.ap[0], [0, 2]] + bvb2.ap[2:])
            for d in range(ND):
                psv = pre_v.tile([P, 2, C], F32, tag="psv")
                for i in range(2):
                    ck = 2 * d + i
                    nc.tensor.matmul(psv[:, i, :],
                                     y8_sb[:, :, ck * P:(ck + 1) * P],
                                     wv8_sb, start=True, stop=True,
                                     perf_mode=DR)
                e = nc.vector if d % 2 == 0 else nc.gpsimd
                e.tensor_add(
                    v8_sb[:, d, :, :, 0:HD],
                    psv.rearrange("p i (h c) -> p i h c", c=HD),
                    bvb2)

        # ---------------- attention ----------------
        # weighted round-robin of the exp half-ops across ACT / Pool / DVE
        exp_w = [(nc.scalar, 0.82), (nc.gpsimd, 1.08), (nc.vector, 0.66)]
        credits = [0.0, 0.0, 0.0]
        rr = []
        for _ in range(4 * ND * 2 * 2):
            for ii in range(3):
                credits[ii] += exp_w[ii][1]
            pick = max(range(3), key=lambda ii: credits[ii])
            credits[pick] -= sum(w for _, w in exp_w)
            rr.append(exp_w[pick][0])

        with tc.tile_pool(name="st", bufs=6, space="PSUM") as st_pool, \
             tc.tile_pool(name="avcs", bufs=1, space="PSUM") as avcs_pool, \
             tc.tile_pool(name="at", bufs=8) as at_pool, \
             tc.tile_pool(name="nrm", bufs=1) as nrm_pool:

            def make_normalize(hg, g, jp, avcs):
                # attn_h = av_h / cs_h  (bv already folded into v8)
                def _norm():
                    rec_row = nrm_pool.tile([1, 2, SH], F32, tag="rec_row",
                                            name=f"rec_row_{hg}")
                    nc.vector.reciprocal_approx_fast(
                        out=rec_row, in_=avcs[HD:HD + 1, :, :])
                    rec_all = st_pool.tile([P, SH], F32, tag="st",
                                           name=f"rec_all_{hg}")
                    rr_r = rec_row.bitcast(F32R)
                    for jj in range(2):
                        j = 2 * jp + jj
                        nc.tensor.matmul(rec_all[32 * j:32 * (j + 1), :],
                                         ones132, rr_r[:, jj, :],
                                         start=True, stop=True,
                                         tile_position=(0, 32 * j))
                    for jj in range(2):
                        j = 2 * jp + jj
                        e = [nc.vector, nc.gpsimd][(hg + jj) % 2]
                        e.tensor_mul(attn_sb[32 * j:32 * (j + 1), g, :],
                                     avcs[0:HD, jj, :],
                                     rec_all[32 * j:32 * (j + 1), :])
                return _norm

            uu = 0
            pending_norm = None
            for hg in range(4):      # half-groups: 2 heads x 16 d-chunks
                g, jp = hg // 2, hg % 2
                avcs = avcs_pool.tile([HD + 1, 2, SH], F32, tag="avcs",
                                      name=f"avcs_{hg}")
                pend = []
                at = None
                for d in range(ND):
                    for jj in range(2):
                        j = 2 * jp + jj
                        for i in range(2):
                            st = st_pool.tile([P, SH], F32, tag="st")
                            nc.tensor.matmul(
                                st,
                                k8_sb[32 * j:32 * j + 16, g, :,
                                      256 * d + 128 * i:256 * d + 128 * i + 128],
                                q8_sb[32 * j:32 * j + 16, g],
                                start=True, stop=True, perf_mode=DR,
                                tile_position=(32 * j, 0))
                            if i == 0:
                                at = at_pool.tile([P, 2, SH], FP8E5,
                                                  tag="at")
                            e = rr[uu]
                            uu += 1
                            if e is nc.scalar:
                                nc.scalar.activation(out=at[:, i, :], in_=st,
                                                     func=AFT.Exp,
                                                     scale=INV_SQRT_HD)
                            else:
                                e.tensor_scalar(out=at[:, i, :].bitcast(U8),
                                                in0=st,
                                                scalar1=HACK_A,
                                                scalar2=HACK_B,
                                                op0=ALU.mult, op1=ALU.add)
                        pend.append((d, jj, at))
                        if len(pend) > 3:
                            pd, pjj, pat = pend.pop(0)
                            nc.tensor.matmul(
                                avcs[:, pjj, :],
                                v8_sb[:, pd, :, 4 * g + 2 * jp + pjj, :],
                                pat, start=(pd == 0), stop=(pd == ND - 1),
                                perf_mode=DR)
                        if d == 1 and jj == 1 and pending_norm is not None:
                            pending_norm()
                            pending_norm = None
                for pd, pjj, pat in pend:
                    nc.tensor.matmul(
                        avcs[:, pjj, :],
                        v8_sb[:, pd, :, 4 * g + 2 * jp + pjj, :],
                        pat, start=(pd == 0), stop=(pd == ND - 1),
                        perf_mode=DR)
                pending_norm = make_normalize(hg, g, jp, avcs)
            pending_norm()

        # ---------------- tail: out-proj, LN1, FFN, LN2 ----------------
        with tc.tile_pool(name="mm", bufs=3, space="PSUM") as mm_pool, \
             tc.tile_pool(name="stat", bufs=1, space="PSUM") as stat_pool, \
             tc.tile_pool(name="rep", bufs=1, space="PSUM") as rep_pool, \
             tc.tile_pool(name="tl", bufs=2) as tl_pool, \
             tc.tile_pool(name="tr", bufs=1) as tr_pool:

            # dummy ops to pull the Sqrt/Square act-table load off the LN
            # critical chain (executes while ACT is otherwise idle)
            warm = tr_pool.tile([1, 1], F32, tag="warm")
            nc.scalar.activation(out=warm, in_=eps_sb[:1, :], func=AFT.Square)
            nc.scalar.activation(out=warm, in_=eps_sb[:1, :], func=AFT.Sqrt)

            def layer_norm(x3, gamma, beta, out3):
                """out3 = LN(x3) over the channel axis (2 chunks of 128)."""
                mu_ps = stat_pool.tile([1, SH], F32, tag="mu")
                e2_ps = stat_pool.tile([1, SH], F32, tag="e2")
                for kc in range(NKC):
                    nc.tensor.matmul(mu_ps, ones1r, x3[:, kc, :],
                                     start=(kc == 0), stop=(kc == NKC - 1))
                for kc in range(NKC):
                    sq = tl_pool.tile([P, SH], F32R, tag="sq")
                    if kc == 0:
                        nc.scalar.activation(out=sq, in_=x3[:, kc, :],
                                             func=AFT.Square)
                    else:
                        nc.gpsimd.tensor_mul(sq, x3[:, kc, :], x3[:, kc, :])
                    nc.tensor.matmul(e2_ps, ones1r, sq,
                                     start=(kc == 0), stop=(kc == NKC - 1))
                mu_row = tr_pool.tile([1, SH], F32, tag="mu_row")
                nc.vector.tensor_scalar_mul(out=mu_row, in0=mu_ps,
                                            scalar1=INV_C)
                mu2_row = tr_pool.tile([1, SH], F32, tag="mu2_row")
                nc.vector.tensor_mul(mu2_row, mu_row, mu_row)
                var_row = tr_pool.tile([1, SH], F32, tag="var_row")
                # var = E[x^2] - mu^2 = e2/C - mu^2
                nc.vector.scalar_tensor_tensor(
                    out=var_row, in0=e2_ps, scalar=INV_C, in1=mu2_row,
                    op0=ALU.mult, op1=ALU.subtract)
                std_row = tr_pool.tile([1, SH], F32, tag="std_row")
                nc.scalar.activation(out=std_row, in_=var_row, func=AFT.Sqrt,
                                     bias=eps_sb[:1, :])
                rstd_row = tr_pool.tile([1, SH], F32, tag="rstd_row")
                nc.vector.reciprocal_approx_fast(out=rstd_row, in_=std_row)
                mu_rep = rep_pool.tile([P, SH], F32, tag="mu_rep")
                nc.tensor.matmul(mu_rep, ones_rep, mu_row.bitcast(F32R),
                                 start=True, stop=True)
                rstd_rep = rep_pool.tile([P, SH], F32, tag="rstd_rep")
                nc.tensor.matmul(rstd_rep, ones_rep, rstd_row.bitcast(F32R),
                                 start=True, stop=True)
                for kc in range(NKC):
                    t = tl_pool.tile([P, SH], F32, tag="t")
                    e1 = [nc.vector, nc.gpsimd][kc]
                    e1.tensor_sub(t, x3[:, kc, :], mu_rep)
                    # t2 = (t * gamma) * rstd_rep
                    t2 = tl_pool.tile([P, SH], F32, tag="t2")
                    e1.scalar_tensor_tensor(
                        out=t2, in0=t, scalar=gamma[:, kc:kc + 1],
                        in1=rstd_rep, op0=ALU.mult, op1=ALU.mult)
                    e2 = [nc.gpsimd, nc.vector][kc]
                    e2.tensor_scalar_add(out=out3[:, kc, :], in0=t2,
                                         scalar1=beta[:, kc:kc + 1])

            # out-projection + residual (r = qT + Wo^T attn + bo)
            for mc in range(NKC):
                ps = mm_pool.tile([P, SH], F32, tag="mm")
                for kc in range(NKC):
                    nc.tensor.matmul(
                        ps, wo_sb[:, kc, mc * P:(mc + 1) * P],
                        attn_sb[:, kc, :],
                        start=(kc == 0), stop=(kc == NKC - 1))
                # r = (ps + bo) + qT in one fused op
                e = [nc.vector, nc.gpsimd][mc]
                e.scalar_tensor_tensor(
                    out=r_sb[:, mc, :], in0=ps, scalar=bo_sb[:, mc:mc + 1],
                    in1=qT_sb[:, mc, :], op0=ALU.add, op1=ALU.add)

            layer_norm(r_sb, g1_sb, b1_sb, z_sb)

            # FFN1 + relu
            for mf in range(NFC):
                ps = mm_pool.tile([P, SH], F32, tag="mm")
                for kc in range(NKC):
                    nc.tensor.matmul(
                        ps, w1_sb[:, kc, mf * P:(mf + 1) * P],
                        z_sb[:, kc, :],
                        start=(kc == 0), stop=(kc == NKC - 1))
                if mf % 2 == 0:
                    nc.scalar.activation(
                        out=h1_sb[:, mf, :], in_=ps, func=AFT.Relu,
                        bias=bp_sb[:, 8 + mf // 2, mf % 2:mf % 2 + 1])
                else:
                    nc.vector.tensor_scalar(
                        out=h1_sb[:, mf, :], in0=ps,
                        scalar1=bp_sb[:, 8 + mf // 2, mf % 2:mf % 2 + 1],
                        scalar2=0.0,
                        op0=ALU.add, op1=ALU.max)
            # FFN2 + bias + residual
            for mc in range(NKC):
                ps = mm_pool.tile([P, SH], F32, tag="mm")
                for kf in range(NFC):
                    nc.tensor.matmul(
                        ps, w2_sb[:, kf, mc * P:(mc + 1) * P],
                        h1_sb[:, kf, :],
                        start=(kf == 0), stop=(kf == NFC - 1))
                # r = (ps + bf2) + z in one fused op
                e = [nc.vector, nc.gpsimd][mc]
                e.scalar_tensor_tensor(
                    out=r_sb[:, mc, :], in0=ps, scalar=bf2_sb[:, mc:mc + 1],
                    in1=z_sb[:, mc, :], op0=ALU.add, op1=ALU.add)

            layer_norm(r_sb, g2_sb, b2_sb, out_sb)

            out_r = out.rearrange("(kc p) s -> p kc s", p=P)
            nc.sync.dma_start(out=out_r[:, 0, :], in_=out_sb[:, 0, :])
            nc.gpsimd.dma_start(out=out_r[:, 1, :], in_=out_sb[:, 1, :])


_NC_CACHE = None


def _get_nc():
    global _NC_CACHE
    if _NC_CACHE is None:
        _NC_CACHE = build_bass()
    return _NC_CACHE


FP8_NP = ml_dtypes.float8_e4m3


def _pack_qk8(W):
    """Permute+pad Wq/Wk columns into the [g, hf, chl, chh, m] fp8 layout.

    Column m = 32*j + p' (p' < 16) of pass (g, hf) holds original column
    c' = (4g + j)*32 + hf*16 + p'; columns with p' >= 16 are zero."""
    W8 = np.asarray(W, np.float32).astype(FP8_NP)
    outp = np.zeros((2, 2, P, 2, P), FP8_NP)
    for g in range(2):
        for hf in range(2):
            for j in range(4):
                cols = (4 * g + j) * 32 + hf * 16 + np.arange(16)
                blk = W8[:, cols]                       # [C, 16]
                blk = blk.reshape(2, P, 16)             # (chh, chl, p')
                outp[g, hf, :, :, 32 * j:32 * j + 16] = \
                    blk.transpose(1, 0, 2)
    return np.ascontiguousarray(outp)


def make_in_maps(lidar_features, image_features, Wq, bq, Wk, bk, Wv, bv,
                 Wo, bo, g1, b1, W1, bf1, W2, bf2, g2, b2):
    xT_full = np.ascontiguousarray(
        np.asarray(lidar_features, np.float32).reshape(C, S))
    y_full = np.ascontiguousarray(
        np.asarray(image_features, np.float32).reshape(C, S))
    wqo = np.ascontiguousarray(np.stack([
        np.asarray(Wq, np.float32), np.asarray(Wo, np.float32)]))
    bpack = np.ascontiguousarray(np.concatenate([
        np.asarray(bq, np.float32)[None], np.asarray(bv, np.float32)[None],
        np.asarray(bo, np.float32)[None], np.asarray(bf2, np.float32)[None],
        np.asarray(g1, np.float32)[None], np.asarray(b1, np.float32)[None],
        np.asarray(g2, np.float32)[None], np.asarray(b2, np.float32)[None],
        np.asarray(bf1, np.float32).reshape(4, C)]))
    wv8 = np.asarray(Wv, np.float32).astype(FP8_NP).reshape(2, P, C)
    bvb = np.broadcast_to(np.asarray(bv, np.float32)[None, :],
                          (P, C)).copy()
    common = {
        "y8": y_full.astype(FP8_NP),
        "wqo": wqo,
        "wq8p": _pack_qk8(Wq),
        "wk8p": _pack_qk8(Wk),
        "wv8": np.ascontiguousarray(wv8),
        "w1": np.ascontiguousarray(np.asarray(W1, np.float32)),
        "w2": np.ascontiguousarray(np.asarray(W2, np.float32)),
        "ones32": np.ones((P, 1), np.float32),
        "bpack": bpack,
        "bvb": bvb,
    }
    in_maps = []
    for c in range(NCORES):
        m = dict(common)
        shard = np.ascontiguousarray(xT_full[:, c * SH:(c + 1) * SH])
        m["xT"] = shard
        m["x8"] = shard.astype(FP8_NP)
        in_maps.append(m)
    return in_maps


def kernel(lidar_features, image_features, Wq, bq, Wk, bk, Wv, bv, Wo, bo,
           g1, b1, W1, bf1, W2, bf2, g2, b2, num_heads, **run_kwargs):
    assert int(num_heads) == NH
    nc = _get_nc()
    in_maps = make_in_maps(lidar_features, image_features, Wq, bq, Wk, bk,
                           Wv, bv, Wo, bo, g1, b1, W1, bf1, W2, bf2, g2, b2)
    res = run_bass_kernel_spmd(nc, in_maps, core_ids=list(range(NCORES)),
                               **run_kwargs)
    full = np.concatenate([res.results[c]["out"] for c in range(NCORES)],
                          axis=1)
    kernel.last_results = res
    return full.reshape(1, C, 64, 64).astype(np.float32)


kernel.last_results = None


# revision 35
# speedup vs baseline: 1.9895x; 1.0258x over previous
"""Cross-attention fusion block on 8 trn2 NeuronCores.

Sharding: data-parallel over the query sequence (S=4096 -> 512 rows/core).
K/V projections are computed redundantly on every core. Channel-major
layout [C, S] throughout; no on-chip transposes.

v2 design (vs baseline): fp8 DoubleRow matmuls for the attention phase and
the K/V/Q8 projections, and the softmax exp split across ACT (true exp ->
fp8e5) / DVE / Pool (Schraudolph bit-hack exp via uint8 write + fp8e5
bitcast).  Key layout trick: Wk/Wq columns are permuted+zero-padded on the
host so the projection matmul lands K/Q directly in the [16, 2(half), ...]
partition layout DoubleRow needs (head h in grp g at partition band
32*(h%4), head-dim split 16+16 across the DoubleRow free axis).

Per-core pipeline (q = 512 query rows of this core):
  qT   = Wq^T xT + bq                  [256, 512] fp32r   (residual path)
  q8   = perm(Wq8)^T x8                [bands, 2, 512] fp8e4
  k8   = perm(Wk8)^T y8                [bands, 2, 4096] fp8e4
  v8_h = [y8^T Wv8 + bv | 1]           per head [128, 2, 33] fp8e4
  per (grp g, head j, 256-key chunk d):
    ST[k, q] = k8_h-chunk DR@ q8_h                  (2 DoubleRow matmuls)
    AT       = approx-exp(ST/sqrt(32)) -> fp8e5     (ACT exp | DVE/Pool hack)
    AVCS_h  += v8_h-chunk DR@ AT                    [33, 512] psum
  attn_h = AV_h * (1/CS_h)      (CS rows DMA-gathered, reciprocal, emat
                                 broadcast matmul, per-head psum*rec mul)
  o = Wo^T attn + bo; r = qT + o; z = LN1(r)
  h1 = relu(W1^T z + bf1); h2 = W2^T h1 + bf2; out = LN2(z + h2)
"""

import sys

for _p in ("/opt/trn_rl_repo", "/opt/pypackages"):
    if _p not in sys.path:
        sys.path.append(_p)

import numpy as np
import ml_dtypes

import concourse.bass as bass
import concourse.bacc as bacc
import concourse.tile as tile
from concourse import mybir
from concourse.bass_utils import run_bass_kernel_spmd

F32 = mybir.dt.float32
F32R = mybir.dt.float32r
FP8E4 = mybir.dt.float8e4
FP8E5 = mybir.dt.float8e5
U8 = mybir.dt.uint8
AFT = mybir.ActivationFunctionType
ALU = mybir.AluOpType
DR = mybir.MatmulPerfMode.DoubleRow

P = 128           # SBUF partitions
C = 256           # channels
S = 4096          # sequence (64*64)
NCORES = 8
SH = S // NCORES  # 512 query rows per core
NH = 8            # heads
HD = 32           # head dim
F = 4 * C         # FFN hidden = 1024
NKC = C // P      # 2 channel chunks
NFC = F // P      # 8 ffn chunks
ND = S // 256     # 16 double-row key chunks
NBLK = S // 512   # 8 key blocks for kT production
EPS = 1e-5
INV_SQRT_HD = 1.0 / float(np.sqrt(HD))
INV_C = 1.0 / C
# Schraudolph-style exp for fp8e5(=e5m2) bitcast: i = floor(A*st + B)
HACK_A = float(4.0 * np.log2(np.e)) * INV_SQRT_HD
HACK_B = 60.02


def build_bass():
    nc = bacc.Bacc()

    xT = nc.declare_dram_parameter("xT", [C, SH], F32R, isOutput=False)
    x8 = nc.declare_dram_parameter("x8", [C, SH], FP8E4, isOutput=False)
    y8 = nc.declare_dram_parameter("y8", [C, S], FP8E4, isOutput=False)
    wqo = nc.declare_dram_parameter("wqo", [2, C, C], F32R, isOutput=False)
    wq8p = nc.declare_dram_parameter("wq8p", [2, 2, P, 2, P], FP8E4,
                                     isOutput=False)
    wk8p = nc.declare_dram_parameter("wk8p", [2, 2, P, 2, P], FP8E4,
                                     isOutput=False)
    wv8 = nc.declare_dram_parameter("wv8", [2, P, C], FP8E4, isOutput=False)
    w1 = nc.declare_dram_parameter("w1", [C, F], F32R, isOutput=False)
    w2 = nc.declare_dram_parameter("w2", [F, C], F32R, isOutput=False)
    ones32 = nc.declare_dram_parameter("ones32", [P, 1], F32R, isOutput=False)
    bpack = nc.declare_dram_parameter("bpack", [12, C], F32, isOutput=False)
    bvb = nc.declare_dram_parameter("bvb", [P, C], F32, isOutput=False)
    out = nc.declare_dram_parameter("out", [C, SH], F32, isOutput=True)

    with tile.TileContext(nc) as tc:
        _emit(tc, xT, x8, y8, wqo, wq8p, wk8p, wv8, w1, w2, ones32,
              bpack, bvb, out)
    if not nc.is_finalized():
        nc.finalize()
    return nc


def _emit(tc, xT, x8, y8, wqo, wq8p, wk8p, wv8, w1, w2, ones32,
          bpack, bvb, out):
    nc = tc.nc

    import contextlib
    stack = contextlib.ExitStack()
    with stack:
        consts = stack.enter_context(tc.tile_pool(name="consts", bufs=1))
        big = stack.enter_context(tc.tile_pool(name="big", bufs=1))

        # ---------------- constants / inputs into SBUF ----------------
        y8_sb = big.tile([P, NKC, S], FP8E4)      # y8[ch, s]; ch = kc*128+p
        y8_r = y8.rearrange("(kc p) s -> p kc s", p=P)
        HS = S // 2
        for sh2 in range(2):
            eng = [nc.sync, nc.gpsimd][sh2]
            eng.dma_start(out=y8_sb[:, :, sh2 * HS:(sh2 + 1) * HS],
                          in_=y8_r[:, :, sh2 * HS:(sh2 + 1) * HS])
        xT_sb = big.tile([P, NKC, SH], F32R)
        nc.sync.dma_start(out=xT_sb, in_=xT.rearrange("(kc p) s -> p kc s",
                                                      p=P))
        x8_sb = big.tile([P, NKC, SH], FP8E4)
        nc.gpsimd.dma_start(out=x8_sb, in_=x8.rearrange("(kc p) s -> p kc s",
                                                        p=P))

        wqo_sb = consts.tile([P, 2, NKC, C], F32R)
        nc.sync.dma_start(
            out=wqo_sb, in_=wqo.rearrange("w (kc p) m -> p w kc m", p=P))
        wq_sb, wo_sb = (wqo_sb[:, i] for i in range(2))
        wq8p_sb = consts.tile([P, 2, 2, 2, P], FP8E4)
        nc.gpsimd.dma_start(
            out=wq8p_sb, in_=wq8p.rearrange("g hf p w m -> p g hf w m"))
        wk8p_sb = consts.tile([P, 2, 2, 2, P], FP8E4)
        nc.gpsimd.dma_start(
            out=wk8p_sb, in_=wk8p.rearrange("g hf p w m -> p g hf w m"))
        wv8_sb = consts.tile([P, 2, C], FP8E4)
        nc.gpsimd.dma_start(out=wv8_sb, in_=wv8.rearrange("w p m -> p w m"))

        bp_sb = consts.tile([P, 12, NKC], F32)
        nc.sync.dma_start(out=bp_sb,
                          in_=bpack.rearrange("n (kc p) -> p n kc", p=P))
        bq_sb = bp_sb[:, 0]
        bo_sb, bf2_sb = bp_sb[:, 2], bp_sb[:, 3]
        g1_sb, b1_sb, g2_sb, b2_sb = (bp_sb[:, i] for i in range(4, 8))
        bvb_sb = consts.tile([P, C], F32)
        nc.sync.dma_start(out=bvb_sb, in_=bvb[:])

        # late-needed weights issued after the attention-critical loads
        w1_sb = consts.tile([P, NKC, F], F32R)
        nc.gpsimd.dma_start(out=w1_sb,
                            in_=w1.rearrange("(kc p) m -> p kc m", p=P))
        w2_sb = consts.tile([P, NFC, C], F32R)
        nc.sync.dma_start(out=w2_sb,
                          in_=w2.rearrange("(kc p) m -> p kc m", p=P))
        ones1r = consts.tile([P, 1], F32R)      # LN-stats lhsT (f32r ones)
        nc.sync.dma_start(out=ones1r, in_=ones32[:])
        ones_rep = consts.tile([1, P], F32R)    # K=1 row-replication lhsT
        nc.vector.memset(ones_rep, 1.0)

        ones132 = consts.tile([1, HD], F32R)    # rec band-broadcast lhsT
        nc.vector.memset(ones132, 1.0)
        eps_sb = consts.tile([P, 1], F32)
        nc.vector.memset(eps_sb, EPS)

        # persistent activations
        qT_sb = big.tile([P, NKC, SH], F32R)       # q^T (with bq), residual
        q8_sb = big.tile([P, 2, 2, SH], FP8E4)     # (band, g, half, q)
        k8_sb = big.tile([P, 2, 2, S], FP8E4)      # (band, g, half, s)
        v8_sb = big.tile([P, ND, 2, NH, HD + 1], FP8E4)  # (k, d, i, h, c|1)
        attn_sb = big.tile([P, NKC, SH], F32R)     # (attn@v)/cs + bv
        z_sb = big.tile([P, NKC, SH], F32R)        # LN1 output
        h1_sb = big.tile([P, NFC, SH], F32R)       # relu(ffn1)
        out_sb = big.tile([P, NKC, SH], F32)       # final
        r_sb = big.tile([P, NKC, SH], F32R)        # residual sums (LN inputs)

        # ones column of v8 (CS accumulator rows)
        nc.vector.memset(v8_sb[:, :, :, :, HD:HD + 1], 1.0)

        # ---------------- preamble: projections ----------------
        with tc.tile_pool(name="pre_k", bufs=2, space="PSUM") as pre_k, \
             tc.tile_pool(name="pre_v", bufs=3, space="PSUM") as pre_v:
            # q^T fp32 (residual): q[c',q] = sum_ch Wq[ch,c'] xT[ch,q]
            psq = pre_k.tile([P, 2, SH], F32, tag="ps")
            for mc in range(NKC):
                for kc in range(NKC):
                    nc.tensor.matmul(
                        psq[:, mc, :], wq_sb[:, kc, mc * P:(mc + 1) * P],
                        xT_sb[:, kc, :],
                        start=(kc == 0), stop=(kc == NKC - 1))
            for mc in range(NKC):
                nc.scalar.activation(out=qT_sb[:, mc, :], in_=psq[:, mc, :],
                                     func=AFT.Identity,
                                     bias=bq_sb[:, mc:mc + 1])
            def emit_q8(g):
                ps8 = pre_k.tile([P, 2, SH], F32, tag="ps")
                for hf in range(2):
                    nc.tensor.matmul(ps8[:, hf, :], wq8p_sb[:, g, hf],
                                     x8_sb, start=True, stop=True,
                                     perf_mode=DR)
                [nc.vector, nc.gpsimd][g].tensor_copy(q8_sb[:, g], ps8)

            def emit_k8(g, blk):
                psk = pre_k.tile([P, 2, SH], F32, tag="ps")
                for hf in range(2):
                    nc.tensor.matmul(
                        psk[:, hf, :], wk8p_sb[:, g, hf],
                        y8_sb[:, :, blk * 512:(blk + 1) * 512],
                        start=True, stop=True, perf_mode=DR)
                ke = [nc.scalar, nc.vector, nc.gpsimd][(g * NBLK + blk) % 3]
                if ke is nc.scalar:
                    nc.scalar.activation(
                        out=k8_sb[:, g, :, blk * 512:(blk + 1) * 512],
                        in_=psk, func=AFT.Copy)
                else:
                    ke.tensor_copy(
                        k8_sb[:, g, :, blk * 512:(blk + 1) * 512], psk)

            bvb2 = bvb_sb.rearrange("p (i h c) -> p i h c", i=1, c=HD)
            bvb2 = bass.AP(tensor=bvb2.tensor, offset=bvb2.offset,
                           ap=[bvb2.ap[0], [0, 2]] + bvb2.ap[2:])

            def emit_v8(d):
                # two 128-key DoubleRow matmuls into one psum bank, then one
                # fused convert+bias
                psv = pre_v.tile([P, 2, C], F32, tag="psv")
                for i in range(2):
                    ck = 2 * d + i
                    nc.tensor.matmul(psv[:, i, :],
                                     y8_sb[:, :, ck * P:(ck + 1) * P],
                                     wv8_sb, start=True, stop=True,
                                     perf_mode=DR)
                e = nc.vector if d % 2 == 0 else nc.gpsimd
                e.tensor_add(
                    v8_sb[:, d, :, :, 0:HD],
                    psv.rearrange("p i (h c) -> p i h c", c=HD),
                    bvb2)

            # attention-critical first: q8/k8 of group 0, early v8 chunks
            emit_q8(0)
            for blk in range(NBLK):
                emit_k8(0, blk)
                if blk % 2 == 1:
                    emit_v8(blk // 2)
            emit_q8(1)
            for blk in range(NBLK):
                emit_k8(1, blk)
                emit_v8(4 + blk)
            for d in range(12, ND):
                emit_v8(d)

        # ---------------- attention ----------------
        # weighted round-robin of the exp half-ops across ACT / Pool / DVE
        exp_w = [(nc.scalar, 0.82), (nc.gpsimd, 1.08), (nc.vector, 0.66)]
        credits = [0.0, 0.0, 0.0]
        rr = []
        for _ in range(4 * ND * 2 * 2):
            for ii in range(3):
                credits[ii] += exp_w[ii][1]
            pick = max(range(3), key=lambda ii: credits[ii])
            credits[pick] -= sum(w for _, w in exp_w)
            rr.append(exp_w[pick][0])

        with tc.tile_pool(name="st", bufs=6, space="PSUM") as st_pool, \
             tc.tile_pool(name="avcs", bufs=1, space="PSUM") as avcs_pool, \
             tc.tile_pool(name="at", bufs=8) as at_pool, \
             tc.tile_pool(name="nrm", bufs=1) as nrm_pool:

            def make_normalize(hg, g, jp, avcs):
                # attn_h = av_h / cs_h  (bv already folded into v8)
                def _norm():
                    rec_row = nrm_pool.tile([1, 2, SH], F32, tag="rec_row",
                                            name=f"rec_row_{hg}")
                    nc.vector.reciprocal_approx_fast(
                        out=rec_row, in_=avcs[HD:HD + 1, :, :])
                    rec_all = st_pool.tile([P, SH], F32, tag="st",
                                           name=f"rec_all_{hg}")
                    rr_r = rec_row.bitcast(F32R)
                    for jj in range(2):
                        j = 2 * jp + jj
                        nc.tensor.matmul(rec_all[32 * j:32 * (j + 1), :],
                                         ones132, rr_r[:, jj, :],
                                         start=True, stop=True,
                                         tile_position=(0, 32 * j))
                    for jj in range(2):
                        j = 2 * jp + jj
                        e = [nc.vector, nc.gpsimd][(hg + jj) % 2]
                        e.tensor_mul(attn_sb[32 * j:32 * (j + 1), g, :],
                                     avcs[0:HD, jj, :],
                                     rec_all[32 * j:32 * (j + 1), :])
                return _norm

            uu = 0
            pending_norm = None
            for hg in range(4):      # half-groups: 2 heads x 16 d-chunks
                g, jp = hg // 2, hg % 2
                avcs = avcs_pool.tile([HD + 1, 2, SH], F32, tag="avcs",
                                      name=f"avcs_{hg}")
                pend = []
                at = None
                for d in range(ND):
                    for jj in range(2):
                        j = 2 * jp + jj
                        for i in range(2):
                            st = st_pool.tile([P, SH], F32, tag="st")
                            nc.tensor.matmul(
                                st,
                                k8_sb[32 * j:32 * j + 16, g, :,
                                      256 * d + 128 * i:256 * d + 128 * i + 128],
                                q8_sb[32 * j:32 * j + 16, g],
                                start=True, stop=True, perf_mode=DR,
                                tile_position=(32 * j, 0))
                            if i == 0:
                                at = at_pool.tile([P, 2, SH], FP8E5,
                                                  tag="at")
                            e = rr[uu]
                            uu += 1
                            if e is nc.scalar:
                                nc.scalar.activation(out=at[:, i, :], in_=st,
                                                     func=AFT.Exp,
                                                     scale=INV_SQRT_HD)
                            else:
                                e.tensor_scalar(out=at[:, i, :].bitcast(U8),
                                                in0=st,
                                                scalar1=HACK_A,
                                                scalar2=HACK_B,
                                                op0=ALU.mult, op1=ALU.add)
                        pend.append((d, jj, at))
                        if len(pend) > 3:
                            pd, pjj, pat = pend.pop(0)
                            nc.tensor.matmul(
                                avcs[:, pjj, :],
                                v8_sb[:, pd, :, 4 * g + 2 * jp + pjj, :],
                                pat, start=(pd == 0), stop=(pd == ND - 1),
                                perf_mode=DR)
                        if d == 1 and jj == 1 and pending_norm is not None:
                            pending_norm()
                            pending_norm = None
                for pd, pjj, pat in pend:
                    nc.tensor.matmul(
                        avcs[:, pjj, :],
                        v8_sb[:, pd, :, 4 * g + 2 * jp + pjj, :],
                        pat, start=(pd == 0), stop=(pd == ND - 1),
                        perf_mode=DR)
                pending_norm = make_normalize(hg, g, jp, avcs)
            pending_norm()

        # ---------------- tail: out-proj, LN1, FFN, LN2 ----------------
        with tc.tile_pool(name="mm", bufs=3, space="PSUM") as mm_pool, \
             tc.tile_pool(name="stat", bufs=1, space="PSUM") as stat_pool, \
             tc.tile_pool(name="rep", bufs=1, space="PSUM") as rep_pool, \
             tc.tile_pool(name="tl", bufs=2) as tl_pool, \
             tc.tile_pool(name="tr", bufs=1) as tr_pool:

            # dummy op to pull the Sqrt act-table load off the LN critical
            # chain (executes while ACT is otherwise idle)
            warm = tr_pool.tile([1, 1], F32, tag="warm")
            nc.scalar.activation(out=warm, in_=eps_sb[:1, :], func=AFT.Sqrt)

            def layer_norm(x3, gamma, beta, out3):
                """out3 = LN(x3) over the channel axis (2 chunks of 128)."""
                mu_ps = stat_pool.tile([1, SH], F32, tag="mu")
                e2_ps = stat_pool.tile([1, SH], F32, tag="e2")
                for kc in range(NKC):
                    nc.tensor.matmul(mu_ps, ones1r, x3[:, kc, :],
                                     start=(kc == 0), stop=(kc == NKC - 1))
                for kc in range(NKC):
                    sq = tl_pool.tile([P, SH], F32R, tag="sq")
                    e = [nc.vector, nc.gpsimd][kc]
                    e.tensor_mul(sq, x3[:, kc, :], x3[:, kc, :])
                    nc.tensor.matmul(e2_ps, ones1r, sq,
                                     start=(kc == 0), stop=(kc == NKC - 1))
                # mu2 = (mu_ps/C)^2 in one fused op (reads psum twice)
                mu2_row = tr_pool.tile([1, SH], F32, tag="mu2_row")
                nc.vector.scalar_tensor_tensor(
                    out=mu2_row, in0=mu_ps, scalar=INV_C * INV_C, in1=mu_ps,
                    op0=ALU.mult, op1=ALU.mult)
                mu_row = tr_pool.tile([1, SH], F32, tag="mu_row")
                nc.gpsimd.tensor_scalar_mul(out=mu_row, in0=mu_ps,
                                            scalar1=INV_C)
                var_row = tr_pool.tile([1, SH], F32, tag="var_row")
                # var = E[x^2] - mu^2 = e2/C - mu^2
                nc.vector.scalar_tensor_tensor(
                    out=var_row, in0=e2_ps, scalar=INV_C, in1=mu2_row,
                    op0=ALU.mult, op1=ALU.subtract)
                std_row = tr_pool.tile([1, SH], F32, tag="std_row")
                nc.scalar.activation(out=std_row, in_=var_row, func=AFT.Sqrt,
                                     bias=eps_sb[:1, :])
                rstd_row = tr_pool.tile([1, SH], F32, tag="rstd_row")
                nc.vector.reciprocal_approx_fast(out=rstd_row, in_=std_row)
                mu_rep = rep_pool.tile([P, SH], F32, tag="mu_rep")
                nc.tensor.matmul(mu_rep, ones_rep, mu_row.bitcast(F32R),
                                 start=True, stop=True)
                rstd_rep = rep_pool.tile([P, SH], F32, tag="rstd_rep")
                nc.tensor.matmul(rstd_rep, ones_rep, rstd_row.bitcast(F32R),
                                 start=True, stop=True)
                for kc in range(NKC):
                    t = tl_pool.tile([P, SH], F32, tag="t")
                    e1 = [nc.vector, nc.gpsimd][kc]
                    e1.tensor_sub(t, x3[:, kc, :], mu_rep)
                    # t2 = (t * gamma) * rstd_rep
                    t2 = tl_pool.tile([P, SH], F32, tag="t2")
                    e1.scalar_tensor_tensor(
                        out=t2, in0=t, scalar=gamma[:, kc:kc + 1],
                        in1=rstd_rep, op0=ALU.mult, op1=ALU.mult)
                    e2 = [nc.gpsimd, nc.vector][kc]
                    e2.tensor_scalar_add(out=out3[:, kc, :], in0=t2,
                                         scalar1=beta[:, kc:kc + 1])

            # out-projection + residual (r = qT + Wo^T attn + bo)
            for mc in range(NKC):
                ps = mm_pool.tile([P, SH], F32, tag="mm")
                for kc in range(NKC):
                    nc.tensor.matmul(
                        ps, wo_sb[:, kc, mc * P:(mc + 1) * P],
                        attn_sb[:, kc, :],
                        start=(kc == 0), stop=(kc == NKC - 1))
                # r = (ps + bo) + qT in one fused op
                e = [nc.vector, nc.gpsimd][mc]
                e.scalar_tensor_tensor(
                    out=r_sb[:, mc, :], in0=ps, scalar=bo_sb[:, mc:mc + 1],
                    in1=qT_sb[:, mc, :], op0=ALU.add, op1=ALU.add)

            layer_norm(r_sb, g1_sb, b1_sb, z_sb)

            # FFN1 + relu
            for mf in range(NFC):
                ps = mm_pool.tile([P, SH], F32, tag="mm")
                for kc in range(NKC):
                    nc.tensor.matmul(
                        ps, w1_sb[:, kc, mf * P:(mf + 1) * P],
                        z_sb[:, kc, :],
                        start=(kc == 0), stop=(kc == NKC - 1))
                if mf % 2 == 0:
                    nc.scalar.activation(
                        out=h1_sb[:, mf, :], in_=ps, func=AFT.Relu,
                        bias=bp_sb[:, 8 + mf // 2, mf % 2:mf % 2 + 1])
                else:
                    nc.vector.tensor_scalar(
                        out=h1_sb[:, mf, :], in0=ps,
                        scalar1=bp_sb[:, 8 + mf // 2, mf % 2:mf % 2 + 1],
                        scalar2=0.0,
                        op0=ALU.add, op1=ALU.max)
            # FFN2 + bias + residual
            for mc in range(NKC):
                ps = mm_pool.tile([P, SH], F32, tag="mm")
                for kf in range(NFC):
                    nc.tensor.matmul(
                        ps, w2_sb[:, kf, mc * P:(mc + 1) * P],
                        h1_sb[:, kf, :],
                        start=(kf == 0), stop=(kf == NFC - 1))
                # r = (ps + bf2) + z in one fused op
                e = [nc.vector, nc.gpsimd][mc]
                e.scalar_tensor_tensor(
                    out=r_sb[:, mc, :], in0=ps, scalar=bf2_sb[:, mc:mc + 1],
                    in1=z_sb[:, mc, :], op0=ALU.add, op1=ALU.add)

            layer_norm(r_sb, g2_sb, b2_sb, out_sb)

            out_r = out.rearrange("(kc p) s -> p kc s", p=P)
            nc.sync.dma_start(out=out_r[:, 0, :], in_=out_sb[:, 0, :])
            nc.gpsimd.dma_start(out=out_r[:, 1, :], in_=out_sb[:, 1, :])


_NC_CACHE = None


def _get_nc():
    global _NC_CACHE
    if _NC_CACHE is None:
        _NC_CACHE = build_bass()
    return _NC_CACHE


FP8_NP = ml_dtypes.float8_e4m3


def _pack_qk8(W):
    """Permute+pad Wq/Wk columns into the [g, hf, chl, chh, m] fp8 layout.

    Column m = 32*j + p' (p' < 16) of pass (g, hf) holds original column
    c' = (4g + j)*32 + hf*16 + p'; columns with p' >= 16 are zero."""
    W8 = np.asarray(W, np.float32).astype(FP8_NP)
    outp = np.zeros((2, 2, P, 2, P), FP8_NP)
    for g in range(2):
        for hf in range(2):
            for j in range(4):
                cols = (4 * g + j) * 32 + hf * 16 + np.arange(16)
                blk = W8[:, cols]                       # [C, 16]
                blk = blk.reshape(2, P, 16)             # (chh, chl, p')
                outp[g, hf, :, :, 32 * j:32 * j + 16] = \
                    blk.transpose(1, 0, 2)
    return np.ascontiguousarray(outp)


def make_in_maps(lidar_features, image_features, Wq, bq, Wk, bk, Wv, bv,
                 Wo, bo, g1, b1, W1, bf1, W2, bf2, g2, b2):
    xT_full = np.ascontiguousarray(
        np.asarray(lidar_features, np.float32).reshape(C, S))
    y_full = np.ascontiguousarray(
        np.asarray(image_features, np.float32).reshape(C, S))
    wqo = np.ascontiguousarray(np.stack([
        np.asarray(Wq, np.float32), np.asarray(Wo, np.float32)]))
    bpack = np.ascontiguousarray(np.concatenate([
        np.asarray(bq, np.float32)[None], np.asarray(bv, np.float32)[None],
        np.asarray(bo, np.float32)[None], np.asarray(bf2, np.float32)[None],
        np.asarray(g1, np.float32)[None], np.asarray(b1, np.float32)[None],
        np.asarray(g2, np.float32)[None], np.asarray(b2, np.float32)[None],
        np.asarray(bf1, np.float32).reshape(4, C)]))
    wv8 = np.asarray(Wv, np.float32).astype(FP8_NP).reshape(2, P, C)
    bvb = np.broadcast_to(np.asarray(bv, np.float32)[None, :],
                          (P, C)).copy()
    common = {
        "y8": y_full.astype(FP8_NP),
        "wqo": wqo,
        "wq8p": _pack_qk8(Wq),
        "wk8p": _pack_qk8(Wk),
        "wv8": np.ascontiguousarray(wv8),
        "w1": np.ascontiguousarray(np.asarray(W1, np.float32)),
        "w2": np.ascontiguousarray(np.asarray(W2, np.float32)),
        "ones32": np.ones((P, 1), np.float32),
        "bpack": bpack,
        "bvb": bvb,
    }
    in_maps = []
    for c in range(NCORES):
        m = dict(common)
        shard = np.ascontiguousarray(xT_full[:, c * SH:(c + 1) * SH])
        m["xT"] = shard
        m["x8"] = shard.astype(FP8_NP)
        in_maps.append(m)
    return in_maps


def kernel(lidar_features, image_features, Wq, bq, Wk, bk, Wv, bv, Wo, bo,
           g1, b1, W1, bf1, W2, bf2, g2, b2, num_heads, **run_kwargs):
    assert int(num_heads) == NH
    nc = _get_nc()
    in_maps = make_in_maps(lidar_features, image_features, Wq, bq, Wk, bk,
                           Wv, bv, Wo, bo, g1, b1, W1, bf1, W2, bf2, g2, b2)
    res = run_bass_kernel_spmd(nc, in_maps, core_ids=list(range(NCORES)),
                               **run_kwargs)
    full = np.concatenate([res.results[c]["out"] for c in range(NCORES)],
                          axis=1)
    kernel.last_results = res
    return full.reshape(1, C, 64, 64).astype(np.float32)


kernel.last_results = None
